# revision 36
# baseline (speedup 1.0000x reference)
"""Trainium2 Bass kernel for nn_Cace_74569222193773 (CACE GNN message passing).

Strategy (8 NeuronCores, SPMD, one program shape + per-core data):
  * Host: drop edges with r >= cutoff (fcut = 0 there), assign nodes to 64
    edge-balanced global windows of <=32 nodes (8 "own" windows per core).
  * HALO REPLICATION instead of a collective: each core additionally
    recomputes pass-1 A for the sender nodes of its own edges that live on
    other cores.  Those halo nodes are repacked into private halo windows
    (<=32 nodes, <=256 in-edges each, edge-balanced); the core processes
    own + halo windows in pass 1, writes the node table T = [A row | chi]
    (fp16) to its own DRAM, and pass 2 gathers sender rows locally.
    No inter-core communication at all.
  * All node-feature tensors live in a "half" layout: partition p = x*32+n
    with x = (r or b) mod 4, plus a half index g = (r or b) // 4 in the
    free dimension, so every PE matmul output starts at a 32-aligned
    partition base.
  * Pass 1 (per core): edge geometry + bessel + cutoff + angular on
    DVE/Pool/ACT in edge-major layout [128 partitions = edges]; per-window
    segment-sum via PE matmuls (fp16 operands, fp32 PSUM) with
    lhsT = onehot32 (x) radf-half, rhs = P = ang (x) enc; radial transform
    via block-diag W (x) I32 fp16 matmuls; symmetrize + chi per window
    group (own windows in fp32 A, halo windows from the fp16 copy -- halo
    B0 only feeds chi).
  * Pass 2 (own windows only): indirect-DMA gather of T[send] (fp16 rows),
    A_ar via per-b matmuls (lhsT = onehot (x) fr slice), A_bchi via the
    pass-1 segment-sum machinery with rhs P * chi_send, mem via
    W_mem (x) I32; combine (fp32), symmetrize -> B1.

kernel() takes FULL unsharded inputs and returns the FULL [2000,8,5,9,2]
float32 output; all sharding happens inside.
"""
import heapq
from math import factorial

import numpy as np

# ---- static problem config (mirrors the reference) ----
MAX_L = 3; N_RBF = 8; RB = 8; K = 3
CUTOFF = 5.5
N_NODES = 2000
MP_NORM = 1.0 / np.sqrt(25.0)
C = K * K                      # 9
NB = 1 + (MAX_L + 1)           # 5

def _lxlylz(max_l):
    out = []
    for l in range(max_l + 1):
        for lx in range(l, -1, -1):
            for ly in range(l - lx, -1, -1):
                out.append((lx, ly, l - lx - ly))
    return out

L_LIST = _lxlylz(MAX_L); NL = len(L_LIST)                       # 20
LX = np.array([t[0] for t in L_LIST]); LY = np.array([t[1] for t in L_LIST])
LZ = np.array([t[2] for t in L_LIST]); DEGS = LX + LY + LZ
MULTI = np.array([factorial(int(d)) / (factorial(int(a)) * factorial(int(b)) * factorial(int(c)))
                  for a, b, c, d in zip(LX, LY, LZ, DEGS)], dtype=np.float32)
GRP_SLICES = []                 # (l_start, l_count) per degree; DEGS is sorted
for d in range(MAX_L + 1):
    idx = np.where(DEGS == d)[0]
    GRP_SLICES.append((int(idx[0]), int(len(idx))))

# ---- sharding geometry ----
N_CORES = 8
WIN = 32                        # nodes per window
NWINC = 8                       # own windows per core
NWIN = N_CORES * NWINC          # 64
NSLOT = NWIN * WIN              # 2048 own-node slots globally
EBLK = 128                      # edges per block (partition dim)
NBW = 2                         # blocks per window
NBLK_OWN = NWINC * NBW          # 16 own blocks per core
LC = NL * C                     # 180
GLC = 2 * LC                    # 360 = both halves
TW = RB * LC + WIN              # table row width 1472 (1440 A + 9 chi + pad)

_RSCL = np.sqrt(2.0 / CUTOFF)

F32_FIELDS = ['exyz_s', 'exyz_r', 'eemb_s', 'eemb_r', 'enloc',
              'iota32', 'multi_l', 'war_mp', 'nvec']
FP16_FIELDS = ['wbd_rad', 'wbd_radmp', 'wbd_mem', 'wbd_chi']


def _field_layout(nbt):
    """Column layout of the packed f32 / fp16 input tensors for nbt blocks."""
    fw = dict(exyz_s=3 * nbt, exyz_r=3 * nbt, eemb_s=3 * nbt, eemb_r=3 * nbt,
              enloc=nbt, iota32=WIN, multi_l=NL, war_mp=64, nvec=N_RBF,
              wbd_rad=2048, wbd_radmp=2048, wbd_mem=2048, wbd_chi=2 * NB * WIN)
    off = {}
    o = 0
    for f in F32_FIELDS:
        off[f] = o; o += fw[f]
    totf = o
    o = 0
    for f in FP16_FIELDS:
        off[f] = o; o += fw[f]
    return fw, off, totf, o


def _pack_windows(node_list, deg, nwin_cap):
    """Balanced assignment of node_list into windows (<=WIN nodes each,
    edge-load balanced).  Grows window count until max load <= NBW*EBLK.
    Returns (win_of, pos_of, n_windows)."""
    nodes = sorted(node_list, key=lambda n: -deg[n])
    nwin = max(1, (len(nodes) + WIN - 1) // WIN)
    while True:
        win_cnt = np.zeros(nwin, np.int64); win_load = np.zeros(nwin, np.int64)
        win_of = {}; pos_of = {}
        heap = [(0, w) for w in range(nwin)]
        heapq.heapify(heap)
        ok = True
        for nd in nodes:
            popped = []
            while True:
                load, w = heapq.heappop(heap)
                if win_cnt[w] < WIN:
                    break
                popped.append((load, w))
            for it in popped:
                heapq.heappush(heap, it)
            win_of[nd] = w; pos_of[nd] = int(win_cnt[w])
            win_cnt[w] += 1; win_load[w] += deg[nd]
            heapq.heappush(heap, (int(win_load[w]), w))
        if win_load.max(initial=0) <= NBW * EBLK:
            return win_of, pos_of, nwin
        nwin += 1
        if nwin > nwin_cap:
            raise RuntimeError("halo window packing overflow")


def _host_prep(inputs):
    pos = np.asarray(inputs['positions'], np.float32)
    shifts = np.asarray(inputs['shifts'], np.float32)
    W_embed = np.asarray(inputs['W_embed'], np.float32)
    species = np.asarray(inputs['species'])
    ei = np.asarray(inputs['edge_index'])
    send, recv = ei[0], ei[1]

    vec = (pos[recv] + shifts - pos[send]).astype(np.float64)
    r = np.sqrt((vec * vec).sum(-1))
    keep = np.where(r < CUTOFF)[0]
    deg = np.bincount(recv[keep], minlength=N_NODES)

    # balanced node->global-window assignment (own windows)
    order = np.argsort(-deg, kind='stable')
    win_cnt = np.zeros(NWIN, np.int64); win_load = np.zeros(NWIN, np.int64)
    win_of_node = np.zeros(N_NODES, np.int64); pos_in_win = np.zeros(N_NODES, np.int64)
    heap = [(0, w) for w in range(NWIN)]
    heapq.heapify(heap)
    for nd in order:
        popped = []
        while True:
            load, w = heapq.heappop(heap)
            if win_cnt[w] < WIN:
                break
            popped.append((load, w))
        for it in popped:
            heapq.heappush(heap, it)
        win_of_node[nd] = w; pos_in_win[nd] = win_cnt[w]
        win_cnt[w] += 1; win_load[w] += deg[nd]
        heapq.heappush(heap, (win_load[w], w))
    if win_load.max() > NBW * EBLK:
        raise RuntimeError(f"window overflow: {win_load.max()} > {NBW * EBLK}")

    slot_of_node = win_of_node * WIN + pos_in_win
    emb = W_embed[species]                       # [N, K]

    ks, kr = send[keep], recv[keep]
    in_edges = [[] for _ in range(N_NODES)]      # node -> kept edge ids
    for i, e in enumerate(keep):
        in_edges[kr[i]].append(e)

    # per-core halo structure
    core_halo = []
    nh_list = []
    for ci in range(N_CORES):
        own_w = set(range(ci * NWINC, (ci + 1) * NWINC))
        own_eids = []
        for w in sorted(own_w):
            for nd in np.where(win_of_node == w)[0]:
                own_eids.extend(in_edges[nd])
        senders = set(send[own_eids].tolist()) if own_eids else set()
        halo = [s for s in senders if win_of_node[s] not in own_w]
        hwin_of, hpos_of, nh = _pack_windows(halo, deg, 64)
        core_halo.append((own_w, hwin_of, hpos_of, nh))
        nh_list.append(nh)
    NH = max(nh_list)
    NH = ((NH + 3) // 4) * 4      # pad so WT = 8 + NH is a multiple of 4
    WT = NWINC + NH
    NBT = NBW * WT
    EPAD = NBT * EBLK

    cores = []
    for ci in range(N_CORES):
        own_w, hwin_of, hpos_of, nh = core_halo[ci]
        e_xyz_s = np.zeros((EPAD, 3), np.float32)
        e_xyz_r = np.zeros((EPAD, 3), np.float32)
        e_emb_s = np.zeros((EPAD, K), np.float32)
        e_emb_r = np.zeros((EPAD, K), np.float32)
        e_nloc = np.full((EPAD,), -1.0, np.float32)
        e_srow = np.zeros((EPAD,), np.int32)
        e_xyz_r[:, 0] = 1.0                      # pads: r = 1, finite math

        def srow_of(s):
            w = win_of_node[s]
            if w in own_w:
                return (w - ci * NWINC) * WIN + pos_in_win[s]
            return (NWINC + hwin_of[s]) * WIN + hpos_of[s]

        # local window wl in [0, WT): own first, then halo
        def fill_window(wl, node_ids, pos_of, need_srow):
            base = wl * NBW * EBLK
            eids = []
            for nd in node_ids:
                eids.extend(in_edges[nd])
            eids = np.array(eids, dtype=np.int64)
            cnt = len(eids)
            if cnt == 0:
                return
            if cnt > NBW * EBLK:
                raise RuntimeError("window edge overflow")
            sl = slice(base, base + cnt)
            e_xyz_s[sl] = pos[send[eids]]
            e_xyz_r[sl] = pos[recv[eids]] + shifts[eids]
            e_emb_s[sl] = emb[send[eids]]
            e_emb_r[sl] = emb[recv[eids]]
            e_nloc[sl] = np.array([pos_of[n] for n in recv[eids]], np.float32)
            if need_srow:
                e_srow[sl] = np.array([srow_of(s) for s in send[eids]], np.int32)

        for wl in range(NWINC):
            w = ci * NWINC + wl
            nds = np.where(win_of_node == w)[0]
            fill_window(wl, nds, {int(n): int(pos_in_win[n]) for n in nds}, True)
        halo_by_win = [[] for _ in range(nh)]
        for s, hw in hwin_of.items():
            halo_by_win[hw].append(s)
        for hw in range(nh):
            fill_window(NWINC + hw, halo_by_win[hw],
                        {int(n): int(hpos_of[n]) for n in halo_by_win[hw]}, False)

        def dev(x):
            if x.ndim == 1:
                return np.ascontiguousarray(x.reshape(NBT, EBLK).T)
            return np.ascontiguousarray(np.transpose(x.reshape(NBT, EBLK, -1), (1, 0, 2)))

        def axmajor(x3):
            d = dev(x3)                                  # [128, NBT, 3]
            return np.ascontiguousarray(np.transpose(d, (0, 2, 1)).reshape(EBLK, 3 * NBT))

        cores.append(dict(
            exyz_s=axmajor(e_xyz_s), exyz_r=axmajor(e_xyz_r),
            eemb_s=axmajor(e_emb_s), eemb_r=axmajor(e_emb_r),
            enloc=np.ascontiguousarray(dev(e_nloc)),
            esrow=np.ascontiguousarray(dev(e_srow)[:, :NBLK_OWN]),
        ))

    Wr = np.asarray(inputs['W_radial'], np.float32)   # [4(deg), 8(r), 8(b)]
    Wm = np.asarray(inputs['W_mem'], np.float32)
    Wc = np.asarray(inputs['W_chi'], np.float32)      # [8(b), 5(k)]
    Wa = np.asarray(inputs['W_ar'], np.float32)       # [8(r), 8(b)]
    I32 = np.eye(WIN, dtype=np.float32)

    def bd(W):
        cols = []
        for gout in range(2):
            for d in range(4):
                for gin in range(2):
                    cols.append(np.kron(W[d, gin * 4:gin * 4 + 4, gout * 4:gout * 4 + 4], I32))
        return np.concatenate(cols, axis=1)          # [128, 2048]

    wchi_cols = []
    for g in range(2):
        for k in range(NB):
            wchi_cols.append(np.kron(Wc[g * 4:g * 4 + 4, k:k + 1], I32))   # [128, 32]
    consts_f = dict(
        war_mp=np.tile((Wa * MP_NORM).reshape(1, 64), (EBLK, 1)),
        multi_l=np.tile(np.sqrt(MULTI).reshape(1, NL), (EBLK, 1)),
        iota32=np.tile(np.arange(WIN, dtype=np.float32).reshape(1, WIN), (EBLK, 1)),
        nvec=np.tile((np.arange(1, N_RBF + 1, dtype=np.float32) / CUTOFF).reshape(1, N_RBF),
                     (EBLK, 1)),
    )
    consts_b = dict(
        wbd_rad=bd(Wr),
        wbd_radmp=bd(Wr * MP_NORM),
        wbd_mem=bd(Wm),
        wbd_chi=np.concatenate(wchi_cols, axis=1),                   # [128, 320]
    )
    packed = []
    for ci in range(N_CORES):
        cols_f = [cores[ci][nm] for nm in
                  ['exyz_s', 'exyz_r', 'eemb_s', 'eemb_r', 'enloc']]
        cols_f += [consts_f[nm] for nm in ['iota32', 'multi_l', 'war_mp', 'nvec']]
        edf = np.ascontiguousarray(np.concatenate(cols_f, axis=1), np.float32)
        edb = np.ascontiguousarray(
            np.concatenate([consts_b[nm] for nm in FP16_FIELDS], axis=1)
        ).astype(np.float16)
        packed.append(dict(edf=edf, edb=edb, esrow=cores[ci]['esrow']))
    return packed, slot_of_node, NH


def _build_program(nh, debug=False):
    import concourse.bass as bass
    import concourse.mybir as mybir
    from concourse import bacc
    from concourse.tile import TileContext

    F32 = mybir.dt.float32
    FP16 = mybir.dt.float16
    AF = mybir.ActivationFunctionType
    OP = mybir.AluOpType

    WT = NWINC + nh
    NBT = NBW * WT
    FIELD_W, FIELD_OFF, TOTW_F, TOTW_B = _field_layout(NBT)

    nc = bacc.Bacc("TRN2", target_bir_lowering=False, debug=False,
                   num_devices=N_CORES)

    edf_d = nc.dram_tensor('edf', [EBLK, TOTW_F], F32, kind="ExternalInput")
    edb_d = nc.dram_tensor('edb', [EBLK, TOTW_B], FP16, kind="ExternalInput")
    esrow_d = nc.dram_tensor('esrow', [EBLK, NBLK_OWN], mybir.dt.int32,
                             kind="ExternalInput")
    outB = nc.dram_tensor('outB', [EBLK, 2 * NWINC * 2 * NB * C], F32,
                          kind="ExternalOutput")

    with TileContext(nc) as tc:
        with (tc.tile_pool(name="const", bufs=1) as cp,
              tc.tile_pool(name="work", bufs=2) as wp,
              tc.tile_pool(name="gat", bufs=9) as gp,
              tc.tile_pool(name="psum", bufs=2, space="PSUM") as pp,
              tc.tile_pool(name="dram", bufs=1, space="DRAM") as dp):

            bigf = cp.tile([EBLK, TOTW_F], F32, name='bigf', tag='bigf')
            nc.sync.dma_start(out=bigf[:], in_=edf_d[:])
            bigb = cp.tile([EBLK, TOTW_B], FP16, name='bigb', tag='bigb')
            nc.sync.dma_start(out=bigb[:], in_=edb_d[:])
            esrow_s = cp.tile([EBLK, NBLK_OWN], mybir.dt.int32,
                              name='esrow_s', tag='esrow_s')
            nc.sync.dma_start(out=esrow_s[:], in_=esrow_d[:])

            class _S:
                def __init__(self, tile):
                    self.tile = tile
                def __getitem__(self, nm):
                    off = FIELD_OFF[nm]
                    return self.tile[:, off:off + FIELD_W[nm]]
            s = _S(bigf)
            sb = _S(bigb)

            def ctile(tag, shape, dtype=F32):
                return cp.tile(shape, dtype, name=tag, tag=tag)

            TT = nc.vector.tensor_tensor
            TTP = nc.gpsimd.tensor_tensor
            TS = nc.vector.tensor_scalar

            # ---- geometry, edge-major [128, a*NBT+blk] ----
            vd = ctile('vd', [EBLK, 3 * NBT])
            TT(out=vd[:], in0=s['exyz_r'][:], in1=s['exyz_s'][:], op=OP.subtract)
            sq = ctile('sq', [EBLK, 3 * NBT])
            TT(out=sq[:], in0=vd[:], in1=vd[:], op=OP.mult)
            r2 = ctile('r2', [EBLK, NBT])
            TT(out=r2[:], in0=sq[:, 0:NBT], in1=sq[:, NBT:2 * NBT], op=OP.add)
            TT(out=r2[:], in0=r2[:], in1=sq[:, 2 * NBT:3 * NBT], op=OP.add)
            rr = ctile('rr', [EBLK, NBT])
            nc.scalar.activation(out=rr[:], in_=r2[:], func=AF.Sqrt)
            rpe = ctile('rpe', [EBLK, NBT])
            TS(out=rpe[:], in0=rr[:], scalar1=1e-9, scalar2=None, op0=OP.add)
            rinv = ctile('rinv', [EBLK, NBT])
            nc.vector.reciprocal(out=rinv[:], in_=rpe[:])
            uv = ctile('uv', [EBLK, 3 * NBT])
            TT(out=uv[:].rearrange("p (a b) -> p a b", a=3),
               in0=vd[:].rearrange("p (a b) -> p a b", a=3),
               in1=rinv[:].unsqueeze(1).broadcast_to([EBLK, 3, NBT]), op=OP.mult)

            # bessel: rad[r, blk] = sin((n+1) * pi/c * r) * (sqrt(2/c) * rinv)
            rscl = ctile('rscl', [EBLK, NBT])
            TS(out=rscl[:], in0=rinv[:], scalar1=float(-_RSCL), scalar2=None, op0=OP.mult)
            radp = ctile('radp', [EBLK, N_RBF * NBT])
            marg = ctile('marg', [EBLK, N_RBF * NBT])
            TT(out=marg[:].rearrange("p (r b) -> p r b", r=N_RBF),
               in0=rr[:].unsqueeze(1).broadcast_to([EBLK, N_RBF, NBT]),
               in1=s['nvec'][:].unsqueeze(2).broadcast_to([EBLK, N_RBF, NBT]),
               op=OP.mult)
            mtmp = ctile('mtmp', [EBLK, N_RBF * NBT])
            TS(out=mtmp[:], in0=marg[:], scalar1=4.0, scalar2=4.0,
               op0=OP.is_ge, op1=OP.mult)
            TT(out=marg[:], in0=marg[:], in1=mtmp[:], op=OP.subtract)
            TS(out=mtmp[:], in0=marg[:], scalar1=2.0, scalar2=2.0,
               op0=OP.is_ge, op1=OP.mult)
            TT(out=marg[:], in0=marg[:], in1=mtmp[:], op=OP.subtract)
            biaspi = ctile('biaspi', [EBLK, 1])
            nc.vector.memset(biaspi[:], float(-np.pi))
            nc.scalar.activation(out=radp[:], in_=marg[:], func=AF.Sin,
                                 scale=float(np.pi), bias=biaspi[:])
            TT(out=radp[:].rearrange("p (r b) -> p r b", r=N_RBF),
               in0=radp[:].rearrange("p (r b) -> p r b", r=N_RBF),
               in1=rscl[:].unsqueeze(1).broadcast_to([EBLK, N_RBF, NBT]), op=OP.mult)

            # poly cutoff (p=6); host guarantees u<1 (Pool engine)
            uu = ctile('uu', [EBLK, NBT])
            TS(out=uu[:], in0=rr[:], scalar1=float(1.0 / CUTOFF), scalar2=None, op0=OP.mult)
            u3 = ctile('u3', [EBLK, NBT])
            TTP(out=u3[:], in0=uu[:], in1=uu[:], op=OP.mult)
            TTP(out=u3[:], in0=u3[:], in1=uu[:], op=OP.mult)
            u6 = ctile('u6', [EBLK, NBT]); TTP(out=u6[:], in0=u3[:], in1=u3[:], op=OP.mult)
            u7 = ctile('u7', [EBLK, NBT]); TTP(out=u7[:], in0=u6[:], in1=uu[:], op=OP.mult)
            u8 = ctile('u8', [EBLK, NBT]); TTP(out=u8[:], in0=u7[:], in1=uu[:], op=OP.mult)
            fc = ctile('fc', [EBLK, NBT])
            nc.gpsimd.tensor_scalar(out=fc[:], in0=u6[:], scalar1=-28.0, scalar2=1.0,
                                    op0=OP.mult, op1=OP.add)
            t7 = ctile('t7', [EBLK, NBT])
            nc.gpsimd.tensor_scalar(out=t7[:], in0=u7[:], scalar1=48.0, scalar2=None,
                                    op0=OP.mult)
            TTP(out=fc[:], in0=fc[:], in1=t7[:], op=OP.add)
            nc.gpsimd.tensor_scalar(out=t7[:], in0=u8[:], scalar1=-21.0, scalar2=None,
                                    op0=OP.mult)
            TTP(out=fc[:], in0=fc[:], in1=t7[:], op=OP.add)

            radf = ctile('radf', [EBLK, N_RBF * NBT])
            TT(out=radf[:].rearrange("p (r b) -> p r b", r=N_RBF),
               in0=radp[:].rearrange("p (r b) -> p r b", r=N_RBF),
               in1=fc[:].unsqueeze(1).broadcast_to([EBLK, N_RBF, NBT]), op=OP.mult)

            # onehot [blk, n32] (DVE: Pool lacks is_equal)
            onehot = ctile('onehot', [EBLK, NBT * WIN])
            TT(out=onehot[:].rearrange("p (b n) -> p b n", b=NBT),
               in0=s['enloc'][:].unsqueeze(2).broadcast_to([EBLK, NBT, WIN]),
               in1=s['iota32'][:].unsqueeze(1).broadcast_to([EBLK, NBT, WIN]),
               op=OP.is_equal)

            # enc [blk, ks, kr] (Pool)
            enc = ctile('enc', [EBLK, NBT * C])
            TTP(out=enc[:].rearrange("p (b i j) -> p b i j", i=K, j=K),
                in0=s['eemb_s'][:].rearrange("p (k b) -> p b k", k=K).unsqueeze(3)
                    .broadcast_to([EBLK, NBT, K, K]),
                in1=s['eemb_r'][:].rearrange("p (k b) -> p b k", k=K).unsqueeze(2)
                    .broadcast_to([EBLK, NBT, K, K]),
                op=OP.mult)

            # angular monomials [l, blk]
            ones = ctile('ones', [EBLK, NBT])
            nc.vector.memset(ones[:], 1.0)
            x2 = ctile('x2', [EBLK, 3 * NBT])
            TT(out=x2[:], in0=uv[:], in1=uv[:], op=OP.mult)
            x3 = ctile('x3', [EBLK, 3 * NBT])
            TT(out=x3[:], in0=x2[:], in1=uv[:], op=OP.mult)

            def pow_plane(axis, p_):
                if p_ == 1:
                    return uv[:, axis * NBT:(axis + 1) * NBT]
                if p_ == 2:
                    return x2[:, axis * NBT:(axis + 1) * NBT]
                return x3[:, axis * NBT:(axis + 1) * NBT]

            ang = ctile('ang', [EBLK, NL * NBT])
            for l in range(NL):
                facs = [pow_plane(a, pw) for a, pw in enumerate((LX[l], LY[l], LZ[l])) if pw > 0]
                dst = ang[:, l * NBT:(l + 1) * NBT]
                if len(facs) == 0:
                    nc.scalar.copy(out=dst, in_=ones[:])
                elif len(facs) == 1:
                    nc.scalar.copy(out=dst, in_=facs[0])
                elif len(facs) == 2:
                    TT(out=dst, in0=facs[0], in1=facs[1], op=OP.mult)
                else:
                    TT(out=dst, in0=facs[0], in1=facs[1], op=OP.mult)
                    TT(out=dst, in0=dst, in1=facs[2], op=OP.mult)

            # fold sqrt(MULTI_l) into ang: every downstream tensor (A, table,
            # mem, A_ar, A_bchi) is then consistently per-l scaled, so the
            # symmetrize b2 sum needs no MULTI weighting (b1 has MULTI_0 = 1)
            TT(out=ang[:].rearrange("p (l b) -> p l b", l=NL),
               in0=ang[:].rearrange("p (l b) -> p l b", l=NL),
               in1=s['multi_l'][:].unsqueeze(2).broadcast_to([EBLK, NL, NBT]),
               op=OP.mult)

            # P = ang (x) enc : [blk, l, c] in fp16 (split DVE / Pool by half)
            P = ctile('P', [EBLK, NBT * LC], FP16)
            HB = NBT * 5 // 8
            for half, eng in ((0, TT), (1, TTP)):
                blo = half * HB; bcnt = (NBT - HB) if half else HB
                eng(out=P[:, blo * LC:(blo + bcnt) * LC]
                        .rearrange("p (b l c) -> p b l c", l=NL, c=C),
                    in0=ang[:].rearrange("p (l b) -> p b l", l=NL)[:, blo:blo + bcnt]
                        .unsqueeze(3).broadcast_to([EBLK, bcnt, NL, C]),
                    in1=enc[:].rearrange("p (b c) -> p b c", c=C)[:, blo:blo + bcnt]
                        .unsqueeze(2).broadcast_to([EBLK, bcnt, NL, C]),
                    op=OP.mult)

            # lhsT1_g = radf-half (x) onehot : [blk, r4, n32] (fp16; DVE/Pool split)
            lhsT1 = []
            for g in range(2):
                lt = ctile(f'lhsT1_{g}', [EBLK, NBT * EBLK], FP16)
                eng = TT
                eng(out=lt[:].rearrange("p (b r n) -> p b r n", r=4, n=WIN),
                    in0=radf[:].rearrange("p (r b) -> p b r", r=N_RBF)
                        [:, :, g * 4:(g + 1) * 4].unsqueeze(3)
                        .broadcast_to([EBLK, NBT, 4, WIN]),
                    in1=onehot[:].rearrange("p (b n) -> p b n", b=NBT).unsqueeze(2)
                        .broadcast_to([EBLK, NBT, 4, WIN]),
                    op=OP.mult)
                lhsT1.append(lt)

            # fr = (radf @ W_ar) * MP_NORM : own blocks only [blk, b8] (Pool)
            frA = ctile('frA', [EBLK, NBLK_OWN * N_RBF])
            frB = ctile('frB', [EBLK, NBLK_OWN * N_RBF])
            frt = ctile('frt', [EBLK, NBLK_OWN * N_RBF])
            for r_ in range(N_RBF):
                radv = radf[:, r_ * NBT:r_ * NBT + NBLK_OWN].unsqueeze(2) \
                    .broadcast_to([EBLK, NBLK_OWN, N_RBF])
                warv = s['war_mp'][:, r_ * N_RBF:(r_ + 1) * N_RBF].unsqueeze(1) \
                    .broadcast_to([EBLK, NBLK_OWN, N_RBF])
                if r_ == 0:
                    TTP(out=frA[:].rearrange("p (b k) -> p b k", k=N_RBF),
                        in0=radv, in1=warv, op=OP.mult)
                else:
                    TTP(out=frt[:].rearrange("p (b k) -> p b k", k=N_RBF),
                        in0=radv, in1=warv, op=OP.mult)
                    src, dst = (frA, frB) if r_ % 2 == 1 else (frB, frA)
                    TTP(out=dst[:], in0=src[:], in1=frt[:], op=OP.add)
            fr = frB

            # lhsT_ar_g = fr-half (x) onehot : own blocks only (fp16)
            lhsT_ar = []
            for g in range(2):
                lt = ctile(f'lhsT_ar_{g}', [EBLK, NBLK_OWN * EBLK], FP16)
                TTP(out=lt[:].rearrange("p (w b n) -> p w b n", b=4, n=WIN),
                   in0=fr[:].rearrange("p (w k) -> p w k", k=N_RBF)
                       [:, :, g * 4:(g + 1) * 4].unsqueeze(3)
                       .broadcast_to([EBLK, NBLK_OWN, 4, WIN]),
                   in1=onehot[:].rearrange("p (b n) -> p b n", b=NBT)
                       [:, :NBLK_OWN].unsqueeze(2)
                       .broadcast_to([EBLK, NBLK_OWN, 4, WIN]),
                   op=OP.mult)
                lhsT_ar.append(lt)

            # ---- pass 1: per-window segment sum + radial transform,
            # with symmetrize/chi groups interleaved every GW windows so the
            # DVE/Pool sym work overlaps the PE window matmuls ----
            GW = 4                       # windows per group
            NG = WT // GW                # WT is a multiple of 4 (NH padded)
            NVG = GW * 2
            A_sb = ctile('A_sb', [EBLK, WT * GLC], FP16)
            B0s = ctile('B0s', [EBLK, NWINC * 2 * NB * C])   # own B0 (output, f32)
            chiS = ctile('chiS', [WIN, WT * C])
            chiSb = ctile('chiSb', [WIN, WT * C], FP16)
            memS = ctile('memS', [EBLK, NWINC * GLC])        # parked psMem (f32)

            def sym_group(Ain, Bout, pool_eng):
                # Ain fp16 [p, (8 pseudo-windows, lc)] view; Bout f32 [p, (8, NB*C)]
                tte = TTP if pool_eng else TT
                red = nc.vector.tensor_reduce
                sqs = wp.tile([EBLK, NVG * LC], FP16, name='sqs', tag='sqs')
                tte(out=sqs[:], in0=Ain, in1=Ain, op=OP.mult)
                nc.scalar.copy(
                    out=Bout.rearrange("p (v q) -> p v q", q=NB * C)[:, :, 0:C],
                    in_=Ain.rearrange("p (v q) -> p v q", q=LC)[:, :, 0:C])
                for dd, (ls, lcnt) in enumerate(GRP_SLICES):
                    red(
                        Bout.rearrange("p (v q) -> p v q", q=NB * C)
                            [:, :, (1 + dd) * C:(2 + dd) * C].unsqueeze(3),
                        sqs[:].rearrange("p (v l c) -> p v c l", l=NL, c=C)
                            [:, :, :, ls:ls + lcnt],
                        mybir.AxisListType.X, OP.add)

            def chi_group(gi, Bg):
                Bgb = wp.tile([EBLK, NVG * NB * C], FP16, name='Bgb', tag='Bgb')
                nc.any.tensor_copy(out=Bgb[:], in_=Bg)
                for wl in range(GW):
                    w = gi * GW + wl
                    psC = pp.tile([WIN, C], F32, name='psC', tag='seg')
                    first = True
                    for g in range(2):
                        for k in range(NB):
                            nc.tensor.matmul(
                                out=psC[:],
                                lhsT=sb['wbd_chi'][:, (g * NB + k) * WIN:
                                                   (g * NB + k + 1) * WIN],
                                rhs=Bgb[:, (wl * 2 + g) * NB * C + k * C:
                                           (wl * 2 + g) * NB * C + (k + 1) * C],
                                start=first, stop=(g == 1 and k == NB - 1),
                                skip_group_check=True)
                            first = False
                    nc.any.tensor_copy(out=chiS[:, w * C:(w + 1) * C], in_=psC[:])

            def mem_window(w):
                psMem = pp.tile([EBLK, GLC], F32, name='psMem', tag='memt')
                for gout in range(2):
                    for dd, (ls, lcnt) in enumerate(GRP_SLICES):
                        osl = slice(gout * LC + ls * C, gout * LC + (ls + lcnt) * C)
                        for gin in range(2):
                            wcol = ((gout * 4 + dd) * 2 + gin) * EBLK
                            csl = slice(w * GLC + gin * LC + ls * C,
                                        w * GLC + gin * LC + (ls + lcnt) * C)
                            nc.tensor.matmul(
                                out=psMem[:, osl],
                                lhsT=sb['wbd_mem'][:, wcol:wcol + EBLK],
                                rhs=A_sb[:, csl],
                                start=(gin == 0), stop=(gin == 1),
                                skip_group_check=True)
                nc.any.tensor_copy(out=memS[:, w * GLC:(w + 1) * GLC], in_=psMem[:])

            for w in range(WT):
                psA0 = pp.tile([EBLK, GLC], F32, name='psA0', tag='seg')
                for g in range(2):
                    for bi in range(NBW):
                        blk = w * NBW + bi
                        nc.tensor.matmul(
                            out=psA0[:, g * LC:(g + 1) * LC],
                            lhsT=lhsT1[g][:, blk * EBLK:(blk + 1) * EBLK],
                            rhs=P[:, blk * LC:(blk + 1) * LC],
                            start=(bi == 0), stop=(bi == NBW - 1),
                            skip_group_check=True)
                A0s = wp.tile([EBLK, GLC], FP16, name='A0s', tag='A0s')
                nc.any.tensor_copy(out=A0s[:], in_=psA0[:])
                psA = pp.tile([EBLK, GLC], F32, name='psA', tag='acc')
                for gout in range(2):
                    for dd, (ls, lcnt) in enumerate(GRP_SLICES):
                        osl = slice(gout * LC + ls * C, gout * LC + (ls + lcnt) * C)
                        for gin in range(2):
                            wcol = ((gout * 4 + dd) * 2 + gin) * EBLK
                            csl = slice(gin * LC + ls * C, gin * LC + (ls + lcnt) * C)
                            nc.tensor.matmul(
                                out=psA[:, osl],
                                lhsT=sb['wbd_rad'][:, wcol:wcol + EBLK],
                                rhs=A0s[:, csl],
                                start=(gin == 0), stop=(gin == 1),
                                skip_group_check=True)
                nc.any.tensor_copy(out=A_sb[:, w * GLC:(w + 1) * GLC], in_=psA[:])

            # own A rows are complete: PE parks all psMem results while the
            # DVE/Pool sym groups below run concurrently
            for wm in range(NWINC):
                mem_window(wm)

            for gi in range(NG):
                own_grp = gi < NWINC // GW
                if own_grp:
                    Bg = B0s[:, gi * NVG * NB * C:(gi + 1) * NVG * NB * C]
                else:
                    Bgt = wp.tile([EBLK, NVG * NB * C], F32, name='Bgt', tag='Bgt')
                    Bg = Bgt[:]
                sym_group(A_sb[:, gi * GW * GLC:(gi + 1) * GW * GLC], Bg,
                          pool_eng=(gi % 3 == 2))
                chi_group(gi, Bg)
            nc.any.tensor_copy(out=chiSb[:], in_=chiS[:])

            # ---- node table -> local DRAM (fp16); no collective ----
            T_local = dp.tile([WT * WIN, TW], FP16, name='T_local')
            for x in range(4):
                nc.sync.dma_start(
                    out=T_local[:, x * GLC:(x + 1) * GLC]
                        .rearrange("(w n) q -> n w q", w=WT),
                    in_=A_sb[x * WIN:(x + 1) * WIN, :]
                        .rearrange("n (w q) -> n w q", w=WT))
            nc.sync.dma_start(
                out=T_local[:, RB * LC:RB * LC + C]
                    .rearrange("(w n) c -> n w c", w=WT),
                in_=chiSb[:].rearrange("n (w c) -> n w c", w=WT))

            # ---- pass 2 (own windows only) ----
            # issue all gathers up front: they only depend on the T_local
            # write, so the DMA engines prefetch while sym/chi still run
            ags_all = []
            for blk in range(NBLK_OWN):
                ag = gp.tile([EBLK, TW], FP16, name='ag', tag='ag')
                nc.gpsimd.indirect_dma_start(
                    out=ag[:], out_offset=None, in_=T_local[:],
                    in_offset=bass.IndirectOffsetOnAxis(
                        ap=esrow_s[:, blk:blk + 1], axis=0))
                ags_all.append(ag)
            Anew = ctile('Anew', [EBLK, NWINC * GLC], FP16)
            for w in range(NWINC):
                ags = []
                P2s = []
                for bi in range(NBW):
                    blk = w * NBW + bi
                    ag = ags_all[blk]
                    ags.append(ag)
                    P2 = wp.tile([EBLK, LC], FP16, name='P2', tag='P2')
                    TT(out=P2[:].rearrange("p (l c) -> p l c", c=C),
                        in0=P[:, blk * LC:(blk + 1) * LC].rearrange("p (l c) -> p l c", c=C),
                        in1=ag[:, RB * LC:RB * LC + C].unsqueeze(1)
                            .broadcast_to([EBLK, NL, C]),
                        op=OP.mult)
                    P2s.append(P2)
                psB0 = pp.tile([EBLK, GLC], F32, name='psB0', tag='seg')
                for g in range(2):
                    for bi in range(NBW):
                        blk = w * NBW + bi
                        nc.tensor.matmul(
                            out=psB0[:, g * LC:(g + 1) * LC],
                            lhsT=lhsT1[g][:, blk * EBLK:(blk + 1) * EBLK],
                            rhs=P2s[bi][:],
                            start=(bi == 0), stop=(bi == NBW - 1),
                            skip_group_check=True)
                Ab0 = wp.tile([EBLK, GLC], FP16, name='Ab0', tag='Ab0')
                nc.any.tensor_copy(out=Ab0[:], in_=psB0[:])
                psAb = pp.tile([EBLK, GLC], F32, name='psAb', tag='acc')
                for gout in range(2):
                    for dd, (ls, lcnt) in enumerate(GRP_SLICES):
                        osl = slice(gout * LC + ls * C, gout * LC + (ls + lcnt) * C)
                        for gin in range(2):
                            wcol = ((gout * 4 + dd) * 2 + gin) * EBLK
                            csl = slice(gin * LC + ls * C, gin * LC + (ls + lcnt) * C)
                            nc.tensor.matmul(
                                out=psAb[:, osl],
                                lhsT=sb['wbd_radmp'][:, wcol:wcol + EBLK],
                                rhs=Ab0[:, csl],
                                start=(gin == 0), stop=(gin == 1),
                                skip_group_check=True)
                psAr = pp.tile([EBLK, GLC], F32, name='psAr', tag='ar')
                for b_ in range(RB):
                    g = b_ // 4; xq = b_ % 4
                    scol = xq * 2 + g          # T col-slice index for b_
                    for bi in range(NBW):
                        blk = w * NBW + bi
                        nc.tensor.matmul(
                            out=psAr[xq * WIN:(xq + 1) * WIN, g * LC:(g + 1) * LC],
                            lhsT=lhsT_ar[g][:, blk * EBLK + xq * WIN:
                                            blk * EBLK + (xq + 1) * WIN],
                            rhs=ags[bi][:, scol * LC:(scol + 1) * LC],
                            start=(bi == 0), stop=(bi == NBW - 1),
                            skip_group_check=True,
                            tile_position=(0, xq * WIN))
                comb = wp.tile([EBLK, GLC], F32, name='comb', tag='comb')
                nc.scalar.copy(out=comb[:], in_=psAb[:])
                TT(out=comb[:], in0=comb[:], in1=psAr[:], op=OP.add)
                TT(out=Anew[:, w * GLC:(w + 1) * GLC], in0=comb[:],
                   in1=memS[:, w * GLC:(w + 1) * GLC], op=OP.add)

            # ---- B1 symmetrize (own windows, f32) ----
            B1s = ctile('B1s', [EBLK, NWINC * 2 * NB * C])
            for gi in range(NWINC // GW):
                sym_group(Anew[:, gi * GW * GLC:(gi + 1) * GW * GLC],
                          B1s[:, gi * NVG * NB * C:(gi + 1) * NVG * NB * C],
                          pool_eng=(gi % 2 == 1))

            # ---- output: [t2, (w,g)=16, 45] ----
            half = NWINC * 2 * NB * C
            nc.sync.dma_start(out=outB[:, 0:half], in_=B0s[:])
            nc.sync.dma_start(out=outB[:, half:2 * half], in_=B1s[:])

    nc.compile()
    return nc


_CACHE = {}


def kernel(**inputs) -> np.ndarray:
    return _kernel_impl(inputs)[0]


def _kernel_impl(inputs, trace=False):
    from concourse.bass_utils import run_bass_kernel_spmd

    packed, slot_of_node, nh = _host_prep(inputs)

    key = ('nc', nh)
    if key not in _CACHE:
        _CACHE[key] = _build_program(nh)
    nc = _CACHE[key]

    in_maps = [dict(p) for p in packed]

    res = run_bass_kernel_spmd(nc, in_maps, core_ids=list(range(N_CORES)),
                               trace=trace)

    feats_slots = np.zeros((NSLOT, RB, NB, C, 2), np.float32)
    for ci in range(N_CORES):
        arr = res.results[ci]['outB'].reshape(4, WIN, 2, NWINC, 2, NB, C)
        arr = np.transpose(arr, (3, 1, 4, 0, 5, 6, 2))
        feats_slots[ci * NWINC * WIN:(ci + 1) * NWINC * WIN] = \
            arr.reshape(NWINC * WIN, RB, NB, C, 2)
    return feats_slots[slot_of_node], res


if __name__ == '__main__':
    import pickle, os
    if os.path.exists('/tmp/inputs.pkl'):
        inputs = pickle.load(open('/tmp/inputs.pkl', 'rb'))
    else:
        import reference as Rf
        inputs = {k: np.asarray(v) for k, v in Rf.setup_inputs().items()}
        pickle.dump(inputs, open('/tmp/inputs.pkl', 'wb'))
    out = kernel(**inputs)
    print("kernel out", out.shape, out.dtype, float(np.abs(out).max()))
    if os.path.exists('/tmp/expected.npy'):
        exp = np.load('/tmp/expected.npy')
        err = np.abs(out - exp).max()
        print("max abs err vs expected:", err, "rel:", err / np.abs(exp).max())


# revision 42
# speedup vs baseline: 1.0464x; 1.0464x over previous
"""Trainium2 Bass kernel for nn_Cace_74569222193773 (CACE GNN message passing).

Strategy (8 NeuronCores, SPMD, one program shape + per-core data):
  * Host: drop edges with r >= cutoff (fcut = 0 there), assign nodes to 64
    edge-balanced global windows of <=32 nodes (8 "own" windows per core).
  * HALO REPLICATION instead of a collective: each core additionally
    recomputes pass-1 A for the sender nodes of its own edges that live on
    other cores.  Those halo nodes are repacked into private halo windows
    (<=32 nodes, <=256 in-edges each, edge-balanced); the core processes
    own + halo windows in pass 1, writes the node table T = [A row | chi]
    (fp16) to its own DRAM, and pass 2 gathers sender rows locally.
    No inter-core communication at all.
  * All node-feature tensors live in a "half" layout: partition p = x*32+n
    with x = (r or b) mod 4, plus a half index g = (r or b) // 4 in the
    free dimension, so every PE matmul output starts at a 32-aligned
    partition base.
  * Pass 1 (per core): edge geometry + bessel + cutoff + angular on
    DVE/Pool/ACT in edge-major layout [128 partitions = edges]; per-window
    segment-sum via PE matmuls (fp16 operands, fp32 PSUM) with
    lhsT = onehot32 (x) radf-half, rhs = P = ang (x) enc; radial transform
    via block-diag W (x) I32 fp16 matmuls; symmetrize + chi per window
    group (own windows in fp32 A, halo windows from the fp16 copy -- halo
    B0 only feeds chi).
  * Pass 2 (own windows only): indirect-DMA gather of T[send] (fp16 rows),
    A_ar via per-b matmuls (lhsT = onehot (x) fr slice), A_bchi via the
    pass-1 segment-sum machinery with rhs P * chi_send, mem via
    W_mem (x) I32; combine (fp32), symmetrize -> B1.

kernel() takes FULL unsharded inputs and returns the FULL [2000,8,5,9,2]
float32 output; all sharding happens inside.
"""
import heapq
from math import factorial

import numpy as np

# ---- static problem config (mirrors the reference) ----
MAX_L = 3; N_RBF = 8; RB = 8; K = 3
CUTOFF = 5.5
N_NODES = 2000
MP_NORM = 1.0 / np.sqrt(25.0)
C = K * K                      # 9
NB = 1 + (MAX_L + 1)           # 5

def _lxlylz(max_l):
    out = []
    for l in range(max_l + 1):
        for lx in range(l, -1, -1):
            for ly in range(l - lx, -1, -1):
                out.append((lx, ly, l - lx - ly))
    return out

L_LIST = _lxlylz(MAX_L); NL = len(L_LIST)                       # 20
LX = np.array([t[0] for t in L_LIST]); LY = np.array([t[1] for t in L_LIST])
LZ = np.array([t[2] for t in L_LIST]); DEGS = LX + LY + LZ
MULTI = np.array([factorial(int(d)) / (factorial(int(a)) * factorial(int(b)) * factorial(int(c)))
                  for a, b, c, d in zip(LX, LY, LZ, DEGS)], dtype=np.float32)
GRP_SLICES = []                 # (l_start, l_count) per degree; DEGS is sorted
for d in range(MAX_L + 1):
    idx = np.where(DEGS == d)[0]
    GRP_SLICES.append((int(idx[0]), int(len(idx))))

# ---- sharding geometry ----
N_CORES = 8
WIN = 32                        # nodes per window
NWINC = 8                       # own windows per core
NWIN = N_CORES * NWINC          # 64
NSLOT = NWIN * WIN              # 2048 own-node slots globally
EBLK = 128                      # edges per block (partition dim)
NBW = 2                         # blocks per window
NBLK_OWN = NWINC * NBW          # 16 own blocks per core
LC = NL * C                     # 180
GLC = 2 * LC                    # 360 = both halves
TW = RB * LC + WIN              # table row width 1472 (1440 A + 9 chi + pad)

_RSCL = np.sqrt(2.0 / CUTOFF)

F32_FIELDS = ['exyz_s', 'exyz_r', 'eemb_s', 'eemb_r', 'enloc',
              'iota32', 'multi_l', 'war_mp', 'nvec']
FP16_FIELDS = ['wbd_rad', 'wbd_radmp', 'wbd_mem', 'wbd_chi']


def _field_layout(nbt):
    """Column layout of the packed f32 / fp16 input tensors for nbt blocks."""
    fw = dict(exyz_s=3 * nbt, exyz_r=3 * nbt, eemb_s=3 * nbt, eemb_r=3 * nbt,
              enloc=nbt, iota32=WIN, multi_l=NL, war_mp=64, nvec=N_RBF,
              wbd_rad=2048, wbd_radmp=2048, wbd_mem=2048, wbd_chi=2 * NB * WIN)
    off = {}
    o = 0
    for f in F32_FIELDS:
        off[f] = o; o += fw[f]
    totf = o
    o = 0
    for f in FP16_FIELDS:
        off[f] = o; o += fw[f]
    return fw, off, totf, o


def _pack_windows(node_list, deg, nwin_cap):
    """Balanced assignment of node_list into windows (<=WIN nodes each,
    edge-load balanced).  Grows window count until max load <= NBW*EBLK.
    Returns (win_of, pos_of, n_windows)."""
    nodes = sorted(node_list, key=lambda n: -deg[n])
    nwin = max(1, (len(nodes) + WIN - 1) // WIN)
    while True:
        win_cnt = np.zeros(nwin, np.int64); win_load = np.zeros(nwin, np.int64)
        win_of = {}; pos_of = {}
        heap = [(0, w) for w in range(nwin)]
        heapq.heapify(heap)
        ok = True
        for nd in nodes:
            popped = []
            while True:
                load, w = heapq.heappop(heap)
                if win_cnt[w] < WIN:
                    break
                popped.append((load, w))
            for it in popped:
                heapq.heappush(heap, it)
            win_of[nd] = w; pos_of[nd] = int(win_cnt[w])
            win_cnt[w] += 1; win_load[w] += deg[nd]
            heapq.heappush(heap, (int(win_load[w]), w))
        if win_load.max(initial=0) <= NBW * EBLK:
            return win_of, pos_of, nwin
        nwin += 1
        if nwin > nwin_cap:
            raise RuntimeError("halo window packing overflow")


def _host_prep(inputs):
    pos = np.asarray(inputs['positions'], np.float32)
    shifts = np.asarray(inputs['shifts'], np.float32)
    W_embed = np.asarray(inputs['W_embed'], np.float32)
    species = np.asarray(inputs['species'])
    ei = np.asarray(inputs['edge_index'])
    send, recv = ei[0], ei[1]

    vec = (pos[recv] + shifts - pos[send]).astype(np.float64)
    r = np.sqrt((vec * vec).sum(-1))
    keep = np.where(r < CUTOFF)[0]
    deg = np.bincount(recv[keep], minlength=N_NODES)

    # balanced node->global-window assignment (own windows)
    order = np.argsort(-deg, kind='stable')
    win_cnt = np.zeros(NWIN, np.int64); win_load = np.zeros(NWIN, np.int64)
    win_of_node = np.zeros(N_NODES, np.int64); pos_in_win = np.zeros(N_NODES, np.int64)
    heap = [(0, w) for w in range(NWIN)]
    heapq.heapify(heap)
    for nd in order:
        popped = []
        while True:
            load, w = heapq.heappop(heap)
            if win_cnt[w] < WIN:
                break
            popped.append((load, w))
        for it in popped:
            heapq.heappush(heap, it)
        win_of_node[nd] = w; pos_in_win[nd] = win_cnt[w]
        win_cnt[w] += 1; win_load[w] += deg[nd]
        heapq.heappush(heap, (win_load[w], w))
    if win_load.max() > NBW * EBLK:
        raise RuntimeError(f"window overflow: {win_load.max()} > {NBW * EBLK}")

    slot_of_node = win_of_node * WIN + pos_in_win
    emb = W_embed[species]                       # [N, K]

    ks, kr = send[keep], recv[keep]
    in_edges = [[] for _ in range(N_NODES)]      # node -> kept edge ids
    for i, e in enumerate(keep):
        in_edges[kr[i]].append(e)

    # per-core halo structure
    core_halo = []
    nh_list = []
    for ci in range(N_CORES):
        own_w = set(range(ci * NWINC, (ci + 1) * NWINC))
        own_eids = []
        for w in sorted(own_w):
            for nd in np.where(win_of_node == w)[0]:
                own_eids.extend(in_edges[nd])
        senders = set(send[own_eids].tolist()) if own_eids else set()
        halo = [s for s in senders if win_of_node[s] not in own_w]
        hwin_of, hpos_of, nh = _pack_windows(halo, deg, 64)
        core_halo.append((own_w, hwin_of, hpos_of, nh))
        nh_list.append(nh)
    NH = max(nh_list)
    NH = ((NH + 3) // 4) * 4      # pad so WT = 8 + NH is a multiple of 4
    WT = NWINC + NH
    NBT = NBW * WT
    EPAD = NBT * EBLK

    cores = []
    for ci in range(N_CORES):
        own_w, hwin_of, hpos_of, nh = core_halo[ci]
        e_xyz_s = np.zeros((EPAD, 3), np.float32)
        e_xyz_r = np.zeros((EPAD, 3), np.float32)
        e_emb_s = np.zeros((EPAD, K), np.float32)
        e_emb_r = np.zeros((EPAD, K), np.float32)
        e_nloc = np.full((EPAD,), -1.0, np.float32)
        e_srow = np.zeros((EPAD,), np.int32)
        e_xyz_r[:, 0] = 1.0                      # pads: r = 1, finite math

        def srow_of(s):
            w = win_of_node[s]
            if w in own_w:
                return (w - ci * NWINC) * WIN + pos_in_win[s]
            return (NWINC + hwin_of[s]) * WIN + hpos_of[s]

        # local window wl in [0, WT): own first, then halo
        def fill_window(wl, node_ids, pos_of, need_srow):
            base = wl * NBW * EBLK
            eids = []
            for nd in node_ids:
                eids.extend(in_edges[nd])
            eids = np.array(eids, dtype=np.int64)
            cnt = len(eids)
            if cnt == 0:
                return
            if cnt > NBW * EBLK:
                raise RuntimeError("window edge overflow")
            sl = slice(base, base + cnt)
            e_xyz_s[sl] = pos[send[eids]]
            e_xyz_r[sl] = pos[recv[eids]] + shifts[eids]
            e_emb_s[sl] = emb[send[eids]]
            e_emb_r[sl] = emb[recv[eids]]
            e_nloc[sl] = np.array([pos_of[n] for n in recv[eids]], np.float32)
            if need_srow:
                e_srow[sl] = np.array([srow_of(s) for s in send[eids]], np.int32)

        for wl in range(NWINC):
            w = ci * NWINC + wl
            nds = np.where(win_of_node == w)[0]
            fill_window(wl, nds, {int(n): int(pos_in_win[n]) for n in nds}, True)
        halo_by_win = [[] for _ in range(nh)]
        for s, hw in hwin_of.items():
            halo_by_win[hw].append(s)
        for hw in range(nh):
            fill_window(NWINC + hw, halo_by_win[hw],
                        {int(n): int(hpos_of[n]) for n in halo_by_win[hw]}, False)

        def dev(x):
            if x.ndim == 1:
                return np.ascontiguousarray(x.reshape(NBT, EBLK).T)
            return np.ascontiguousarray(np.transpose(x.reshape(NBT, EBLK, -1), (1, 0, 2)))

        def axmajor(x3):
            d = dev(x3)                                  # [128, NBT, 3]
            return np.ascontiguousarray(np.transpose(d, (0, 2, 1)).reshape(EBLK, 3 * NBT))

        cores.append(dict(
            exyz_s=axmajor(e_xyz_s), exyz_r=axmajor(e_xyz_r),
            eemb_s=axmajor(e_emb_s), eemb_r=axmajor(e_emb_r),
            enloc=np.ascontiguousarray(dev(e_nloc)),
            esrow=np.ascontiguousarray(dev(e_srow)[:, :NBLK_OWN]),
        ))

    Wr = np.asarray(inputs['W_radial'], np.float32)   # [4(deg), 8(r), 8(b)]
    Wm = np.asarray(inputs['W_mem'], np.float32)
    Wc = np.asarray(inputs['W_chi'], np.float32)      # [8(b), 5(k)]
    Wa = np.asarray(inputs['W_ar'], np.float32)       # [8(r), 8(b)]
    I32 = np.eye(WIN, dtype=np.float32)

    def bd(W):
        cols = []
        for gout in range(2):
            for d in range(4):
                for gin in range(2):
                    cols.append(np.kron(W[d, gin * 4:gin * 4 + 4, gout * 4:gout * 4 + 4], I32))
        return np.concatenate(cols, axis=1)          # [128, 2048]

    wchi_cols = []
    for g in range(2):
        for k in range(NB):
            wchi_cols.append(np.kron(Wc[g * 4:g * 4 + 4, k:k + 1], I32))   # [128, 32]
    consts_f = dict(
        war_mp=np.tile((Wa * MP_NORM).reshape(1, 64), (EBLK, 1)),
        multi_l=np.tile(np.sqrt(MULTI).reshape(1, NL), (EBLK, 1)),
        iota32=np.tile(np.arange(WIN, dtype=np.float32).reshape(1, WIN), (EBLK, 1)),
        nvec=np.tile((np.arange(1, N_RBF + 1, dtype=np.float32) / CUTOFF).reshape(1, N_RBF),
                     (EBLK, 1)),
    )
    consts_b = dict(
        wbd_rad=bd(Wr),
        wbd_radmp=bd(Wr * MP_NORM),
        wbd_mem=bd(Wm),
        wbd_chi=np.concatenate(wchi_cols, axis=1),                   # [128, 320]
    )
    packed = []
    for ci in range(N_CORES):
        cols_f = [cores[ci][nm] for nm in
                  ['exyz_s', 'exyz_r', 'eemb_s', 'eemb_r', 'enloc']]
        cols_f += [consts_f[nm] for nm in ['iota32', 'multi_l', 'war_mp', 'nvec']]
        edf = np.ascontiguousarray(np.concatenate(cols_f, axis=1), np.float32)
        edb = np.ascontiguousarray(
            np.concatenate([consts_b[nm] for nm in FP16_FIELDS], axis=1)
        ).astype(np.float16)
        packed.append(dict(edf=edf, edb=edb, esrow=cores[ci]['esrow']))
    return packed, slot_of_node, NH


def _build_program(nh, debug=False):
    import concourse.bass as bass
    import concourse.mybir as mybir
    from concourse import bacc
    from concourse.tile import TileContext

    F32 = mybir.dt.float32
    FP16 = mybir.dt.float16
    AF = mybir.ActivationFunctionType
    OP = mybir.AluOpType

    WT = NWINC + nh
    NBT = NBW * WT
    FIELD_W, FIELD_OFF, TOTW_F, TOTW_B = _field_layout(NBT)

    nc = bacc.Bacc("TRN2", target_bir_lowering=False, debug=False,
                   num_devices=N_CORES)

    edf_d = nc.dram_tensor('edf', [EBLK, TOTW_F], F32, kind="ExternalInput")
    edb_d = nc.dram_tensor('edb', [EBLK, TOTW_B], FP16, kind="ExternalInput")
    esrow_d = nc.dram_tensor('esrow', [EBLK, NBLK_OWN], mybir.dt.int32,
                             kind="ExternalInput")
    outB = nc.dram_tensor('outB', [EBLK, 2 * NWINC * 2 * NB * C], F32,
                          kind="ExternalOutput")

    with TileContext(nc) as tc:
        with (tc.tile_pool(name="const", bufs=1) as cp,
              tc.tile_pool(name="work", bufs=2) as wp,
              tc.tile_pool(name="gat", bufs=9) as gp,
              tc.tile_pool(name="psum", bufs=2, space="PSUM") as pp,
              tc.tile_pool(name="dram", bufs=1, space="DRAM") as dp):

            bigf = cp.tile([EBLK, TOTW_F], F32, name='bigf', tag='bigf')
            nc.sync.dma_start(out=bigf[:], in_=edf_d[:])
            bigb = cp.tile([EBLK, TOTW_B], FP16, name='bigb', tag='bigb')
            nc.sync.dma_start(out=bigb[:], in_=edb_d[:])
            esrow_s = cp.tile([EBLK, NBLK_OWN], mybir.dt.int32,
                              name='esrow_s', tag='esrow_s')
            nc.sync.dma_start(out=esrow_s[:], in_=esrow_d[:])

            class _S:
                def __init__(self, tile):
                    self.tile = tile
                def __getitem__(self, nm):
                    off = FIELD_OFF[nm]
                    return self.tile[:, off:off + FIELD_W[nm]]
            s = _S(bigf)
            sb = _S(bigb)

            def ctile(tag, shape, dtype=F32):
                return cp.tile(shape, dtype, name=tag, tag=tag)

            TT = nc.vector.tensor_tensor
            TTP = nc.gpsimd.tensor_tensor
            TS = nc.vector.tensor_scalar

            # ---- geometry, edge-major [128, a*NBT+blk] ----
            vd = ctile('vd', [EBLK, 3 * NBT])
            TT(out=vd[:], in0=s['exyz_r'][:], in1=s['exyz_s'][:], op=OP.subtract)
            sq = ctile('sq', [EBLK, 3 * NBT])
            TT(out=sq[:], in0=vd[:], in1=vd[:], op=OP.mult)
            r2 = ctile('r2', [EBLK, NBT])
            TT(out=r2[:], in0=sq[:, 0:NBT], in1=sq[:, NBT:2 * NBT], op=OP.add)
            TT(out=r2[:], in0=r2[:], in1=sq[:, 2 * NBT:3 * NBT], op=OP.add)
            rr = ctile('rr', [EBLK, NBT])
            nc.scalar.activation(out=rr[:], in_=r2[:], func=AF.Sqrt)
            rpe = ctile('rpe', [EBLK, NBT])
            TS(out=rpe[:], in0=rr[:], scalar1=1e-9, scalar2=None, op0=OP.add)
            rinv = ctile('rinv', [EBLK, NBT])
            nc.vector.reciprocal(out=rinv[:], in_=rpe[:])
            uv = ctile('uv', [EBLK, 3 * NBT])
            TT(out=uv[:].rearrange("p (a b) -> p a b", a=3),
               in0=vd[:].rearrange("p (a b) -> p a b", a=3),
               in1=rinv[:].unsqueeze(1).broadcast_to([EBLK, 3, NBT]), op=OP.mult)

            # bessel: rad[r, blk] = sin((n+1) * pi/c * r) * (sqrt(2/c) * rinv)
            rscl = ctile('rscl', [EBLK, NBT])
            TS(out=rscl[:], in0=rinv[:], scalar1=float(-_RSCL), scalar2=None, op0=OP.mult)
            radp = ctile('radp', [EBLK, N_RBF * NBT])
            marg = ctile('marg', [EBLK, N_RBF * NBT])
            TT(out=marg[:].rearrange("p (r b) -> p r b", r=N_RBF),
               in0=rr[:].unsqueeze(1).broadcast_to([EBLK, N_RBF, NBT]),
               in1=s['nvec'][:].unsqueeze(2).broadcast_to([EBLK, N_RBF, NBT]),
               op=OP.mult)
            mtmp = ctile('mtmp', [EBLK, N_RBF * NBT])
            TS(out=mtmp[:], in0=marg[:], scalar1=4.0, scalar2=4.0,
               op0=OP.is_ge, op1=OP.mult)
            TT(out=marg[:], in0=marg[:], in1=mtmp[:], op=OP.subtract)
            TS(out=mtmp[:], in0=marg[:], scalar1=2.0, scalar2=2.0,
               op0=OP.is_ge, op1=OP.mult)
            TT(out=marg[:], in0=marg[:], in1=mtmp[:], op=OP.subtract)
            biaspi = ctile('biaspi', [EBLK, 1])
            nc.vector.memset(biaspi[:], float(-np.pi))
            nc.scalar.activation(out=radp[:], in_=marg[:], func=AF.Sin,
                                 scale=float(np.pi), bias=biaspi[:])
            TT(out=radp[:].rearrange("p (r b) -> p r b", r=N_RBF),
               in0=radp[:].rearrange("p (r b) -> p r b", r=N_RBF),
               in1=rscl[:].unsqueeze(1).broadcast_to([EBLK, N_RBF, NBT]), op=OP.mult)

            # poly cutoff (p=6); host guarantees u<1 (Pool engine)
            uu = ctile('uu', [EBLK, NBT])
            TS(out=uu[:], in0=rr[:], scalar1=float(1.0 / CUTOFF), scalar2=None, op0=OP.mult)
            u3 = ctile('u3', [EBLK, NBT])
            TTP(out=u3[:], in0=uu[:], in1=uu[:], op=OP.mult)
            TTP(out=u3[:], in0=u3[:], in1=uu[:], op=OP.mult)
            u6 = ctile('u6', [EBLK, NBT]); TTP(out=u6[:], in0=u3[:], in1=u3[:], op=OP.mult)
            u7 = ctile('u7', [EBLK, NBT]); TTP(out=u7[:], in0=u6[:], in1=uu[:], op=OP.mult)
            u8 = ctile('u8', [EBLK, NBT]); TTP(out=u8[:], in0=u7[:], in1=uu[:], op=OP.mult)
            fc = ctile('fc', [EBLK, NBT])
            nc.gpsimd.tensor_scalar(out=fc[:], in0=u6[:], scalar1=-28.0, scalar2=1.0,
                                    op0=OP.mult, op1=OP.add)
            t7 = ctile('t7', [EBLK, NBT])
            nc.gpsimd.tensor_scalar(out=t7[:], in0=u7[:], scalar1=48.0, scalar2=None,
                                    op0=OP.mult)
            TTP(out=fc[:], in0=fc[:], in1=t7[:], op=OP.add)
            nc.gpsimd.tensor_scalar(out=t7[:], in0=u8[:], scalar1=-21.0, scalar2=None,
                                    op0=OP.mult)
            TTP(out=fc[:], in0=fc[:], in1=t7[:], op=OP.add)

            radf = ctile('radf', [EBLK, N_RBF * NBT])
            TT(out=radf[:].rearrange("p (r b) -> p r b", r=N_RBF),
               in0=radp[:].rearrange("p (r b) -> p r b", r=N_RBF),
               in1=fc[:].unsqueeze(1).broadcast_to([EBLK, N_RBF, NBT]), op=OP.mult)

            # onehot [blk, n32] (DVE: Pool lacks is_equal)
            onehot = ctile('onehot', [EBLK, NBT * WIN])
            TT(out=onehot[:].rearrange("p (b n) -> p b n", b=NBT),
               in0=s['enloc'][:].unsqueeze(2).broadcast_to([EBLK, NBT, WIN]),
               in1=s['iota32'][:].unsqueeze(1).broadcast_to([EBLK, NBT, WIN]),
               op=OP.is_equal)

            # enc [blk, ks, kr] (Pool)
            enc = ctile('enc', [EBLK, NBT * C])
            TTP(out=enc[:].rearrange("p (b i j) -> p b i j", i=K, j=K),
                in0=s['eemb_s'][:].rearrange("p (k b) -> p b k", k=K).unsqueeze(3)
                    .broadcast_to([EBLK, NBT, K, K]),
                in1=s['eemb_r'][:].rearrange("p (k b) -> p b k", k=K).unsqueeze(2)
                    .broadcast_to([EBLK, NBT, K, K]),
                op=OP.mult)

            # angular monomials [l, blk]
            ones = ctile('ones', [EBLK, NBT])
            nc.vector.memset(ones[:], 1.0)
            x2 = ctile('x2', [EBLK, 3 * NBT])
            TT(out=x2[:], in0=uv[:], in1=uv[:], op=OP.mult)
            x3 = ctile('x3', [EBLK, 3 * NBT])
            TT(out=x3[:], in0=x2[:], in1=uv[:], op=OP.mult)

            def pow_plane(axis, p_):
                if p_ == 1:
                    return uv[:, axis * NBT:(axis + 1) * NBT]
                if p_ == 2:
                    return x2[:, axis * NBT:(axis + 1) * NBT]
                return x3[:, axis * NBT:(axis + 1) * NBT]

            ang = ctile('ang', [EBLK, NL * NBT])
            for l in range(NL):
                facs = [pow_plane(a, pw) for a, pw in enumerate((LX[l], LY[l], LZ[l])) if pw > 0]
                dst = ang[:, l * NBT:(l + 1) * NBT]
                if len(facs) == 0:
                    nc.scalar.copy(out=dst, in_=ones[:])
                elif len(facs) == 1:
                    nc.scalar.copy(out=dst, in_=facs[0])
                elif len(facs) == 2:
                    TT(out=dst, in0=facs[0], in1=facs[1], op=OP.mult)
                else:
                    TT(out=dst, in0=facs[0], in1=facs[1], op=OP.mult)
                    TT(out=dst, in0=dst, in1=facs[2], op=OP.mult)

            # fold sqrt(MULTI_l) into ang: every downstream tensor (A, table,
            # mem, A_ar, A_bchi) is then consistently per-l scaled, so the
            # symmetrize b2 sum needs no MULTI weighting (b1 has MULTI_0 = 1)
            TT(out=ang[:].rearrange("p (l b) -> p l b", l=NL),
               in0=ang[:].rearrange("p (l b) -> p l b", l=NL),
               in1=s['multi_l'][:].unsqueeze(2).broadcast_to([EBLK, NL, NBT]),
               op=OP.mult)

            # P = ang (x) enc : [blk, l, c] in fp16 (split DVE / Pool by half)
            P = ctile('P', [EBLK, NBT * LC], FP16)
            HB = NBT // 8
            for half, eng in ((0, TT), (1, TTP)):
                blo = half * HB; bcnt = (NBT - HB) if half else HB
                eng(out=P[:, blo * LC:(blo + bcnt) * LC]
                        .rearrange("p (b l c) -> p b l c", l=NL, c=C),
                    in0=ang[:].rearrange("p (l b) -> p b l", l=NL)[:, blo:blo + bcnt]
                        .unsqueeze(3).broadcast_to([EBLK, bcnt, NL, C]),
                    in1=enc[:].rearrange("p (b c) -> p b c", c=C)[:, blo:blo + bcnt]
                        .unsqueeze(2).broadcast_to([EBLK, bcnt, NL, C]),
                    op=OP.mult)

            # lhsT1_g = radf-half (x) onehot : [blk, r4, n32] (fp16; DVE/Pool split)
            lhsT1 = []
            for g in range(2):
                lt = ctile(f'lhsT1_{g}', [EBLK, NBT * EBLK], FP16)
                eng = TT
                eng(out=lt[:].rearrange("p (b r n) -> p b r n", r=4, n=WIN),
                    in0=radf[:].rearrange("p (r b) -> p b r", r=N_RBF)
                        [:, :, g * 4:(g + 1) * 4].unsqueeze(3)
                        .broadcast_to([EBLK, NBT, 4, WIN]),
                    in1=onehot[:].rearrange("p (b n) -> p b n", b=NBT).unsqueeze(2)
                        .broadcast_to([EBLK, NBT, 4, WIN]),
                    op=OP.mult)
                lhsT1.append(lt)

            # fr = (radf @ W_ar) * MP_NORM : own blocks only [blk, b8] (Pool)
            frA = ctile('frA', [EBLK, NBLK_OWN * N_RBF])
            frB = ctile('frB', [EBLK, NBLK_OWN * N_RBF])
            frt = ctile('frt', [EBLK, NBLK_OWN * N_RBF])
            for r_ in range(N_RBF):
                radv = radf[:, r_ * NBT:r_ * NBT + NBLK_OWN].unsqueeze(2) \
                    .broadcast_to([EBLK, NBLK_OWN, N_RBF])
                warv = s['war_mp'][:, r_ * N_RBF:(r_ + 1) * N_RBF].unsqueeze(1) \
                    .broadcast_to([EBLK, NBLK_OWN, N_RBF])
                if r_ == 0:
                    TTP(out=frA[:].rearrange("p (b k) -> p b k", k=N_RBF),
                        in0=radv, in1=warv, op=OP.mult)
                else:
                    TTP(out=frt[:].rearrange("p (b k) -> p b k", k=N_RBF),
                        in0=radv, in1=warv, op=OP.mult)
                    src, dst = (frA, frB) if r_ % 2 == 1 else (frB, frA)
                    TTP(out=dst[:], in0=src[:], in1=frt[:], op=OP.add)
            fr = frB

            # lhsT_ar_g = fr-half (x) onehot : own blocks only (fp16)
            lhsT_ar = []
            for g in range(2):
                lt = ctile(f'lhsT_ar_{g}', [EBLK, NBLK_OWN * EBLK], FP16)
                TTP(out=lt[:].rearrange("p (w b n) -> p w b n", b=4, n=WIN),
                   in0=fr[:].rearrange("p (w k) -> p w k", k=N_RBF)
                       [:, :, g * 4:(g + 1) * 4].unsqueeze(3)
                       .broadcast_to([EBLK, NBLK_OWN, 4, WIN]),
                   in1=onehot[:].rearrange("p (b n) -> p b n", b=NBT)
                       [:, :NBLK_OWN].unsqueeze(2)
                       .broadcast_to([EBLK, NBLK_OWN, 4, WIN]),
                   op=OP.mult)
                lhsT_ar.append(lt)

            # ---- pass 1: per-window segment sum + radial transform,
            # with symmetrize/chi groups interleaved every GW windows so the
            # DVE/Pool sym work overlaps the PE window matmuls ----
            GW = 4                       # windows per group
            NG = WT // GW                # WT is a multiple of 4 (NH padded)
            NVG = GW * 2
            A_sb = ctile('A_sb', [EBLK, WT * GLC], FP16)
            B0s = ctile('B0s', [EBLK, NWINC * 2 * NB * C])   # own B0 (output, f32)
            chiS = ctile('chiS', [WIN, WT * C])
            chiSb = ctile('chiSb', [WIN, WT * C], FP16)
            memS = ctile('memS', [EBLK, NWINC * GLC])        # parked psMem (f32)

            def sym_group(Ain, Bout, pool_eng):
                # Ain fp16 [p, (8 pseudo-windows, lc)] view; Bout f32 [p, (8, NB*C)]
                tte = TTP if pool_eng else TT
                red = nc.vector.tensor_reduce
                sqs = wp.tile([EBLK, NVG * LC], FP16, name='sqs', tag='sqs')
                tte(out=sqs[:], in0=Ain, in1=Ain, op=OP.mult)
                nc.scalar.copy(
                    out=Bout.rearrange("p (v q) -> p v q", q=NB * C)[:, :, 0:C],
                    in_=Ain.rearrange("p (v q) -> p v q", q=LC)[:, :, 0:C])
                for dd, (ls, lcnt) in enumerate(GRP_SLICES):
                    red(
                        Bout.rearrange("p (v q) -> p v q", q=NB * C)
                            [:, :, (1 + dd) * C:(2 + dd) * C].unsqueeze(3),
                        sqs[:].rearrange("p (v l c) -> p v c l", l=NL, c=C)
                            [:, :, :, ls:ls + lcnt],
                        mybir.AxisListType.X, OP.add)

            def chi_group(gi, Bg):
                Bgb = wp.tile([EBLK, NVG * NB * C], FP16, name='Bgb', tag='Bgb')
                nc.any.tensor_copy(out=Bgb[:], in_=Bg)
                for wl in range(GW):
                    w = gi * GW + wl
                    psC = pp.tile([WIN, C], F32, name='psC', tag='seg')
                    first = True
                    for g in range(2):
                        for k in range(NB):
                            nc.tensor.matmul(
                                out=psC[:],
                                lhsT=sb['wbd_chi'][:, (g * NB + k) * WIN:
                                                   (g * NB + k + 1) * WIN],
                                rhs=Bgb[:, (wl * 2 + g) * NB * C + k * C:
                                           (wl * 2 + g) * NB * C + (k + 1) * C],
                                start=first, stop=(g == 1 and k == NB - 1),
                                skip_group_check=True)
                            first = False
                    nc.any.tensor_copy(out=chiS[:, w * C:(w + 1) * C], in_=psC[:])

            def mem_window(w):
                psMem = pp.tile([EBLK, GLC], F32, name='psMem', tag='memt')
                for gout in range(2):
                    for dd, (ls, lcnt) in enumerate(GRP_SLICES):
                        osl = slice(gout * LC + ls * C, gout * LC + (ls + lcnt) * C)
                        for gin in range(2):
                            wcol = ((gout * 4 + dd) * 2 + gin) * EBLK
                            csl = slice(w * GLC + gin * LC + ls * C,
                                        w * GLC + gin * LC + (ls + lcnt) * C)
                            nc.tensor.matmul(
                                out=psMem[:, osl],
                                lhsT=sb['wbd_mem'][:, wcol:wcol + EBLK],
                                rhs=A_sb[:, csl],
                                start=(gin == 0), stop=(gin == 1),
                                skip_group_check=True)
                nc.any.tensor_copy(out=memS[:, w * GLC:(w + 1) * GLC], in_=psMem[:])

            for w in range(WT):
                psA0 = pp.tile([EBLK, GLC], F32, name='psA0', tag='seg')
                for g in range(2):
                    for bi in range(NBW):
                        blk = w * NBW + bi
                        nc.tensor.matmul(
                            out=psA0[:, g * LC:(g + 1) * LC],
                            lhsT=lhsT1[g][:, blk * EBLK:(blk + 1) * EBLK],
                            rhs=P[:, blk * LC:(blk + 1) * LC],
                            start=(bi == 0), stop=(bi == NBW - 1),
                            skip_group_check=True)
                A0s = wp.tile([EBLK, GLC], FP16, name='A0s', tag='A0s')
                nc.any.tensor_copy(out=A0s[:], in_=psA0[:])
                psA = pp.tile([EBLK, GLC], F32, name='psA', tag='acc')
                for gout in range(2):
                    for dd, (ls, lcnt) in enumerate(GRP_SLICES):
                        osl = slice(gout * LC + ls * C, gout * LC + (ls + lcnt) * C)
                        for gin in range(2):
                            wcol = ((gout * 4 + dd) * 2 + gin) * EBLK
                            csl = slice(gin * LC + ls * C, gin * LC + (ls + lcnt) * C)
                            nc.tensor.matmul(
                                out=psA[:, osl],
                                lhsT=sb['wbd_rad'][:, wcol:wcol + EBLK],
                                rhs=A0s[:, csl],
                                start=(gin == 0), stop=(gin == 1),
                                skip_group_check=True)
                nc.any.tensor_copy(out=A_sb[:, w * GLC:(w + 1) * GLC], in_=psA[:])

            # own A rows are complete: PE parks all psMem results while the
            # DVE/Pool sym groups below run concurrently
            for wm in range(NWINC):
                mem_window(wm)

            for gi in range(NG):
                own_grp = gi < NWINC // GW
                if own_grp:
                    Bg = B0s[:, gi * NVG * NB * C:(gi + 1) * NVG * NB * C]
                else:
                    Bgt = wp.tile([EBLK, NVG * NB * C], F32, name='Bgt', tag='Bgt')
                    Bg = Bgt[:]
                sym_group(A_sb[:, gi * GW * GLC:(gi + 1) * GW * GLC], Bg,
                          pool_eng=(gi % 3 == 2))
                chi_group(gi, Bg)
            nc.any.tensor_copy(out=chiSb[:], in_=chiS[:])

            # ---- node table -> local DRAM (fp16); no collective ----
            T_local = dp.tile([WT * WIN, TW], FP16, name='T_local')
            for x in range(4):
                nc.sync.dma_start(
                    out=T_local[:, x * GLC:(x + 1) * GLC]
                        .rearrange("(w n) q -> n w q", w=WT),
                    in_=A_sb[x * WIN:(x + 1) * WIN, :]
                        .rearrange("n (w q) -> n w q", w=WT))
            nc.sync.dma_start(
                out=T_local[:, RB * LC:RB * LC + C]
                    .rearrange("(w n) c -> n w c", w=WT),
                in_=chiSb[:].rearrange("n (w c) -> n w c", w=WT))

            # ---- pass 2 (own windows only) ----
            # issue all gathers up front: they only depend on the T_local
            # write, so the DMA engines prefetch while sym/chi still run
            ags_all = []
            for blk in range(NBLK_OWN):
                ag = gp.tile([EBLK, TW], FP16, name='ag', tag='ag')
                nc.gpsimd.indirect_dma_start(
                    out=ag[:], out_offset=None, in_=T_local[:],
                    in_offset=bass.IndirectOffsetOnAxis(
                        ap=esrow_s[:, blk:blk + 1], axis=0))
                ags_all.append(ag)
            Anew = ctile('Anew', [EBLK, NWINC * GLC], FP16)
            for w in range(NWINC):
                ags = []
                P2s = []
                for bi in range(NBW):
                    blk = w * NBW + bi
                    ag = ags_all[blk]
                    ags.append(ag)
                    P2 = wp.tile([EBLK, LC], FP16, name='P2', tag='P2')
                    TT(out=P2[:].rearrange("p (l c) -> p l c", c=C),
                        in0=P[:, blk * LC:(blk + 1) * LC].rearrange("p (l c) -> p l c", c=C),
                        in1=ag[:, RB * LC:RB * LC + C].unsqueeze(1)
                            .broadcast_to([EBLK, NL, C]),
                        op=OP.mult)
                    P2s.append(P2)
                psB0 = pp.tile([EBLK, GLC], F32, name='psB0', tag='seg')
                for g in range(2):
                    for bi in range(NBW):
                        blk = w * NBW + bi
                        nc.tensor.matmul(
                            out=psB0[:, g * LC:(g + 1) * LC],
                            lhsT=lhsT1[g][:, blk * EBLK:(blk + 1) * EBLK],
                            rhs=P2s[bi][:],
                            start=(bi == 0), stop=(bi == NBW - 1),
                            skip_group_check=True)
                Ab0 = wp.tile([EBLK, GLC], FP16, name='Ab0', tag='Ab0')
                nc.any.tensor_copy(out=Ab0[:], in_=psB0[:])
                psAb = pp.tile([EBLK, GLC], F32, name='psAb', tag='acc')
                for gout in range(2):
                    for dd, (ls, lcnt) in enumerate(GRP_SLICES):
                        osl = slice(gout * LC + ls * C, gout * LC + (ls + lcnt) * C)
                        for gin in range(2):
                            wcol = ((gout * 4 + dd) * 2 + gin) * EBLK
                            csl = slice(gin * LC + ls * C, gin * LC + (ls + lcnt) * C)
                            nc.tensor.matmul(
                                out=psAb[:, osl],
                                lhsT=sb['wbd_radmp'][:, wcol:wcol + EBLK],
                                rhs=Ab0[:, csl],
                                start=(gin == 0), stop=(gin == 1),
                                skip_group_check=True)
                psAr = pp.tile([EBLK, GLC], F32, name='psAr', tag='ar')
                for b_ in range(RB):
                    g = b_ // 4; xq = b_ % 4
                    scol = xq * 2 + g          # T col-slice index for b_
                    for bi in range(NBW):
                        blk = w * NBW + bi
                        nc.tensor.matmul(
                            out=psAr[xq * WIN:(xq + 1) * WIN, g * LC:(g + 1) * LC],
                            lhsT=lhsT_ar[g][:, blk * EBLK + xq * WIN:
                                            blk * EBLK + (xq + 1) * WIN],
                            rhs=ags[bi][:, scol * LC:(scol + 1) * LC],
                            start=(bi == 0), stop=(bi == NBW - 1),
                            skip_group_check=True,
                            tile_position=(0, xq * WIN))
                comb = wp.tile([EBLK, GLC], F32, name='comb', tag='comb')
                nc.scalar.copy(out=comb[:], in_=psAb[:])
                TT(out=comb[:], in0=comb[:], in1=psAr[:], op=OP.add)
                TT(out=Anew[:, w * GLC:(w + 1) * GLC], in0=comb[:],
                   in1=memS[:, w * GLC:(w + 1) * GLC], op=OP.add)

            # ---- B1 symmetrize (own windows, f32) ----
            B1s = ctile('B1s', [EBLK, NWINC * 2 * NB * C])
            for gi in range(NWINC // GW):
                sym_group(Anew[:, gi * GW * GLC:(gi + 1) * GW * GLC],
                          B1s[:, gi * NVG * NB * C:(gi + 1) * NVG * NB * C],
                          pool_eng=(gi % 2 == 1))

            # ---- output: [t2, (w,g)=16, 45] ----
            half = NWINC * 2 * NB * C
            nc.sync.dma_start(out=outB[:, 0:half], in_=B0s[:])
            nc.sync.dma_start(out=outB[:, half:2 * half], in_=B1s[:])

    nc.compile()
    return nc


_CACHE = {}


def kernel(**inputs) -> np.ndarray:
    return _kernel_impl(inputs)[0]


def _kernel_impl(inputs, trace=False):
    from concourse.bass_utils import run_bass_kernel_spmd

    packed, slot_of_node, nh = _host_prep(inputs)

    key = ('nc', nh)
    if key not in _CACHE:
        _CACHE[key] = _build_program(nh)
    nc = _CACHE[key]

    in_maps = [dict(p) for p in packed]

    res = run_bass_kernel_spmd(nc, in_maps, core_ids=list(range(N_CORES)),
                               trace=trace)

    feats_slots = np.zeros((NSLOT, RB, NB, C, 2), np.float32)
    for ci in range(N_CORES):
        arr = res.results[ci]['outB'].reshape(4, WIN, 2, NWINC, 2, NB, C)
        arr = np.transpose(arr, (3, 1, 4, 0, 5, 6, 2))
        feats_slots[ci * NWINC * WIN:(ci + 1) * NWINC * WIN] = \
            arr.reshape(NWINC * WIN, RB, NB, C, 2)
    return feats_slots[slot_of_node], res


if __name__ == '__main__':
    import pickle, os
    if os.path.exists('/tmp/inputs.pkl'):
        inputs = pickle.load(open('/tmp/inputs.pkl', 'rb'))
    else:
        import reference as Rf
        inputs = {k: np.asarray(v) for k, v in Rf.setup_inputs().items()}
        pickle.dump(inputs, open('/tmp/inputs.pkl', 'wb'))
    out = kernel(**inputs)
    print("kernel out", out.shape, out.dtype, float(np.abs(out).max()))
    if os.path.exists('/tmp/expected.npy'):
        exp = np.load('/tmp/expected.npy')
        err = np.abs(out - exp).max()
        print("max abs err vs expected:", err, "rel:", err / np.abs(exp).max())


# revision 51
# speedup vs baseline: 1.0657x; 1.0185x over previous
"""Trainium2 Bass kernel for nn_Cace_74569222193773 (CACE GNN message passing).

Strategy (8 NeuronCores, SPMD, one program shape + per-core data):
  * Host: drop edges with r >= cutoff (fcut = 0 there), assign nodes to 64
    edge-balanced global windows of <=32 nodes (8 "own" windows per core).
  * HALO REPLICATION instead of a collective: each core additionally
    recomputes pass-1 A for the sender nodes of its own edges that live on
    other cores.  Those halo nodes are repacked into private halo windows
    (<=32 nodes, <=256 in-edges each, edge-balanced); the core processes
    own + halo windows in pass 1, writes the node table T = [A row | chi]
    (fp16) to its own DRAM, and pass 2 gathers sender rows locally.
    No inter-core communication at all.
  * All node-feature tensors live in a "half" layout: partition p = x*32+n
    with x = (r or b) mod 4, plus a half index g = (r or b) // 4 in the
    free dimension, so every PE matmul output starts at a 32-aligned
    partition base.
  * Pass 1 (per core): edge geometry + bessel + cutoff + angular on
    DVE/Pool/ACT in edge-major layout [128 partitions = edges]; per-window
    segment-sum via PE matmuls (fp16 operands, fp32 PSUM) with
    lhsT = onehot32 (x) radf-half, rhs = P = ang (x) enc; radial transform
    via block-diag W (x) I32 fp16 matmuls; symmetrize + chi per window
    group (own windows in fp32 A, halo windows from the fp16 copy -- halo
    B0 only feeds chi).
  * Pass 2 (own windows only): indirect-DMA gather of T[send] (fp16 rows),
    A_ar via per-b matmuls (lhsT = onehot (x) fr slice), A_bchi via the
    pass-1 segment-sum machinery with rhs P * chi_send, mem via
    W_mem (x) I32; combine (fp32), symmetrize -> B1.

kernel() takes FULL unsharded inputs and returns the FULL [2000,8,5,9,2]
float32 output; all sharding happens inside.
"""
import heapq
from math import factorial

import numpy as np

# ---- static problem config (mirrors the reference) ----
MAX_L = 3; N_RBF = 8; RB = 8; K = 3
CUTOFF = 5.5
N_NODES = 2000
MP_NORM = 1.0 / np.sqrt(25.0)
C = K * K                      # 9
NB = 1 + (MAX_L + 1)           # 5

def _lxlylz(max_l):
    out = []
    for l in range(max_l + 1):
        for lx in range(l, -1, -1):
            for ly in range(l - lx, -1, -1):
                out.append((lx, ly, l - lx - ly))
    return out

L_LIST = _lxlylz(MAX_L); NL = len(L_LIST)                       # 20
LX = np.array([t[0] for t in L_LIST]); LY = np.array([t[1] for t in L_LIST])
LZ = np.array([t[2] for t in L_LIST]); DEGS = LX + LY + LZ
MULTI = np.array([factorial(int(d)) / (factorial(int(a)) * factorial(int(b)) * factorial(int(c)))
                  for a, b, c, d in zip(LX, LY, LZ, DEGS)], dtype=np.float32)
GRP_SLICES = []                 # (l_start, l_count) per degree; DEGS is sorted
for d in range(MAX_L + 1):
    idx = np.where(DEGS == d)[0]
    GRP_SLICES.append((int(idx[0]), int(len(idx))))

# ---- sharding geometry ----
N_CORES = 8
WIN = 32                        # nodes per window
NWINC = 8                       # own windows per core
NWIN = N_CORES * NWINC          # 64
NSLOT = NWIN * WIN              # 2048 own-node slots globally
EBLK = 128                      # edges per block (partition dim)
NBW = 2                         # blocks per window
NBLK_OWN = NWINC * NBW          # 16 own blocks per core
LC = NL * C                     # 180
GLC = 2 * LC                    # 360 = both halves
TW = RB * LC + WIN              # table row width 1472 (1440 A + 9 chi + pad)

_RSCL = np.sqrt(2.0 / CUTOFF)

F32_FIELDS = ['exyz_s', 'exyz_r', 'eemb_s', 'eemb_r', 'enloc',
              'iota32', 'multi_l', 'war_mp', 'nvec']
FP16_FIELDS = ['wbd_rad', 'wbd_radmp', 'wbd_mem', 'wbd_chi']


def _field_layout(nbt):
    """Column layout of the packed f32 / fp16 input tensors for nbt blocks."""
    fw = dict(exyz_s=3 * nbt, exyz_r=3 * nbt, eemb_s=3 * nbt, eemb_r=3 * nbt,
              enloc=nbt, iota32=WIN, multi_l=NL, war_mp=64, nvec=N_RBF,
              wbd_rad=2048, wbd_radmp=2048, wbd_mem=2048, wbd_chi=2 * NB * WIN)
    off = {}
    o = 0
    for f in F32_FIELDS:
        off[f] = o; o += fw[f]
    totf = o
    o = 0
    for f in FP16_FIELDS:
        off[f] = o; o += fw[f]
    return fw, off, totf, o


def _pack_windows(node_list, deg, nwin_cap):
    """Balanced assignment of node_list into windows (<=WIN nodes each,
    edge-load balanced).  Grows window count until max load <= NBW*EBLK.
    Returns (win_of, pos_of, n_windows)."""
    nodes = sorted(node_list, key=lambda n: -deg[n])
    nwin = max(1, (len(nodes) + WIN - 1) // WIN)
    while True:
        win_cnt = np.zeros(nwin, np.int64); win_load = np.zeros(nwin, np.int64)
        win_of = {}; pos_of = {}
        heap = [(0, w) for w in range(nwin)]
        heapq.heapify(heap)
        ok = True
        for nd in nodes:
            popped = []
            while True:
                load, w = heapq.heappop(heap)
                if win_cnt[w] < WIN:
                    break
                popped.append((load, w))
            for it in popped:
                heapq.heappush(heap, it)
            win_of[nd] = w; pos_of[nd] = int(win_cnt[w])
            win_cnt[w] += 1; win_load[w] += deg[nd]
            heapq.heappush(heap, (int(win_load[w]), w))
        if win_load.max(initial=0) <= NBW * EBLK:
            return win_of, pos_of, nwin
        nwin += 1
        if nwin > nwin_cap:
            raise RuntimeError("halo window packing overflow")


def _host_prep(inputs):
    pos = np.asarray(inputs['positions'], np.float32)
    shifts = np.asarray(inputs['shifts'], np.float32)
    W_embed = np.asarray(inputs['W_embed'], np.float32)
    species = np.asarray(inputs['species'])
    ei = np.asarray(inputs['edge_index'])
    send, recv = ei[0], ei[1]

    vec = (pos[recv] + shifts - pos[send]).astype(np.float64)
    r = np.sqrt((vec * vec).sum(-1))
    keep = np.where(r < CUTOFF)[0]
    deg = np.bincount(recv[keep], minlength=N_NODES)

    # balanced node->global-window assignment (own windows)
    order = np.argsort(-deg, kind='stable')
    win_cnt = np.zeros(NWIN, np.int64); win_load = np.zeros(NWIN, np.int64)
    win_of_node = np.zeros(N_NODES, np.int64); pos_in_win = np.zeros(N_NODES, np.int64)
    heap = [(0, w) for w in range(NWIN)]
    heapq.heapify(heap)
    for nd in order:
        popped = []
        while True:
            load, w = heapq.heappop(heap)
            if win_cnt[w] < WIN:
                break
            popped.append((load, w))
        for it in popped:
            heapq.heappush(heap, it)
        win_of_node[nd] = w; pos_in_win[nd] = win_cnt[w]
        win_cnt[w] += 1; win_load[w] += deg[nd]
        heapq.heappush(heap, (win_load[w], w))
    if win_load.max() > NBW * EBLK:
        raise RuntimeError(f"window overflow: {win_load.max()} > {NBW * EBLK}")

    slot_of_node = win_of_node * WIN + pos_in_win
    emb = W_embed[species]                       # [N, K]

    ks, kr = send[keep], recv[keep]
    in_edges = [[] for _ in range(N_NODES)]      # node -> kept edge ids
    for i, e in enumerate(keep):
        in_edges[kr[i]].append(e)

    # per-core halo structure
    core_halo = []
    nh_list = []
    for ci in range(N_CORES):
        own_w = set(range(ci * NWINC, (ci + 1) * NWINC))
        own_eids = []
        for w in sorted(own_w):
            for nd in np.where(win_of_node == w)[0]:
                own_eids.extend(in_edges[nd])
        senders = set(send[own_eids].tolist()) if own_eids else set()
        halo = [s for s in senders if win_of_node[s] not in own_w]
        hwin_of, hpos_of, nh = _pack_windows(halo, deg, 64)
        core_halo.append((own_w, hwin_of, hpos_of, nh))
        nh_list.append(nh)
    NH = max(nh_list)
    NH = ((NH + 3) // 4) * 4      # pad so WT = 8 + NH is a multiple of 4
    WT = NWINC + NH
    NBT = NBW * WT
    EPAD = NBT * EBLK

    cores = []
    for ci in range(N_CORES):
        own_w, hwin_of, hpos_of, nh = core_halo[ci]
        e_xyz_s = np.zeros((EPAD, 3), np.float32)
        e_xyz_r = np.zeros((EPAD, 3), np.float32)
        e_emb_s = np.zeros((EPAD, K), np.float32)
        e_emb_r = np.zeros((EPAD, K), np.float32)
        e_nloc = np.full((EPAD,), -1.0, np.float32)
        e_srow = np.zeros((EPAD,), np.int32)
        e_xyz_r[:, 0] = 1.0                      # pads: r = 1, finite math

        def srow_of(s):
            w = win_of_node[s]
            if w in own_w:
                return (w - ci * NWINC) * WIN + pos_in_win[s]
            return (NWINC + hwin_of[s]) * WIN + hpos_of[s]

        # local window wl in [0, WT): own first, then halo
        def fill_window(wl, node_ids, pos_of, need_srow):
            base = wl * NBW * EBLK
            eids = []
            for nd in node_ids:
                eids.extend(in_edges[nd])
            eids = np.array(eids, dtype=np.int64)
            cnt = len(eids)
            if cnt == 0:
                return
            if cnt > NBW * EBLK:
                raise RuntimeError("window edge overflow")
            sl = slice(base, base + cnt)
            e_xyz_s[sl] = pos[send[eids]]
            e_xyz_r[sl] = pos[recv[eids]] + shifts[eids]
            e_emb_s[sl] = emb[send[eids]]
            e_emb_r[sl] = emb[recv[eids]]
            e_nloc[sl] = np.array([pos_of[n] for n in recv[eids]], np.float32)
            if need_srow:
                e_srow[sl] = np.array([srow_of(s) for s in send[eids]], np.int32)

        for wl in range(NWINC):
            w = ci * NWINC + wl
            nds = np.where(win_of_node == w)[0]
            fill_window(wl, nds, {int(n): int(pos_in_win[n]) for n in nds}, True)
        halo_by_win = [[] for _ in range(nh)]
        for s, hw in hwin_of.items():
            halo_by_win[hw].append(s)
        for hw in range(nh):
            fill_window(NWINC + hw, halo_by_win[hw],
                        {int(n): int(hpos_of[n]) for n in halo_by_win[hw]}, False)

        def dev(x):
            if x.ndim == 1:
                return np.ascontiguousarray(x.reshape(NBT, EBLK).T)
            return np.ascontiguousarray(np.transpose(x.reshape(NBT, EBLK, -1), (1, 0, 2)))

        def axmajor(x3):
            d = dev(x3)                                  # [128, NBT, 3]
            return np.ascontiguousarray(np.transpose(d, (0, 2, 1)).reshape(EBLK, 3 * NBT))

        cores.append(dict(
            exyz_s=axmajor(e_xyz_s), exyz_r=axmajor(e_xyz_r),
            eemb_s=axmajor(e_emb_s), eemb_r=axmajor(e_emb_r),
            enloc=np.ascontiguousarray(dev(e_nloc)),
            esrow=np.ascontiguousarray(dev(e_srow)[:, :NBLK_OWN]),
        ))

    Wr = np.asarray(inputs['W_radial'], np.float32)   # [4(deg), 8(r), 8(b)]
    Wm = np.asarray(inputs['W_mem'], np.float32)
    Wc = np.asarray(inputs['W_chi'], np.float32)      # [8(b), 5(k)]
    Wa = np.asarray(inputs['W_ar'], np.float32)       # [8(r), 8(b)]
    I32 = np.eye(WIN, dtype=np.float32)

    def bd(W):
        cols = []
        for gout in range(2):
            for d in range(4):
                for gin in range(2):
                    cols.append(np.kron(W[d, gin * 4:gin * 4 + 4, gout * 4:gout * 4 + 4], I32))
        return np.concatenate(cols, axis=1)          # [128, 2048]

    wchi_cols = []
    for g in range(2):
        for k in range(NB):
            wchi_cols.append(np.kron(Wc[g * 4:g * 4 + 4, k:k + 1], I32))   # [128, 32]
    consts_f = dict(
        war_mp=np.tile((Wa * MP_NORM).reshape(1, 64), (EBLK, 1)),
        multi_l=np.tile(np.sqrt(MULTI).reshape(1, NL), (EBLK, 1)),
        iota32=np.tile(np.arange(WIN, dtype=np.float32).reshape(1, WIN), (EBLK, 1)),
        nvec=np.tile((np.arange(1, N_RBF + 1, dtype=np.float32) / CUTOFF).reshape(1, N_RBF),
                     (EBLK, 1)),
    )
    consts_b = dict(
        wbd_rad=bd(Wr),
        wbd_radmp=bd(Wr * MP_NORM),
        wbd_mem=bd(Wm),
        wbd_chi=np.concatenate(wchi_cols, axis=1),                   # [128, 320]
    )
    packed = []
    for ci in range(N_CORES):
        cols_f = [cores[ci][nm] for nm in
                  ['exyz_s', 'exyz_r', 'eemb_s', 'eemb_r', 'enloc']]
        cols_f += [consts_f[nm] for nm in ['iota32', 'multi_l', 'war_mp', 'nvec']]
        edf = np.ascontiguousarray(np.concatenate(cols_f, axis=1), np.float32)
        edb = np.ascontiguousarray(
            np.concatenate([consts_b[nm] for nm in FP16_FIELDS], axis=1)
        ).astype(np.float16)
        packed.append(dict(edf=edf, edb=edb, esrow=cores[ci]['esrow']))
    return packed, slot_of_node, NH


def _build_program(nh, debug=False):
    import concourse.bass as bass
    import concourse.mybir as mybir
    from concourse import bacc
    from concourse.tile import TileContext

    F32 = mybir.dt.float32
    FP16 = mybir.dt.float16
    AF = mybir.ActivationFunctionType
    OP = mybir.AluOpType

    WT = NWINC + nh
    NBT = NBW * WT
    FIELD_W, FIELD_OFF, TOTW_F, TOTW_B = _field_layout(NBT)

    nc = bacc.Bacc("TRN2", target_bir_lowering=False, debug=False,
                   num_devices=N_CORES)

    edf_d = nc.dram_tensor('edf', [EBLK, TOTW_F], F32, kind="ExternalInput")
    edb_d = nc.dram_tensor('edb', [EBLK, TOTW_B], FP16, kind="ExternalInput")
    esrow_d = nc.dram_tensor('esrow', [EBLK, NBLK_OWN], mybir.dt.int32,
                             kind="ExternalInput")
    outB = nc.dram_tensor('outB', [EBLK, 2 * NWINC * 2 * NB * C], F32,
                          kind="ExternalOutput")

    with TileContext(nc) as tc:
        with (tc.tile_pool(name="const", bufs=1) as cp,
              tc.tile_pool(name="work", bufs=2) as wp,
              tc.tile_pool(name="gat", bufs=9) as gp,
              tc.tile_pool(name="psum", bufs=2, space="PSUM") as pp,
              tc.tile_pool(name="dram", bufs=1, space="DRAM") as dp):

            bigf = cp.tile([EBLK, TOTW_F], F32, name='bigf', tag='bigf')
            nc.sync.dma_start(out=bigf[:], in_=edf_d[:])
            bigb = cp.tile([EBLK, TOTW_B], FP16, name='bigb', tag='bigb')
            nc.sync.dma_start(out=bigb[:], in_=edb_d[:])
            esrow_s = cp.tile([EBLK, NBLK_OWN], mybir.dt.int32,
                              name='esrow_s', tag='esrow_s')
            nc.sync.dma_start(out=esrow_s[:], in_=esrow_d[:])

            class _S:
                def __init__(self, tile):
                    self.tile = tile
                def __getitem__(self, nm):
                    off = FIELD_OFF[nm]
                    return self.tile[:, off:off + FIELD_W[nm]]
            s = _S(bigf)
            sb = _S(bigb)

            def ctile(tag, shape, dtype=F32):
                return cp.tile(shape, dtype, name=tag, tag=tag)

            TT = nc.vector.tensor_tensor
            TTP = nc.gpsimd.tensor_tensor
            TS = nc.vector.tensor_scalar

            # ---- geometry, edge-major [128, a*NBT+blk] ----
            vd = ctile('vd', [EBLK, 3 * NBT])
            TT(out=vd[:], in0=s['exyz_r'][:], in1=s['exyz_s'][:], op=OP.subtract)
            sq = ctile('sq', [EBLK, 3 * NBT])
            TT(out=sq[:], in0=vd[:], in1=vd[:], op=OP.mult)
            r2 = ctile('r2', [EBLK, NBT])
            TT(out=r2[:], in0=sq[:, 0:NBT], in1=sq[:, NBT:2 * NBT], op=OP.add)
            TT(out=r2[:], in0=r2[:], in1=sq[:, 2 * NBT:3 * NBT], op=OP.add)
            rr = ctile('rr', [EBLK, NBT])
            nc.scalar.activation(out=rr[:], in_=r2[:], func=AF.Sqrt)
            rpe = ctile('rpe', [EBLK, NBT])
            TS(out=rpe[:], in0=rr[:], scalar1=1e-9, scalar2=None, op0=OP.add)
            rinv = ctile('rinv', [EBLK, NBT])
            nc.vector.reciprocal(out=rinv[:], in_=rpe[:])
            uv = ctile('uv', [EBLK, 3 * NBT])
            TT(out=uv[:].rearrange("p (a b) -> p a b", a=3),
               in0=vd[:].rearrange("p (a b) -> p a b", a=3),
               in1=rinv[:].unsqueeze(1).broadcast_to([EBLK, 3, NBT]), op=OP.mult)

            # bessel: rad[r, blk] = sin((n+1) * pi/c * r) * (sqrt(2/c) * rinv)
            rscl = ctile('rscl', [EBLK, NBT])
            TS(out=rscl[:], in0=rinv[:], scalar1=float(-_RSCL), scalar2=None, op0=OP.mult)
            radp = ctile('radp', [EBLK, N_RBF * NBT])
            marg = ctile('marg', [EBLK, N_RBF * NBT])
            TT(out=marg[:].rearrange("p (r b) -> p r b", r=N_RBF),
               in0=rr[:].unsqueeze(1).broadcast_to([EBLK, N_RBF, NBT]),
               in1=s['nvec'][:].unsqueeze(2).broadcast_to([EBLK, N_RBF, NBT]),
               op=OP.mult)
            mtmp = ctile('mtmp', [EBLK, N_RBF * NBT])
            TS(out=mtmp[:], in0=marg[:], scalar1=4.0, scalar2=4.0,
               op0=OP.is_ge, op1=OP.mult)
            TT(out=marg[:], in0=marg[:], in1=mtmp[:], op=OP.subtract)
            TS(out=mtmp[:], in0=marg[:], scalar1=2.0, scalar2=2.0,
               op0=OP.is_ge, op1=OP.mult)
            TT(out=marg[:], in0=marg[:], in1=mtmp[:], op=OP.subtract)
            biaspi = ctile('biaspi', [EBLK, 1])
            nc.vector.memset(biaspi[:], float(-np.pi))
            nc.scalar.activation(out=radp[:], in_=marg[:], func=AF.Sin,
                                 scale=float(np.pi), bias=biaspi[:])
            TT(out=radp[:].rearrange("p (r b) -> p r b", r=N_RBF),
               in0=radp[:].rearrange("p (r b) -> p r b", r=N_RBF),
               in1=rscl[:].unsqueeze(1).broadcast_to([EBLK, N_RBF, NBT]), op=OP.mult)

            # poly cutoff (p=6); host guarantees u<1 (Pool engine)
            uu = ctile('uu', [EBLK, NBT])
            TS(out=uu[:], in0=rr[:], scalar1=float(1.0 / CUTOFF), scalar2=None, op0=OP.mult)
            u3 = ctile('u3', [EBLK, NBT])
            TTP(out=u3[:], in0=uu[:], in1=uu[:], op=OP.mult)
            TTP(out=u3[:], in0=u3[:], in1=uu[:], op=OP.mult)
            u6 = ctile('u6', [EBLK, NBT]); TTP(out=u6[:], in0=u3[:], in1=u3[:], op=OP.mult)
            u7 = ctile('u7', [EBLK, NBT]); TTP(out=u7[:], in0=u6[:], in1=uu[:], op=OP.mult)
            u8 = ctile('u8', [EBLK, NBT]); TTP(out=u8[:], in0=u7[:], in1=uu[:], op=OP.mult)
            fc = ctile('fc', [EBLK, NBT])
            nc.gpsimd.tensor_scalar(out=fc[:], in0=u6[:], scalar1=-28.0, scalar2=1.0,
                                    op0=OP.mult, op1=OP.add)
            t7 = ctile('t7', [EBLK, NBT])
            nc.gpsimd.tensor_scalar(out=t7[:], in0=u7[:], scalar1=48.0, scalar2=None,
                                    op0=OP.mult)
            TTP(out=fc[:], in0=fc[:], in1=t7[:], op=OP.add)
            nc.gpsimd.tensor_scalar(out=t7[:], in0=u8[:], scalar1=-21.0, scalar2=None,
                                    op0=OP.mult)
            TTP(out=fc[:], in0=fc[:], in1=t7[:], op=OP.add)

            radf = ctile('radf', [EBLK, N_RBF * NBT])
            TT(out=radf[:].rearrange("p (r b) -> p r b", r=N_RBF),
               in0=radp[:].rearrange("p (r b) -> p r b", r=N_RBF),
               in1=fc[:].unsqueeze(1).broadcast_to([EBLK, N_RBF, NBT]), op=OP.mult)

            # onehot [blk, n32] (DVE: Pool lacks is_equal)
            onehot = ctile('onehot', [EBLK, NBT * WIN])
            TT(out=onehot[:].rearrange("p (b n) -> p b n", b=NBT),
               in0=s['enloc'][:].unsqueeze(2).broadcast_to([EBLK, NBT, WIN]),
               in1=s['iota32'][:].unsqueeze(1).broadcast_to([EBLK, NBT, WIN]),
               op=OP.is_equal)

            # enc [blk, ks, kr] (Pool)
            enc = ctile('enc', [EBLK, NBT * C])
            TTP(out=enc[:].rearrange("p (b i j) -> p b i j", i=K, j=K),
                in0=s['eemb_s'][:].rearrange("p (k b) -> p b k", k=K).unsqueeze(3)
                    .broadcast_to([EBLK, NBT, K, K]),
                in1=s['eemb_r'][:].rearrange("p (k b) -> p b k", k=K).unsqueeze(2)
                    .broadcast_to([EBLK, NBT, K, K]),
                op=OP.mult)

            # angular monomials [l, blk]
            ones = ctile('ones', [EBLK, NBT])
            nc.vector.memset(ones[:], 1.0)
            x2 = ctile('x2', [EBLK, 3 * NBT])
            TT(out=x2[:], in0=uv[:], in1=uv[:], op=OP.mult)
            x3 = ctile('x3', [EBLK, 3 * NBT])
            TT(out=x3[:], in0=x2[:], in1=uv[:], op=OP.mult)

            def pow_plane(axis, p_):
                if p_ == 1:
                    return uv[:, axis * NBT:(axis + 1) * NBT]
                if p_ == 2:
                    return x2[:, axis * NBT:(axis + 1) * NBT]
                return x3[:, axis * NBT:(axis + 1) * NBT]

            ang = ctile('ang', [EBLK, NL * NBT])
            for l in range(NL):
                facs = [pow_plane(a, pw) for a, pw in enumerate((LX[l], LY[l], LZ[l])) if pw > 0]
                dst = ang[:, l * NBT:(l + 1) * NBT]
                if len(facs) == 0:
                    nc.scalar.copy(out=dst, in_=ones[:])
                elif len(facs) == 1:
                    nc.scalar.copy(out=dst, in_=facs[0])
                elif len(facs) == 2:
                    TT(out=dst, in0=facs[0], in1=facs[1], op=OP.mult)
                else:
                    TT(out=dst, in0=facs[0], in1=facs[1], op=OP.mult)
                    TT(out=dst, in0=dst, in1=facs[2], op=OP.mult)

            # fold sqrt(MULTI_l) into ang: every downstream tensor (A, table,
            # mem, A_ar, A_bchi) is then consistently per-l scaled, so the
            # symmetrize b2 sum needs no MULTI weighting (b1 has MULTI_0 = 1)
            TT(out=ang[:].rearrange("p (l b) -> p l b", l=NL),
               in0=ang[:].rearrange("p (l b) -> p l b", l=NL),
               in1=s['multi_l'][:].unsqueeze(2).broadcast_to([EBLK, NL, NBT]),
               op=OP.mult)

            # P = ang (x) enc : [blk, l, c] in fp16 (split DVE / Pool by half)
            P = ctile('P', [EBLK, NBT * LC], FP16)
            HB = NBT // 8
            for half, eng in ((0, TT), (1, TTP)):
                blo = half * HB; bcnt = (NBT - HB) if half else HB
                eng(out=P[:, blo * LC:(blo + bcnt) * LC]
                        .rearrange("p (b l c) -> p b l c", l=NL, c=C),
                    in0=ang[:].rearrange("p (l b) -> p b l", l=NL)[:, blo:blo + bcnt]
                        .unsqueeze(3).broadcast_to([EBLK, bcnt, NL, C]),
                    in1=enc[:].rearrange("p (b c) -> p b c", c=C)[:, blo:blo + bcnt]
                        .unsqueeze(2).broadcast_to([EBLK, bcnt, NL, C]),
                    op=OP.mult)

            # lhsT1_g = radf-half (x) onehot : [blk, r4, n32] (fp16; DVE/Pool split)
            lhsT1 = []
            for g in range(2):
                lt = ctile(f'lhsT1_{g}', [EBLK, NBT * EBLK], FP16)
                eng = TT
                eng(out=lt[:].rearrange("p (b r n) -> p b r n", r=4, n=WIN),
                    in0=radf[:].rearrange("p (r b) -> p b r", r=N_RBF)
                        [:, :, g * 4:(g + 1) * 4].unsqueeze(3)
                        .broadcast_to([EBLK, NBT, 4, WIN]),
                    in1=onehot[:].rearrange("p (b n) -> p b n", b=NBT).unsqueeze(2)
                        .broadcast_to([EBLK, NBT, 4, WIN]),
                    op=OP.mult)
                lhsT1.append(lt)

            # fr = (radf @ W_ar) * MP_NORM : own blocks only [blk, b8] (Pool)
            frA = ctile('frA', [EBLK, NBLK_OWN * N_RBF])
            frB = ctile('frB', [EBLK, NBLK_OWN * N_RBF])
            frt = ctile('frt', [EBLK, NBLK_OWN * N_RBF])
            for r_ in range(N_RBF):
                radv = radf[:, r_ * NBT:r_ * NBT + NBLK_OWN].unsqueeze(2) \
                    .broadcast_to([EBLK, NBLK_OWN, N_RBF])
                warv = s['war_mp'][:, r_ * N_RBF:(r_ + 1) * N_RBF].unsqueeze(1) \
                    .broadcast_to([EBLK, NBLK_OWN, N_RBF])
                if r_ == 0:
                    TTP(out=frA[:].rearrange("p (b k) -> p b k", k=N_RBF),
                        in0=radv, in1=warv, op=OP.mult)
                else:
                    TTP(out=frt[:].rearrange("p (b k) -> p b k", k=N_RBF),
                        in0=radv, in1=warv, op=OP.mult)
                    src, dst = (frA, frB) if r_ % 2 == 1 else (frB, frA)
                    TTP(out=dst[:], in0=src[:], in1=frt[:], op=OP.add)
            fr = frB

            # lhsT_ar_g = fr-half (x) onehot : own blocks only (fp16)
            lhsT_ar = []
            for g in range(2):
                lt = ctile(f'lhsT_ar_{g}', [EBLK, NBLK_OWN * EBLK], FP16)
                TTP(out=lt[:].rearrange("p (w b n) -> p w b n", b=4, n=WIN),
                   in0=fr[:].rearrange("p (w k) -> p w k", k=N_RBF)
                       [:, :, g * 4:(g + 1) * 4].unsqueeze(3)
                       .broadcast_to([EBLK, NBLK_OWN, 4, WIN]),
                   in1=onehot[:].rearrange("p (b n) -> p b n", b=NBT)
                       [:, :NBLK_OWN].unsqueeze(2)
                       .broadcast_to([EBLK, NBLK_OWN, 4, WIN]),
                   op=OP.mult)
                lhsT_ar.append(lt)

            # ---- pass 1: per-window segment sum + radial transform,
            # with symmetrize/chi groups interleaved every GW windows so the
            # DVE/Pool sym work overlaps the PE window matmuls ----
            GW = 4                       # windows per group
            NG = WT // GW                # WT is a multiple of 4 (NH padded)
            NVG = GW * 2
            A_sb = ctile('A_sb', [EBLK, WT * GLC], FP16)
            B0s = ctile('B0s', [EBLK, NWINC * 2 * NB * C])   # own B0 (output, f32)
            chiS = ctile('chiS', [WIN, WT * C])
            chiSb = ctile('chiSb', [WIN, WT * C], FP16)
            memS = ctile('memS', [EBLK, NWINC * GLC])        # parked psMem (f32)

            def sym_group(Ain, Bout, pool_eng):
                # Ain fp16 [p, (8 pseudo-windows, lc)] view; Bout f32 [p, (8, NB*C)]
                tte = TTP if pool_eng else TT
                red = nc.vector.tensor_reduce
                sqs = wp.tile([EBLK, NVG * LC], FP16, name='sqs', tag='sqs')
                tte(out=sqs[:], in0=Ain, in1=Ain, op=OP.mult)
                nc.scalar.copy(
                    out=Bout.rearrange("p (v q) -> p v q", q=NB * C)[:, :, 0:C],
                    in_=Ain.rearrange("p (v q) -> p v q", q=LC)[:, :, 0:C])
                for dd, (ls, lcnt) in enumerate(GRP_SLICES):
                    red(
                        Bout.rearrange("p (v q) -> p v q", q=NB * C)
                            [:, :, (1 + dd) * C:(2 + dd) * C].unsqueeze(3),
                        sqs[:].rearrange("p (v l c) -> p v c l", l=NL, c=C)
                            [:, :, :, ls:ls + lcnt],
                        mybir.AxisListType.X, OP.add)

            def chi_group(gi, Bg):
                Bgb = wp.tile([EBLK, NVG * NB * C], FP16, name='Bgb', tag='Bgb')
                nc.any.tensor_copy(out=Bgb[:], in_=Bg)
                for wl in range(GW):
                    w = gi * GW + wl
                    psC = pp.tile([WIN, C], F32, name='psC', tag='seg')
                    first = True
                    for g in range(2):
                        for k in range(NB):
                            nc.tensor.matmul(
                                out=psC[:],
                                lhsT=sb['wbd_chi'][:, (g * NB + k) * WIN:
                                                   (g * NB + k + 1) * WIN],
                                rhs=Bgb[:, (wl * 2 + g) * NB * C + k * C:
                                           (wl * 2 + g) * NB * C + (k + 1) * C],
                                start=first, stop=(g == 1 and k == NB - 1),
                                skip_group_check=True)
                            first = False
                    nc.any.tensor_copy(out=chiS[:, w * C:(w + 1) * C], in_=psC[:])

            def mem_window(w):
                psMem = pp.tile([EBLK, GLC], F32, name='psMem', tag='memt')
                for gout in range(2):
                    for dd, (ls, lcnt) in enumerate(GRP_SLICES):
                        osl = slice(gout * LC + ls * C, gout * LC + (ls + lcnt) * C)
                        for gin in range(2):
                            wcol = ((gout * 4 + dd) * 2 + gin) * EBLK
                            csl = slice(w * GLC + gin * LC + ls * C,
                                        w * GLC + gin * LC + (ls + lcnt) * C)
                            nc.tensor.matmul(
                                out=psMem[:, osl],
                                lhsT=sb['wbd_mem'][:, wcol:wcol + EBLK],
                                rhs=A_sb[:, csl],
                                start=(gin == 0), stop=(gin == 1),
                                skip_group_check=True)
                nc.any.tensor_copy(out=memS[:, w * GLC:(w + 1) * GLC], in_=psMem[:])

            for w in range(WT):
                psA0 = pp.tile([EBLK, GLC], F32, name='psA0', tag='seg')
                for g in range(2):
                    for bi in range(NBW):
                        blk = w * NBW + bi
                        nc.tensor.matmul(
                            out=psA0[:, g * LC:(g + 1) * LC],
                            lhsT=lhsT1[g][:, blk * EBLK:(blk + 1) * EBLK],
                            rhs=P[:, blk * LC:(blk + 1) * LC],
                            start=(bi == 0), stop=(bi == NBW - 1),
                            skip_group_check=True)
                A0s = wp.tile([EBLK, GLC], FP16, name='A0s', tag='A0s')
                nc.any.tensor_copy(out=A0s[:], in_=psA0[:])
                psA = pp.tile([EBLK, GLC], F32, name='psA', tag='acc')
                for gout in range(2):
                    for dd, (ls, lcnt) in enumerate(GRP_SLICES):
                        osl = slice(gout * LC + ls * C, gout * LC + (ls + lcnt) * C)
                        for gin in range(2):
                            wcol = ((gout * 4 + dd) * 2 + gin) * EBLK
                            csl = slice(gin * LC + ls * C, gin * LC + (ls + lcnt) * C)
                            nc.tensor.matmul(
                                out=psA[:, osl],
                                lhsT=sb['wbd_rad'][:, wcol:wcol + EBLK],
                                rhs=A0s[:, csl],
                                start=(gin == 0), stop=(gin == 1),
                                skip_group_check=True)
                nc.any.tensor_copy(out=A_sb[:, w * GLC:(w + 1) * GLC], in_=psA[:])

            # own A rows are complete: PE parks all psMem results while the
            # DVE/Pool sym groups below run concurrently
            for wm in range(NWINC):
                mem_window(wm)

            for gi in range(NG):
                own_grp = gi < NWINC // GW
                if own_grp:
                    Bg = B0s[:, gi * NVG * NB * C:(gi + 1) * NVG * NB * C]
                else:
                    Bgt = wp.tile([EBLK, NVG * NB * C], F32, name='Bgt', tag='Bgt')
                    Bg = Bgt[:]
                sym_group(A_sb[:, gi * GW * GLC:(gi + 1) * GW * GLC], Bg,
                          pool_eng=(gi % 3 == 2))
                chi_group(gi, Bg)
            nc.any.tensor_copy(out=chiSb[:], in_=chiS[:])

            # ---- node table -> local DRAM (fp16); no collective ----
            T_local = dp.tile([WT * WIN, TW], FP16, name='T_local')
            for x in range(4):
                nc.sync.dma_start(
                    out=T_local[:, x * GLC:(x + 1) * GLC]
                        .rearrange("(w n) q -> n w q", w=WT),
                    in_=A_sb[x * WIN:(x + 1) * WIN, :]
                        .rearrange("n (w q) -> n w q", w=WT))
            nc.sync.dma_start(
                out=T_local[:, RB * LC:RB * LC + C]
                    .rearrange("(w n) c -> n w c", w=WT),
                in_=chiSb[:].rearrange("n (w c) -> n w c", w=WT))

            # ---- pass 2 (own windows only) ----
            # issue all gathers up front: they only depend on the T_local
            # write, so the DMA engines prefetch while sym/chi still run
            ags_all = []
            for blk in range(NBLK_OWN):
                ag = gp.tile([EBLK, TW], FP16, name='ag', tag='ag')
                nc.gpsimd.indirect_dma_start(
                    out=ag[:], out_offset=None, in_=T_local[:],
                    in_offset=bass.IndirectOffsetOnAxis(
                        ap=esrow_s[:, blk:blk + 1], axis=0))
                ags_all.append(ag)
            Anew = ctile('Anew', [EBLK, NWINC * GLC], FP16)
            for w in range(NWINC):
                ags = []
                P2s = []
                for bi in range(NBW):
                    blk = w * NBW + bi
                    ag = ags_all[blk]
                    ags.append(ag)
                    P2 = wp.tile([EBLK, LC], FP16, name='P2', tag='P2')
                    TT(out=P2[:].rearrange("p (l c) -> p l c", c=C),
                        in0=P[:, blk * LC:(blk + 1) * LC].rearrange("p (l c) -> p l c", c=C),
                        in1=ag[:, RB * LC:RB * LC + C].unsqueeze(1)
                            .broadcast_to([EBLK, NL, C]),
                        op=OP.mult)
                    P2s.append(P2)
                psB0 = pp.tile([EBLK, GLC], F32, name='psB0', tag='seg')
                for g in range(2):
                    for bi in range(NBW):
                        blk = w * NBW + bi
                        nc.tensor.matmul(
                            out=psB0[:, g * LC:(g + 1) * LC],
                            lhsT=lhsT1[g][:, blk * EBLK:(blk + 1) * EBLK],
                            rhs=P2s[bi][:],
                            start=(bi == 0), stop=(bi == NBW - 1),
                            skip_group_check=True)
                Ab0 = wp.tile([EBLK, GLC], FP16, name='Ab0', tag='Ab0')
                nc.any.tensor_copy(out=Ab0[:], in_=psB0[:])
                psAb = pp.tile([EBLK, GLC], F32, name='psAb', tag='acc')
                for gout in range(2):
                    for dd, (ls, lcnt) in enumerate(GRP_SLICES):
                        osl = slice(gout * LC + ls * C, gout * LC + (ls + lcnt) * C)
                        for gin in range(2):
                            wcol = ((gout * 4 + dd) * 2 + gin) * EBLK
                            csl = slice(gin * LC + ls * C, gin * LC + (ls + lcnt) * C)
                            nc.tensor.matmul(
                                out=psAb[:, osl],
                                lhsT=sb['wbd_radmp'][:, wcol:wcol + EBLK],
                                rhs=Ab0[:, csl],
                                start=(gin == 0), stop=(gin == 1),
                                skip_group_check=True)
                psAr = pp.tile([EBLK, GLC], F32, name='psAr', tag='ar')
                for b_ in range(RB):
                    g = b_ // 4; xq = b_ % 4
                    scol = xq * 2 + g          # T col-slice index for b_
                    for bi in range(NBW):
                        blk = w * NBW + bi
                        nc.tensor.matmul(
                            out=psAr[xq * WIN:(xq + 1) * WIN, g * LC:(g + 1) * LC],
                            lhsT=lhsT_ar[g][:, blk * EBLK + xq * WIN:
                                            blk * EBLK + (xq + 1) * WIN],
                            rhs=ags[bi][:, scol * LC:(scol + 1) * LC],
                            start=(bi == 0), stop=(bi == NBW - 1),
                            skip_group_check=True,
                            tile_position=(0, xq * WIN))
                comb = wp.tile([EBLK, GLC], F32, name='comb', tag='comb')
                nc.scalar.copy(out=comb[:], in_=psAb[:])
                TT(out=comb[:], in0=comb[:], in1=psAr[:], op=OP.add)
                TT(out=Anew[:, w * GLC:(w + 1) * GLC], in0=comb[:],
                   in1=memS[:, w * GLC:(w + 1) * GLC], op=OP.add)

            # ---- B1 symmetrize (own windows, f32) ----
            B1s = ctile('B1s', [EBLK, NWINC * 2 * NB * C])
            for gi in range(NWINC // GW):
                sym_group(Anew[:, gi * GW * GLC:(gi + 1) * GW * GLC],
                          B1s[:, gi * NVG * NB * C:(gi + 1) * NVG * NB * C],
                          pool_eng=False)

            # ---- output: [t2, (w,g)=16, 45] ----
            half = NWINC * 2 * NB * C
            nc.sync.dma_start(out=outB[:, 0:half], in_=B0s[:])
            nc.sync.dma_start(out=outB[:, half:2 * half], in_=B1s[:])

    nc.compile()
    return nc


_CACHE = {}


def kernel(**inputs) -> np.ndarray:
    return _kernel_impl(inputs)[0]


def _kernel_impl(inputs, trace=False):
    from concourse.bass_utils import run_bass_kernel_spmd

    packed, slot_of_node, nh = _host_prep(inputs)

    key = ('nc', nh)
    if key not in _CACHE:
        _CACHE[key] = _build_program(nh)
    nc = _CACHE[key]

    in_maps = [dict(p) for p in packed]

    res = run_bass_kernel_spmd(nc, in_maps, core_ids=list(range(N_CORES)),
                               trace=trace)

    feats_slots = np.zeros((NSLOT, RB, NB, C, 2), np.float32)
    for ci in range(N_CORES):
        arr = res.results[ci]['outB'].reshape(4, WIN, 2, NWINC, 2, NB, C)
        arr = np.transpose(arr, (3, 1, 4, 0, 5, 6, 2))
        feats_slots[ci * NWINC * WIN:(ci + 1) * NWINC * WIN] = \
            arr.reshape(NWINC * WIN, RB, NB, C, 2)
    return feats_slots[slot_of_node], res


if __name__ == '__main__':
    import pickle, os
    if os.path.exists('/tmp/inputs.pkl'):
        inputs = pickle.load(open('/tmp/inputs.pkl', 'rb'))
    else:
        import reference as Rf
        inputs = {k: np.asarray(v) for k, v in Rf.setup_inputs().items()}
        pickle.dump(inputs, open('/tmp/inputs.pkl', 'wb'))
    out = kernel(**inputs)
    print("kernel out", out.shape, out.dtype, float(np.abs(out).max()))
    if os.path.exists('/tmp/expected.npy'):
        exp = np.load('/tmp/expected.npy')
        err = np.abs(out - exp).max()
        print("max abs err vs expected:", err, "rel:", err / np.abs(exp).max())


# revision 55
# speedup vs baseline: 1.0756x; 1.0093x over previous
"""Trainium2 Bass kernel for nn_Cace_74569222193773 (CACE GNN message passing).

Strategy (8 NeuronCores, SPMD, one program shape + per-core data):
  * Host: drop edges with r >= cutoff (fcut = 0 there), assign nodes to 64
    edge-balanced global windows of <=32 nodes (8 "own" windows per core).
  * HALO REPLICATION instead of a collective: each core additionally
    recomputes pass-1 A for the sender nodes of its own edges that live on
    other cores.  Those halo nodes are repacked into private halo windows
    (<=32 nodes, <=256 in-edges each, edge-balanced); the core processes
    own + halo windows in pass 1, writes the node table T = [A row | chi]
    (fp16) to its own DRAM, and pass 2 gathers sender rows locally.
    No inter-core communication at all.
  * All node-feature tensors live in a "half" layout: partition p = x*32+n
    with x = (r or b) mod 4, plus a half index g = (r or b) // 4 in the
    free dimension, so every PE matmul output starts at a 32-aligned
    partition base.
  * Pass 1 (per core): edge geometry + bessel + cutoff + angular on
    DVE/Pool/ACT in edge-major layout [128 partitions = edges]; per-window
    segment-sum via PE matmuls (fp16 operands, fp32 PSUM) with
    lhsT = onehot32 (x) radf-half, rhs = P = ang (x) enc; radial transform
    via block-diag W (x) I32 fp16 matmuls; symmetrize + chi per window
    group (own windows in fp32 A, halo windows from the fp16 copy -- halo
    B0 only feeds chi).
  * Pass 2 (own windows only): indirect-DMA gather of T[send] (fp16 rows),
    A_ar via per-b matmuls (lhsT = onehot (x) fr slice), A_bchi via the
    pass-1 segment-sum machinery with rhs P * chi_send, mem via
    W_mem (x) I32; combine (fp32), symmetrize -> B1.

kernel() takes FULL unsharded inputs and returns the FULL [2000,8,5,9,2]
float32 output; all sharding happens inside.
"""
import heapq
from math import factorial

import numpy as np

# ---- static problem config (mirrors the reference) ----
MAX_L = 3; N_RBF = 8; RB = 8; K = 3
CUTOFF = 5.5
N_NODES = 2000
MP_NORM = 1.0 / np.sqrt(25.0)
C = K * K                      # 9
NB = 1 + (MAX_L + 1)           # 5

def _lxlylz(max_l):
    out = []
    for l in range(max_l + 1):
        for lx in range(l, -1, -1):
            for ly in range(l - lx, -1, -1):
                out.append((lx, ly, l - lx - ly))
    return out

L_LIST = _lxlylz(MAX_L); NL = len(L_LIST)                       # 20
LX = np.array([t[0] for t in L_LIST]); LY = np.array([t[1] for t in L_LIST])
LZ = np.array([t[2] for t in L_LIST]); DEGS = LX + LY + LZ
MULTI = np.array([factorial(int(d)) / (factorial(int(a)) * factorial(int(b)) * factorial(int(c)))
                  for a, b, c, d in zip(LX, LY, LZ, DEGS)], dtype=np.float32)
GRP_SLICES = []                 # (l_start, l_count) per degree; DEGS is sorted
for d in range(MAX_L + 1):
    idx = np.where(DEGS == d)[0]
    GRP_SLICES.append((int(idx[0]), int(len(idx))))

# ---- sharding geometry ----
N_CORES = 8
WIN = 32                        # nodes per window
NWINC = 8                       # own windows per core
NWIN = N_CORES * NWINC          # 64
NSLOT = NWIN * WIN              # 2048 own-node slots globally
EBLK = 128                      # edges per block (partition dim)
NBW = 2                         # blocks per window
NBLK_OWN = NWINC * NBW          # 16 own blocks per core
LC = NL * C                     # 180
GLC = 2 * LC                    # 360 = both halves
TW = RB * LC + WIN              # table row width 1472 (1440 A + 9 chi + pad)

_RSCL = np.sqrt(2.0 / CUTOFF)

F32_FIELDS = ['exyz_s', 'exyz_r', 'eemb_s', 'eemb_r', 'enloc',
              'iota32', 'multi_l', 'war_mp', 'nvec']
FP16_FIELDS = ['wbd_rad', 'wbd_radmp', 'wbd_mem', 'wbd_chi']


def _field_layout(nbt):
    """Column layout of the packed f32 / fp16 input tensors for nbt blocks."""
    fw = dict(exyz_s=3 * nbt, exyz_r=3 * nbt, eemb_s=3 * nbt, eemb_r=3 * nbt,
              enloc=nbt, iota32=WIN, multi_l=NL, war_mp=64, nvec=N_RBF,
              wbd_rad=2048, wbd_radmp=2048, wbd_mem=2048, wbd_chi=2 * NB * WIN)
    off = {}
    o = 0
    for f in F32_FIELDS:
        off[f] = o; o += fw[f]
    totf = o
    o = 0
    for f in FP16_FIELDS:
        off[f] = o; o += fw[f]
    return fw, off, totf, o


def _pack_windows(node_list, deg, nwin_cap):
    """Balanced assignment of node_list into windows (<=WIN nodes each,
    edge-load balanced).  Grows window count until max load <= NBW*EBLK.
    Returns (win_of, pos_of, n_windows)."""
    nodes = sorted(node_list, key=lambda n: -deg[n])
    nwin = max(1, (len(nodes) + WIN - 1) // WIN)
    while True:
        win_cnt = np.zeros(nwin, np.int64); win_load = np.zeros(nwin, np.int64)
        win_of = {}; pos_of = {}
        heap = [(0, w) for w in range(nwin)]
        heapq.heapify(heap)
        ok = True
        for nd in nodes:
            popped = []
            while True:
                load, w = heapq.heappop(heap)
                if win_cnt[w] < WIN:
                    break
                popped.append((load, w))
            for it in popped:
                heapq.heappush(heap, it)
            win_of[nd] = w; pos_of[nd] = int(win_cnt[w])
            win_cnt[w] += 1; win_load[w] += deg[nd]
            heapq.heappush(heap, (int(win_load[w]), w))
        if win_load.max(initial=0) <= NBW * EBLK:
            return win_of, pos_of, nwin
        nwin += 1
        if nwin > nwin_cap:
            raise RuntimeError("halo window packing overflow")


def _host_prep(inputs):
    pos = np.asarray(inputs['positions'], np.float32)
    shifts = np.asarray(inputs['shifts'], np.float32)
    W_embed = np.asarray(inputs['W_embed'], np.float32)
    species = np.asarray(inputs['species'])
    ei = np.asarray(inputs['edge_index'])
    send, recv = ei[0], ei[1]

    vec = (pos[recv] + shifts - pos[send]).astype(np.float64)
    r = np.sqrt((vec * vec).sum(-1))
    keep = np.where(r < CUTOFF)[0]
    deg = np.bincount(recv[keep], minlength=N_NODES)

    # balanced node->global-window assignment (own windows)
    order = np.argsort(-deg, kind='stable')
    win_cnt = np.zeros(NWIN, np.int64); win_load = np.zeros(NWIN, np.int64)
    win_of_node = np.zeros(N_NODES, np.int64); pos_in_win = np.zeros(N_NODES, np.int64)
    heap = [(0, w) for w in range(NWIN)]
    heapq.heapify(heap)
    for nd in order:
        popped = []
        while True:
            load, w = heapq.heappop(heap)
            if win_cnt[w] < WIN:
                break
            popped.append((load, w))
        for it in popped:
            heapq.heappush(heap, it)
        win_of_node[nd] = w; pos_in_win[nd] = win_cnt[w]
        win_cnt[w] += 1; win_load[w] += deg[nd]
        heapq.heappush(heap, (win_load[w], w))
    if win_load.max() > NBW * EBLK:
        raise RuntimeError(f"window overflow: {win_load.max()} > {NBW * EBLK}")

    slot_of_node = win_of_node * WIN + pos_in_win
    emb = W_embed[species]                       # [N, K]

    ks, kr = send[keep], recv[keep]
    in_edges = [[] for _ in range(N_NODES)]      # node -> kept edge ids
    for i, e in enumerate(keep):
        in_edges[kr[i]].append(e)

    # per-core halo structure
    core_halo = []
    nh_list = []
    for ci in range(N_CORES):
        own_w = set(range(ci * NWINC, (ci + 1) * NWINC))
        own_eids = []
        for w in sorted(own_w):
            for nd in np.where(win_of_node == w)[0]:
                own_eids.extend(in_edges[nd])
        senders = set(send[own_eids].tolist()) if own_eids else set()
        halo = [s for s in senders if win_of_node[s] not in own_w]
        hwin_of, hpos_of, nh = _pack_windows(halo, deg, 64)
        core_halo.append((own_w, hwin_of, hpos_of, nh))
        nh_list.append(nh)
    NH = max(nh_list)
    NH = ((NH + 3) // 4) * 4      # pad so WT = 8 + NH is a multiple of 4
    WT = NWINC + NH
    NBT = NBW * WT
    EPAD = NBT * EBLK

    cores = []
    for ci in range(N_CORES):
        own_w, hwin_of, hpos_of, nh = core_halo[ci]
        e_xyz_s = np.zeros((EPAD, 3), np.float32)
        e_xyz_r = np.zeros((EPAD, 3), np.float32)
        e_emb_s = np.zeros((EPAD, K), np.float32)
        e_emb_r = np.zeros((EPAD, K), np.float32)
        e_nloc = np.full((EPAD,), -1.0, np.float32)
        e_srow = np.zeros((EPAD,), np.int32)
        e_xyz_r[:, 0] = 1.0                      # pads: r = 1, finite math

        def srow_of(s):
            w = win_of_node[s]
            if w in own_w:
                return (w - ci * NWINC) * WIN + pos_in_win[s]
            return (NWINC + hwin_of[s]) * WIN + hpos_of[s]

        # local window wl in [0, WT): own first, then halo
        def fill_window(wl, node_ids, pos_of, need_srow):
            base = wl * NBW * EBLK
            eids = []
            for nd in node_ids:
                eids.extend(in_edges[nd])
            eids = np.array(eids, dtype=np.int64)
            cnt = len(eids)
            if cnt == 0:
                return
            if cnt > NBW * EBLK:
                raise RuntimeError("window edge overflow")
            sl = slice(base, base + cnt)
            e_xyz_s[sl] = pos[send[eids]]
            e_xyz_r[sl] = pos[recv[eids]] + shifts[eids]
            e_emb_s[sl] = emb[send[eids]]
            e_emb_r[sl] = emb[recv[eids]]
            e_nloc[sl] = np.array([pos_of[n] for n in recv[eids]], np.float32)
            if need_srow:
                e_srow[sl] = np.array([srow_of(s) for s in send[eids]], np.int32)

        for wl in range(NWINC):
            w = ci * NWINC + wl
            nds = np.where(win_of_node == w)[0]
            fill_window(wl, nds, {int(n): int(pos_in_win[n]) for n in nds}, True)
        halo_by_win = [[] for _ in range(nh)]
        for s, hw in hwin_of.items():
            halo_by_win[hw].append(s)
        for hw in range(nh):
            fill_window(NWINC + hw, halo_by_win[hw],
                        {int(n): int(hpos_of[n]) for n in halo_by_win[hw]}, False)

        def dev(x):
            if x.ndim == 1:
                return np.ascontiguousarray(x.reshape(NBT, EBLK).T)
            return np.ascontiguousarray(np.transpose(x.reshape(NBT, EBLK, -1), (1, 0, 2)))

        def axmajor(x3):
            d = dev(x3)                                  # [128, NBT, 3]
            return np.ascontiguousarray(np.transpose(d, (0, 2, 1)).reshape(EBLK, 3 * NBT))

        cores.append(dict(
            exyz_s=axmajor(e_xyz_s), exyz_r=axmajor(e_xyz_r),
            eemb_s=axmajor(e_emb_s), eemb_r=axmajor(e_emb_r),
            enloc=np.ascontiguousarray(dev(e_nloc)),
            esrow=np.ascontiguousarray(dev(e_srow)[:, :NBLK_OWN]),
        ))

    Wr = np.asarray(inputs['W_radial'], np.float32)   # [4(deg), 8(r), 8(b)]
    Wm = np.asarray(inputs['W_mem'], np.float32)
    Wc = np.asarray(inputs['W_chi'], np.float32)      # [8(b), 5(k)]
    Wa = np.asarray(inputs['W_ar'], np.float32)       # [8(r), 8(b)]
    I32 = np.eye(WIN, dtype=np.float32)

    def bd(W):
        cols = []
        for gout in range(2):
            for d in range(4):
                for gin in range(2):
                    cols.append(np.kron(W[d, gin * 4:gin * 4 + 4, gout * 4:gout * 4 + 4], I32))
        return np.concatenate(cols, axis=1)          # [128, 2048]

    wchi_cols = []
    for g in range(2):
        for k in range(NB):
            wchi_cols.append(np.kron(Wc[g * 4:g * 4 + 4, k:k + 1], I32))   # [128, 32]
    consts_f = dict(
        war_mp=np.tile((Wa * MP_NORM).reshape(1, 64), (EBLK, 1)),
        multi_l=np.tile(np.sqrt(MULTI).reshape(1, NL), (EBLK, 1)),
        iota32=np.tile(np.arange(WIN, dtype=np.float32).reshape(1, WIN), (EBLK, 1)),
        nvec=np.tile((np.arange(1, N_RBF + 1, dtype=np.float32) / CUTOFF).reshape(1, N_RBF),
                     (EBLK, 1)),
    )
    consts_b = dict(
        wbd_rad=bd(Wr),
        wbd_radmp=bd(Wr * MP_NORM),
        wbd_mem=bd(Wm),
        wbd_chi=np.concatenate(wchi_cols, axis=1),                   # [128, 320]
    )
    packed = []
    for ci in range(N_CORES):
        cols_f = [cores[ci][nm] for nm in
                  ['exyz_s', 'exyz_r', 'eemb_s', 'eemb_r', 'enloc']]
        cols_f += [consts_f[nm] for nm in ['iota32', 'multi_l', 'war_mp', 'nvec']]
        edf = np.ascontiguousarray(np.concatenate(cols_f, axis=1), np.float32)
        edb = np.ascontiguousarray(
            np.concatenate([consts_b[nm] for nm in FP16_FIELDS], axis=1)
        ).astype(np.float16)
        packed.append(dict(edf=edf, edb=edb, esrow=cores[ci]['esrow']))
    return packed, slot_of_node, NH


def _build_program(nh, debug=False):
    import concourse.bass as bass
    import concourse.mybir as mybir
    from concourse import bacc
    from concourse.tile import TileContext

    F32 = mybir.dt.float32
    FP16 = mybir.dt.float16
    AF = mybir.ActivationFunctionType
    OP = mybir.AluOpType

    WT = NWINC + nh
    NBT = NBW * WT
    FIELD_W, FIELD_OFF, TOTW_F, TOTW_B = _field_layout(NBT)

    nc = bacc.Bacc("TRN2", target_bir_lowering=False, debug=False,
                   num_devices=N_CORES)

    edf_d = nc.dram_tensor('edf', [EBLK, TOTW_F], F32, kind="ExternalInput")
    edb_d = nc.dram_tensor('edb', [EBLK, TOTW_B], FP16, kind="ExternalInput")
    esrow_d = nc.dram_tensor('esrow', [EBLK, NBLK_OWN], mybir.dt.int32,
                             kind="ExternalInput")
    outB = nc.dram_tensor('outB', [EBLK, 2 * NWINC * 2 * NB * C], F32,
                          kind="ExternalOutput")

    with TileContext(nc) as tc:
        with (tc.tile_pool(name="const", bufs=1) as cp,
              tc.tile_pool(name="work", bufs=2) as wp,
              tc.tile_pool(name="gat", bufs=9) as gp,
              tc.tile_pool(name="psum", bufs=2, space="PSUM") as pp,
              tc.tile_pool(name="dram", bufs=1, space="DRAM") as dp):

            bigf = cp.tile([EBLK, TOTW_F], F32, name='bigf', tag='bigf')
            nc.sync.dma_start(out=bigf[:], in_=edf_d[:])
            bigb = cp.tile([EBLK, TOTW_B], FP16, name='bigb', tag='bigb')
            nc.sync.dma_start(out=bigb[:], in_=edb_d[:])
            esrow_s = cp.tile([EBLK, NBLK_OWN], mybir.dt.int32,
                              name='esrow_s', tag='esrow_s')
            nc.sync.dma_start(out=esrow_s[:], in_=esrow_d[:])

            class _S:
                def __init__(self, tile):
                    self.tile = tile
                def __getitem__(self, nm):
                    off = FIELD_OFF[nm]
                    return self.tile[:, off:off + FIELD_W[nm]]
            s = _S(bigf)
            sb = _S(bigb)

            def ctile(tag, shape, dtype=F32):
                return cp.tile(shape, dtype, name=tag, tag=tag)

            TT = nc.vector.tensor_tensor
            TTP = nc.gpsimd.tensor_tensor
            TS = nc.vector.tensor_scalar

            # ---- geometry, edge-major [128, a*NBT+blk] ----
            vd = ctile('vd', [EBLK, 3 * NBT])
            TT(out=vd[:], in0=s['exyz_r'][:], in1=s['exyz_s'][:], op=OP.subtract)
            sq = ctile('sq', [EBLK, 3 * NBT])
            TT(out=sq[:], in0=vd[:], in1=vd[:], op=OP.mult)
            r2 = ctile('r2', [EBLK, NBT])
            TT(out=r2[:], in0=sq[:, 0:NBT], in1=sq[:, NBT:2 * NBT], op=OP.add)
            TT(out=r2[:], in0=r2[:], in1=sq[:, 2 * NBT:3 * NBT], op=OP.add)
            rr = ctile('rr', [EBLK, NBT])
            nc.scalar.activation(out=rr[:], in_=r2[:], func=AF.Sqrt)
            # reference adds 1e-9 to r before dividing; r >= cutoff-filtered
            # lengths here (>0.1), so the epsilon is numerically invisible
            rinv = ctile('rinv', [EBLK, NBT])
            nc.vector.reciprocal(out=rinv[:], in_=rr[:])
            uv = ctile('uv', [EBLK, 3 * NBT])
            TT(out=uv[:].rearrange("p (a b) -> p a b", a=3),
               in0=vd[:].rearrange("p (a b) -> p a b", a=3),
               in1=rinv[:].unsqueeze(1).broadcast_to([EBLK, 3, NBT]), op=OP.mult)

            # bessel: rad[r, blk] = sin((n+1) * pi/c * r) * (sqrt(2/c) * rinv)
            rscl = ctile('rscl', [EBLK, NBT])
            TS(out=rscl[:], in0=rinv[:], scalar1=float(-_RSCL), scalar2=None, op0=OP.mult)
            radp = ctile('radp', [EBLK, N_RBF * NBT])
            marg = ctile('marg', [EBLK, N_RBF * NBT])
            TT(out=marg[:].rearrange("p (r b) -> p r b", r=N_RBF),
               in0=rr[:].unsqueeze(1).broadcast_to([EBLK, N_RBF, NBT]),
               in1=s['nvec'][:].unsqueeze(2).broadcast_to([EBLK, N_RBF, NBT]),
               op=OP.mult)
            mtmp = ctile('mtmp', [EBLK, N_RBF * NBT])
            TS(out=mtmp[:], in0=marg[:], scalar1=4.0, scalar2=4.0,
               op0=OP.is_ge, op1=OP.mult)
            TT(out=marg[:], in0=marg[:], in1=mtmp[:], op=OP.subtract)
            TS(out=mtmp[:], in0=marg[:], scalar1=2.0, scalar2=2.0,
               op0=OP.is_ge, op1=OP.mult)
            TT(out=marg[:], in0=marg[:], in1=mtmp[:], op=OP.subtract)
            biaspi = ctile('biaspi', [EBLK, 1])
            nc.vector.memset(biaspi[:], float(-np.pi))
            nc.scalar.activation(out=radp[:], in_=marg[:], func=AF.Sin,
                                 scale=float(np.pi), bias=biaspi[:])
            TT(out=radp[:].rearrange("p (r b) -> p r b", r=N_RBF),
               in0=radp[:].rearrange("p (r b) -> p r b", r=N_RBF),
               in1=rscl[:].unsqueeze(1).broadcast_to([EBLK, N_RBF, NBT]), op=OP.mult)

            # poly cutoff (p=6); host guarantees u<1 (Pool engine)
            uu = ctile('uu', [EBLK, NBT])
            TS(out=uu[:], in0=rr[:], scalar1=float(1.0 / CUTOFF), scalar2=None, op0=OP.mult)
            u3 = ctile('u3', [EBLK, NBT])
            TTP(out=u3[:], in0=uu[:], in1=uu[:], op=OP.mult)
            TTP(out=u3[:], in0=u3[:], in1=uu[:], op=OP.mult)
            u6 = ctile('u6', [EBLK, NBT]); TTP(out=u6[:], in0=u3[:], in1=u3[:], op=OP.mult)
            u7 = ctile('u7', [EBLK, NBT]); TTP(out=u7[:], in0=u6[:], in1=uu[:], op=OP.mult)
            u8 = ctile('u8', [EBLK, NBT]); TTP(out=u8[:], in0=u7[:], in1=uu[:], op=OP.mult)
            fc = ctile('fc', [EBLK, NBT])
            nc.gpsimd.tensor_scalar(out=fc[:], in0=u6[:], scalar1=-28.0, scalar2=1.0,
                                    op0=OP.mult, op1=OP.add)
            t7 = ctile('t7', [EBLK, NBT])
            nc.gpsimd.tensor_scalar(out=t7[:], in0=u7[:], scalar1=48.0, scalar2=None,
                                    op0=OP.mult)
            TTP(out=fc[:], in0=fc[:], in1=t7[:], op=OP.add)
            nc.gpsimd.tensor_scalar(out=t7[:], in0=u8[:], scalar1=-21.0, scalar2=None,
                                    op0=OP.mult)
            TTP(out=fc[:], in0=fc[:], in1=t7[:], op=OP.add)

            radf = ctile('radf', [EBLK, N_RBF * NBT])
            TT(out=radf[:].rearrange("p (r b) -> p r b", r=N_RBF),
               in0=radp[:].rearrange("p (r b) -> p r b", r=N_RBF),
               in1=fc[:].unsqueeze(1).broadcast_to([EBLK, N_RBF, NBT]), op=OP.mult)

            # onehot [blk, n32] (DVE: Pool lacks is_equal)
            onehot = ctile('onehot', [EBLK, NBT * WIN])
            TT(out=onehot[:].rearrange("p (b n) -> p b n", b=NBT),
               in0=s['enloc'][:].unsqueeze(2).broadcast_to([EBLK, NBT, WIN]),
               in1=s['iota32'][:].unsqueeze(1).broadcast_to([EBLK, NBT, WIN]),
               op=OP.is_equal)

            # enc [blk, ks, kr] (Pool)
            enc = ctile('enc', [EBLK, NBT * C])
            TTP(out=enc[:].rearrange("p (b i j) -> p b i j", i=K, j=K),
                in0=s['eemb_s'][:].rearrange("p (k b) -> p b k", k=K).unsqueeze(3)
                    .broadcast_to([EBLK, NBT, K, K]),
                in1=s['eemb_r'][:].rearrange("p (k b) -> p b k", k=K).unsqueeze(2)
                    .broadcast_to([EBLK, NBT, K, K]),
                op=OP.mult)

            # angular monomials [l, blk]
            ones = ctile('ones', [EBLK, NBT])
            nc.vector.memset(ones[:], 1.0)
            x2 = ctile('x2', [EBLK, 3 * NBT])
            TT(out=x2[:], in0=uv[:], in1=uv[:], op=OP.mult)
            x3 = ctile('x3', [EBLK, 3 * NBT])
            TT(out=x3[:], in0=x2[:], in1=uv[:], op=OP.mult)

            def pow_plane(axis, p_):
                if p_ == 1:
                    return uv[:, axis * NBT:(axis + 1) * NBT]
                if p_ == 2:
                    return x2[:, axis * NBT:(axis + 1) * NBT]
                return x3[:, axis * NBT:(axis + 1) * NBT]

            ang = ctile('ang', [EBLK, NL * NBT])
            for l in range(NL):
                facs = [pow_plane(a, pw) for a, pw in enumerate((LX[l], LY[l], LZ[l])) if pw > 0]
                dst = ang[:, l * NBT:(l + 1) * NBT]
                if len(facs) == 0:
                    nc.scalar.copy(out=dst, in_=ones[:])
                elif len(facs) == 1:
                    nc.scalar.copy(out=dst, in_=facs[0])
                elif len(facs) == 2:
                    TT(out=dst, in0=facs[0], in1=facs[1], op=OP.mult)
                else:
                    TT(out=dst, in0=facs[0], in1=facs[1], op=OP.mult)
                    TT(out=dst, in0=dst, in1=facs[2], op=OP.mult)

            # fold sqrt(MULTI_l) into ang: every downstream tensor (A, table,
            # mem, A_ar, A_bchi) is then consistently per-l scaled, so the
            # symmetrize b2 sum needs no MULTI weighting (b1 has MULTI_0 = 1)
            TT(out=ang[:].rearrange("p (l b) -> p l b", l=NL),
               in0=ang[:].rearrange("p (l b) -> p l b", l=NL),
               in1=s['multi_l'][:].unsqueeze(2).broadcast_to([EBLK, NL, NBT]),
               op=OP.mult)

            # P = ang (x) enc : [blk, l, c] in fp16 (split DVE / Pool by half)
            P = ctile('P', [EBLK, NBT * LC], FP16)
            HB = NBT // 8
            for half, eng in ((0, TT), (1, TTP)):
                blo = half * HB; bcnt = (NBT - HB) if half else HB
                eng(out=P[:, blo * LC:(blo + bcnt) * LC]
                        .rearrange("p (b l c) -> p b l c", l=NL, c=C),
                    in0=ang[:].rearrange("p (l b) -> p b l", l=NL)[:, blo:blo + bcnt]
                        .unsqueeze(3).broadcast_to([EBLK, bcnt, NL, C]),
                    in1=enc[:].rearrange("p (b c) -> p b c", c=C)[:, blo:blo + bcnt]
                        .unsqueeze(2).broadcast_to([EBLK, bcnt, NL, C]),
                    op=OP.mult)

            # lhsT1_g = radf-half (x) onehot : [blk, r4, n32] (fp16; DVE/Pool split)
            lhsT1 = []
            for g in range(2):
                lt = ctile(f'lhsT1_{g}', [EBLK, NBT * EBLK], FP16)
                eng = TT
                eng(out=lt[:].rearrange("p (b r n) -> p b r n", r=4, n=WIN),
                    in0=radf[:].rearrange("p (r b) -> p b r", r=N_RBF)
                        [:, :, g * 4:(g + 1) * 4].unsqueeze(3)
                        .broadcast_to([EBLK, NBT, 4, WIN]),
                    in1=onehot[:].rearrange("p (b n) -> p b n", b=NBT).unsqueeze(2)
                        .broadcast_to([EBLK, NBT, 4, WIN]),
                    op=OP.mult)
                lhsT1.append(lt)

            # fr = (radf @ W_ar) * MP_NORM : own blocks only [blk, b8] (Pool)
            frA = ctile('frA', [EBLK, NBLK_OWN * N_RBF])
            frB = ctile('frB', [EBLK, NBLK_OWN * N_RBF])
            frt = ctile('frt', [EBLK, NBLK_OWN * N_RBF])
            for r_ in range(N_RBF):
                radv = radf[:, r_ * NBT:r_ * NBT + NBLK_OWN].unsqueeze(2) \
                    .broadcast_to([EBLK, NBLK_OWN, N_RBF])
                warv = s['war_mp'][:, r_ * N_RBF:(r_ + 1) * N_RBF].unsqueeze(1) \
                    .broadcast_to([EBLK, NBLK_OWN, N_RBF])
                if r_ == 0:
                    TTP(out=frA[:].rearrange("p (b k) -> p b k", k=N_RBF),
                        in0=radv, in1=warv, op=OP.mult)
                else:
                    TTP(out=frt[:].rearrange("p (b k) -> p b k", k=N_RBF),
                        in0=radv, in1=warv, op=OP.mult)
                    src, dst = (frA, frB) if r_ % 2 == 1 else (frB, frA)
                    TTP(out=dst[:], in0=src[:], in1=frt[:], op=OP.add)
            fr = frB

            # lhsT_ar_g = fr-half (x) onehot : own blocks only (fp16)
            lhsT_ar = []
            for g in range(2):
                lt = ctile(f'lhsT_ar_{g}', [EBLK, NBLK_OWN * EBLK], FP16)
                TTP(out=lt[:].rearrange("p (w b n) -> p w b n", b=4, n=WIN),
                   in0=fr[:].rearrange("p (w k) -> p w k", k=N_RBF)
                       [:, :, g * 4:(g + 1) * 4].unsqueeze(3)
                       .broadcast_to([EBLK, NBLK_OWN, 4, WIN]),
                   in1=onehot[:].rearrange("p (b n) -> p b n", b=NBT)
                       [:, :NBLK_OWN].unsqueeze(2)
                       .broadcast_to([EBLK, NBLK_OWN, 4, WIN]),
                   op=OP.mult)
                lhsT_ar.append(lt)

            # ---- pass 1: per-window segment sum + radial transform,
            # with symmetrize/chi groups interleaved every GW windows so the
            # DVE/Pool sym work overlaps the PE window matmuls ----
            GW = 4                       # windows per group
            NG = WT // GW                # WT is a multiple of 4 (NH padded)
            NVG = GW * 2
            A_sb = ctile('A_sb', [EBLK, WT * GLC], FP16)
            B0s = ctile('B0s', [EBLK, NWINC * 2 * NB * C])   # own B0 (output, f32)
            chiS = ctile('chiS', [WIN, WT * C])
            chiSb = ctile('chiSb', [WIN, WT * C], FP16)
            memS = ctile('memS', [EBLK, NWINC * GLC])        # parked psMem (f32)

            def sym_group(Ain, Bout, pool_eng):
                # Ain fp16 [p, (8 pseudo-windows, lc)] view; Bout f32 [p, (8, NB*C)]
                tte = TTP if pool_eng else TT
                red = nc.vector.tensor_reduce
                sqs = wp.tile([EBLK, NVG * LC], FP16, name='sqs', tag='sqs')
                tte(out=sqs[:], in0=Ain, in1=Ain, op=OP.mult)
                nc.scalar.copy(
                    out=Bout.rearrange("p (v q) -> p v q", q=NB * C)[:, :, 0:C],
                    in_=Ain.rearrange("p (v q) -> p v q", q=LC)[:, :, 0:C])
                for dd, (ls, lcnt) in enumerate(GRP_SLICES):
                    red(
                        Bout.rearrange("p (v q) -> p v q", q=NB * C)
                            [:, :, (1 + dd) * C:(2 + dd) * C].unsqueeze(3),
                        sqs[:].rearrange("p (v l c) -> p v c l", l=NL, c=C)
                            [:, :, :, ls:ls + lcnt],
                        mybir.AxisListType.X, OP.add)

            def chi_group(gi, Bg):
                Bgb = wp.tile([EBLK, NVG * NB * C], FP16, name='Bgb', tag='Bgb')
                nc.any.tensor_copy(out=Bgb[:], in_=Bg)
                for wl in range(GW):
                    w = gi * GW + wl
                    psC = pp.tile([WIN, C], F32, name='psC', tag='seg')
                    first = True
                    for g in range(2):
                        for k in range(NB):
                            nc.tensor.matmul(
                                out=psC[:],
                                lhsT=sb['wbd_chi'][:, (g * NB + k) * WIN:
                                                   (g * NB + k + 1) * WIN],
                                rhs=Bgb[:, (wl * 2 + g) * NB * C + k * C:
                                           (wl * 2 + g) * NB * C + (k + 1) * C],
                                start=first, stop=(g == 1 and k == NB - 1),
                                skip_group_check=True)
                            first = False
                    nc.any.tensor_copy(out=chiS[:, w * C:(w + 1) * C], in_=psC[:])

            def mem_window(w):
                psMem = pp.tile([EBLK, GLC], F32, name='psMem', tag='memt')
                for gout in range(2):
                    for dd, (ls, lcnt) in enumerate(GRP_SLICES):
                        osl = slice(gout * LC + ls * C, gout * LC + (ls + lcnt) * C)
                        for gin in range(2):
                            wcol = ((gout * 4 + dd) * 2 + gin) * EBLK
                            csl = slice(w * GLC + gin * LC + ls * C,
                                        w * GLC + gin * LC + (ls + lcnt) * C)
                            nc.tensor.matmul(
                                out=psMem[:, osl],
                                lhsT=sb['wbd_mem'][:, wcol:wcol + EBLK],
                                rhs=A_sb[:, csl],
                                start=(gin == 0), stop=(gin == 1),
                                skip_group_check=True)
                nc.any.tensor_copy(out=memS[:, w * GLC:(w + 1) * GLC], in_=psMem[:])

            for w in range(WT):
                psA0 = pp.tile([EBLK, GLC], F32, name='psA0', tag='seg')
                for g in range(2):
                    for bi in range(NBW):
                        blk = w * NBW + bi
                        nc.tensor.matmul(
                            out=psA0[:, g * LC:(g + 1) * LC],
                            lhsT=lhsT1[g][:, blk * EBLK:(blk + 1) * EBLK],
                            rhs=P[:, blk * LC:(blk + 1) * LC],
                            start=(bi == 0), stop=(bi == NBW - 1),
                            skip_group_check=True)
                A0s = wp.tile([EBLK, GLC], FP16, name='A0s', tag='A0s')
                nc.any.tensor_copy(out=A0s[:], in_=psA0[:])
                psA = pp.tile([EBLK, GLC], F32, name='psA', tag='acc')
                for gout in range(2):
                    for dd, (ls, lcnt) in enumerate(GRP_SLICES):
                        osl = slice(gout * LC + ls * C, gout * LC + (ls + lcnt) * C)
                        for gin in range(2):
                            wcol = ((gout * 4 + dd) * 2 + gin) * EBLK
                            csl = slice(gin * LC + ls * C, gin * LC + (ls + lcnt) * C)
                            nc.tensor.matmul(
                                out=psA[:, osl],
                                lhsT=sb['wbd_rad'][:, wcol:wcol + EBLK],
                                rhs=A0s[:, csl],
                                start=(gin == 0), stop=(gin == 1),
                                skip_group_check=True)
                nc.any.tensor_copy(out=A_sb[:, w * GLC:(w + 1) * GLC], in_=psA[:])

            # own A rows are complete: PE parks all psMem results while the
            # DVE/Pool sym groups below run concurrently
            for wm in range(NWINC):
                mem_window(wm)

            for gi in range(NG):
                own_grp = gi < NWINC // GW
                if own_grp:
                    Bg = B0s[:, gi * NVG * NB * C:(gi + 1) * NVG * NB * C]
                else:
                    Bgt = wp.tile([EBLK, NVG * NB * C], F32, name='Bgt', tag='Bgt')
                    Bg = Bgt[:]
                sym_group(A_sb[:, gi * GW * GLC:(gi + 1) * GW * GLC], Bg,
                          pool_eng=(gi % 3 == 2))
                chi_group(gi, Bg)
            nc.any.tensor_copy(out=chiSb[:], in_=chiS[:])

            # ---- node table -> local DRAM (fp16); no collective ----
            T_local = dp.tile([WT * WIN, TW], FP16, name='T_local')
            for x in range(4):
                nc.sync.dma_start(
                    out=T_local[:, x * GLC:(x + 1) * GLC]
                        .rearrange("(w n) q -> n w q", w=WT),
                    in_=A_sb[x * WIN:(x + 1) * WIN, :]
                        .rearrange("n (w q) -> n w q", w=WT))
            nc.sync.dma_start(
                out=T_local[:, RB * LC:RB * LC + C]
                    .rearrange("(w n) c -> n w c", w=WT),
                in_=chiSb[:].rearrange("n (w c) -> n w c", w=WT))

            # ---- pass 2 (own windows only) ----
            # issue all gathers up front: they only depend on the T_local
            # write, so the DMA engines prefetch while sym/chi still run
            ags_all = []
            for blk in range(NBLK_OWN):
                ag = gp.tile([EBLK, TW], FP16, name='ag', tag='ag')
                nc.gpsimd.indirect_dma_start(
                    out=ag[:], out_offset=None, in_=T_local[:],
                    in_offset=bass.IndirectOffsetOnAxis(
                        ap=esrow_s[:, blk:blk + 1], axis=0))
                ags_all.append(ag)
            Anew = ctile('Anew', [EBLK, NWINC * GLC], FP16)
            for w in range(NWINC):
                ags = []
                P2s = []
                for bi in range(NBW):
                    blk = w * NBW + bi
                    ag = ags_all[blk]
                    ags.append(ag)
                    P2 = wp.tile([EBLK, LC], FP16, name='P2', tag='P2')
                    TT(out=P2[:].rearrange("p (l c) -> p l c", c=C),
                        in0=P[:, blk * LC:(blk + 1) * LC].rearrange("p (l c) -> p l c", c=C),
                        in1=ag[:, RB * LC:RB * LC + C].unsqueeze(1)
                            .broadcast_to([EBLK, NL, C]),
                        op=OP.mult)
                    P2s.append(P2)
                psB0 = pp.tile([EBLK, GLC], F32, name='psB0', tag='seg')
                for g in range(2):
                    for bi in range(NBW):
                        blk = w * NBW + bi
                        nc.tensor.matmul(
                            out=psB0[:, g * LC:(g + 1) * LC],
                            lhsT=lhsT1[g][:, blk * EBLK:(blk + 1) * EBLK],
                            rhs=P2s[bi][:],
                            start=(bi == 0), stop=(bi == NBW - 1),
                            skip_group_check=True)
                Ab0 = wp.tile([EBLK, GLC], FP16, name='Ab0', tag='Ab0')
                nc.any.tensor_copy(out=Ab0[:], in_=psB0[:])
                psAb = pp.tile([EBLK, GLC], F32, name='psAb', tag='acc')
                for gout in range(2):
                    for dd, (ls, lcnt) in enumerate(GRP_SLICES):
                        osl = slice(gout * LC + ls * C, gout * LC + (ls + lcnt) * C)
                        for gin in range(2):
                            wcol = ((gout * 4 + dd) * 2 + gin) * EBLK
                            csl = slice(gin * LC + ls * C, gin * LC + (ls + lcnt) * C)
                            nc.tensor.matmul(
                                out=psAb[:, osl],
                                lhsT=sb['wbd_radmp'][:, wcol:wcol + EBLK],
                                rhs=Ab0[:, csl],
                                start=(gin == 0), stop=(gin == 1),
                                skip_group_check=True)
                psAr = pp.tile([EBLK, GLC], F32, name='psAr', tag='ar')
                for b_ in range(RB):
                    g = b_ // 4; xq = b_ % 4
                    scol = xq * 2 + g          # T col-slice index for b_
                    for bi in range(NBW):
                        blk = w * NBW + bi
                        nc.tensor.matmul(
                            out=psAr[xq * WIN:(xq + 1) * WIN, g * LC:(g + 1) * LC],
                            lhsT=lhsT_ar[g][:, blk * EBLK + xq * WIN:
                                            blk * EBLK + (xq + 1) * WIN],
                            rhs=ags[bi][:, scol * LC:(scol + 1) * LC],
                            start=(bi == 0), stop=(bi == NBW - 1),
                            skip_group_check=True,
                            tile_position=(0, xq * WIN))
                comb = wp.tile([EBLK, GLC], F32, name='comb', tag='comb')
                nc.scalar.copy(out=comb[:], in_=psAb[:])
                TT(out=comb[:], in0=comb[:], in1=psAr[:], op=OP.add)
                TT(out=Anew[:, w * GLC:(w + 1) * GLC], in0=comb[:],
                   in1=memS[:, w * GLC:(w + 1) * GLC], op=OP.add)

            # ---- B1 symmetrize (own windows, f32) ----
            B1s = ctile('B1s', [EBLK, NWINC * 2 * NB * C])
            for gi in range(NWINC // GW):
                sym_group(Anew[:, gi * GW * GLC:(gi + 1) * GW * GLC],
                          B1s[:, gi * NVG * NB * C:(gi + 1) * NVG * NB * C],
                          pool_eng=False)

            # ---- output: [t2, (w,g)=16, 45] ----
            half = NWINC * 2 * NB * C
            nc.sync.dma_start(out=outB[:, 0:half], in_=B0s[:])
            nc.sync.dma_start(out=outB[:, half:2 * half], in_=B1s[:])

    nc.compile()
    return nc


_CACHE = {}


def kernel(**inputs) -> np.ndarray:
    return _kernel_impl(inputs)[0]


def _kernel_impl(inputs, trace=False):
    from concourse.bass_utils import run_bass_kernel_spmd

    packed, slot_of_node, nh = _host_prep(inputs)

    key = ('nc', nh)
    if key not in _CACHE:
        _CACHE[key] = _build_program(nh)
    nc = _CACHE[key]

    in_maps = [dict(p) for p in packed]

    res = run_bass_kernel_spmd(nc, in_maps, core_ids=list(range(N_CORES)),
                               trace=trace)

    feats_slots = np.zeros((NSLOT, RB, NB, C, 2), np.float32)
    for ci in range(N_CORES):
        arr = res.results[ci]['outB'].reshape(4, WIN, 2, NWINC, 2, NB, C)
        arr = np.transpose(arr, (3, 1, 4, 0, 5, 6, 2))
        feats_slots[ci * NWINC * WIN:(ci + 1) * NWINC * WIN] = \
            arr.reshape(NWINC * WIN, RB, NB, C, 2)
    return feats_slots[slot_of_node], res


if __name__ == '__main__':
    import pickle, os
    if os.path.exists('/tmp/inputs.pkl'):
        inputs = pickle.load(open('/tmp/inputs.pkl', 'rb'))
    else:
        import reference as Rf
        inputs = {k: np.asarray(v) for k, v in Rf.setup_inputs().items()}
        pickle.dump(inputs, open('/tmp/inputs.pkl', 'wb'))
    out = kernel(**inputs)
    print("kernel out", out.shape, out.dtype, float(np.abs(out).max()))
    if os.path.exists('/tmp/expected.npy'):
        exp = np.load('/tmp/expected.npy')
        err = np.abs(out - exp).max()
        print("max abs err vs expected:", err, "rel:", err / np.abs(exp).max())


# revision 57
# speedup vs baseline: 1.0765x; 1.0008x over previous
"""Trainium2 Bass kernel for nn_Cace_74569222193773 (CACE GNN message passing).

Strategy (8 NeuronCores, SPMD, one program shape + per-core data):
  * Host: drop edges with r >= cutoff (fcut = 0 there), assign nodes to 64
    edge-balanced global windows of <=32 nodes (8 "own" windows per core).
  * HALO REPLICATION instead of a collective: each core additionally
    recomputes pass-1 A for the sender nodes of its own edges that live on
    other cores.  Those halo nodes are repacked into private halo windows
    (<=32 nodes, <=256 in-edges each, edge-balanced); the core processes
    own + halo windows in pass 1, writes the node table T = [A row | chi]
    (fp16) to its own DRAM, and pass 2 gathers sender rows locally.
    No inter-core communication at all.
  * All node-feature tensors live in a "half" layout: partition p = x*32+n
    with x = (r or b) mod 4, plus a half index g = (r or b) // 4 in the
    free dimension, so every PE matmul output starts at a 32-aligned
    partition base.
  * Pass 1 (per core): edge geometry + bessel + cutoff + angular on
    DVE/Pool/ACT in edge-major layout [128 partitions = edges]; per-window
    segment-sum via PE matmuls (fp16 operands, fp32 PSUM) with
    lhsT = onehot32 (x) radf-half, rhs = P = ang (x) enc; radial transform
    via block-diag W (x) I32 fp16 matmuls; symmetrize + chi per window
    group (own windows in fp32 A, halo windows from the fp16 copy -- halo
    B0 only feeds chi).
  * Pass 2 (own windows only): indirect-DMA gather of T[send] (fp16 rows),
    A_ar via per-b matmuls (lhsT = onehot (x) fr slice), A_bchi via the
    pass-1 segment-sum machinery with rhs P * chi_send, mem via
    W_mem (x) I32; combine (fp32), symmetrize -> B1.

kernel() takes FULL unsharded inputs and returns the FULL [2000,8,5,9,2]
float32 output; all sharding happens inside.
"""
import heapq
from math import factorial

import numpy as np

# ---- static problem config (mirrors the reference) ----
MAX_L = 3; N_RBF = 8; RB = 8; K = 3
CUTOFF = 5.5
N_NODES = 2000
MP_NORM = 1.0 / np.sqrt(25.0)
C = K * K                      # 9
NB = 1 + (MAX_L + 1)           # 5

def _lxlylz(max_l):
    out = []
    for l in range(max_l + 1):
        for lx in range(l, -1, -1):
            for ly in range(l - lx, -1, -1):
                out.append((lx, ly, l - lx - ly))
    return out

L_LIST = _lxlylz(MAX_L); NL = len(L_LIST)                       # 20
LX = np.array([t[0] for t in L_LIST]); LY = np.array([t[1] for t in L_LIST])
LZ = np.array([t[2] for t in L_LIST]); DEGS = LX + LY + LZ
MULTI = np.array([factorial(int(d)) / (factorial(int(a)) * factorial(int(b)) * factorial(int(c)))
                  for a, b, c, d in zip(LX, LY, LZ, DEGS)], dtype=np.float32)
GRP_SLICES = []                 # (l_start, l_count) per degree; DEGS is sorted
for d in range(MAX_L + 1):
    idx = np.where(DEGS == d)[0]
    GRP_SLICES.append((int(idx[0]), int(len(idx))))

# ---- sharding geometry ----
N_CORES = 8
WIN = 32                        # nodes per window
NWINC = 8                       # own windows per core
NWIN = N_CORES * NWINC          # 64
NSLOT = NWIN * WIN              # 2048 own-node slots globally
EBLK = 128                      # edges per block (partition dim)
NBW = 2                         # blocks per window
NBLK_OWN = NWINC * NBW          # 16 own blocks per core
LC = NL * C                     # 180
GLC = 2 * LC                    # 360 = both halves
TW = RB * LC + WIN              # table row width 1472 (1440 A + 9 chi + pad)

_RSCL = np.sqrt(2.0 / CUTOFF)

F32_FIELDS = ['exyz_s', 'exyz_r', 'eemb_s', 'eemb_r', 'enloc',
              'iota32', 'multi_l', 'war_mp', 'nvec']
FP16_FIELDS = ['wbd_rad', 'wbd_radmp', 'wbd_mem', 'wbd_chi']


def _field_layout(nbt):
    """Column layout of the packed f32 / fp16 input tensors for nbt blocks."""
    fw = dict(exyz_s=3 * nbt, exyz_r=3 * nbt, eemb_s=3 * nbt, eemb_r=3 * nbt,
              enloc=nbt, iota32=WIN, multi_l=NL, war_mp=64, nvec=N_RBF,
              wbd_rad=2048, wbd_radmp=2048, wbd_mem=2048, wbd_chi=2 * NB * WIN)
    off = {}
    o = 0
    for f in F32_FIELDS:
        off[f] = o; o += fw[f]
    totf = o
    o = 0
    for f in FP16_FIELDS:
        off[f] = o; o += fw[f]
    return fw, off, totf, o


def _pack_windows(node_list, deg, nwin_cap):
    """Balanced assignment of node_list into windows (<=WIN nodes each,
    edge-load balanced).  Grows window count until max load <= NBW*EBLK.
    Returns (win_of, pos_of, n_windows)."""
    nodes = sorted(node_list, key=lambda n: -deg[n])
    nwin = max(1, (len(nodes) + WIN - 1) // WIN)
    while True:
        win_cnt = np.zeros(nwin, np.int64); win_load = np.zeros(nwin, np.int64)
        win_of = {}; pos_of = {}
        heap = [(0, w) for w in range(nwin)]
        heapq.heapify(heap)
        ok = True
        for nd in nodes:
            popped = []
            while True:
                load, w = heapq.heappop(heap)
                if win_cnt[w] < WIN:
                    break
                popped.append((load, w))
            for it in popped:
                heapq.heappush(heap, it)
            win_of[nd] = w; pos_of[nd] = int(win_cnt[w])
            win_cnt[w] += 1; win_load[w] += deg[nd]
            heapq.heappush(heap, (int(win_load[w]), w))
        if win_load.max(initial=0) <= NBW * EBLK:
            return win_of, pos_of, nwin
        nwin += 1
        if nwin > nwin_cap:
            raise RuntimeError("halo window packing overflow")


def _host_prep(inputs):
    pos = np.asarray(inputs['positions'], np.float32)
    shifts = np.asarray(inputs['shifts'], np.float32)
    W_embed = np.asarray(inputs['W_embed'], np.float32)
    species = np.asarray(inputs['species'])
    ei = np.asarray(inputs['edge_index'])
    send, recv = ei[0], ei[1]

    vec = (pos[recv] + shifts - pos[send]).astype(np.float64)
    r = np.sqrt((vec * vec).sum(-1))
    keep = np.where(r < CUTOFF)[0]
    deg = np.bincount(recv[keep], minlength=N_NODES)

    # balanced node->global-window assignment (own windows)
    order = np.argsort(-deg, kind='stable')
    win_cnt = np.zeros(NWIN, np.int64); win_load = np.zeros(NWIN, np.int64)
    win_of_node = np.zeros(N_NODES, np.int64); pos_in_win = np.zeros(N_NODES, np.int64)
    heap = [(0, w) for w in range(NWIN)]
    heapq.heapify(heap)
    for nd in order:
        popped = []
        while True:
            load, w = heapq.heappop(heap)
            if win_cnt[w] < WIN:
                break
            popped.append((load, w))
        for it in popped:
            heapq.heappush(heap, it)
        win_of_node[nd] = w; pos_in_win[nd] = win_cnt[w]
        win_cnt[w] += 1; win_load[w] += deg[nd]
        heapq.heappush(heap, (win_load[w], w))
    if win_load.max() > NBW * EBLK:
        raise RuntimeError(f"window overflow: {win_load.max()} > {NBW * EBLK}")

    slot_of_node = win_of_node * WIN + pos_in_win
    emb = W_embed[species]                       # [N, K]

    ks, kr = send[keep], recv[keep]
    in_edges = [[] for _ in range(N_NODES)]      # node -> kept edge ids
    for i, e in enumerate(keep):
        in_edges[kr[i]].append(e)

    # per-core halo structure
    core_halo = []
    nh_list = []
    for ci in range(N_CORES):
        own_w = set(range(ci * NWINC, (ci + 1) * NWINC))
        own_eids = []
        for w in sorted(own_w):
            for nd in np.where(win_of_node == w)[0]:
                own_eids.extend(in_edges[nd])
        senders = set(send[own_eids].tolist()) if own_eids else set()
        halo = [s for s in senders if win_of_node[s] not in own_w]
        hwin_of, hpos_of, nh = _pack_windows(halo, deg, 64)
        core_halo.append((own_w, hwin_of, hpos_of, nh))
        nh_list.append(nh)
    NH = max(nh_list)
    NH = ((NH + 3) // 4) * 4      # pad so WT = 8 + NH is a multiple of 4
    WT = NWINC + NH
    NBT = NBW * WT
    EPAD = NBT * EBLK

    cores = []
    for ci in range(N_CORES):
        own_w, hwin_of, hpos_of, nh = core_halo[ci]
        e_xyz_s = np.zeros((EPAD, 3), np.float32)
        e_xyz_r = np.zeros((EPAD, 3), np.float32)
        e_emb_s = np.zeros((EPAD, K), np.float32)
        e_emb_r = np.zeros((EPAD, K), np.float32)
        e_nloc = np.full((EPAD,), -1.0, np.float32)
        e_srow = np.zeros((EPAD,), np.int32)
        e_xyz_r[:, 0] = 1.0                      # pads: r = 1, finite math

        def srow_of(s):
            w = win_of_node[s]
            if w in own_w:
                return (w - ci * NWINC) * WIN + pos_in_win[s]
            return (NWINC + hwin_of[s]) * WIN + hpos_of[s]

        # local window wl in [0, WT): own first, then halo
        def fill_window(wl, node_ids, pos_of, need_srow):
            base = wl * NBW * EBLK
            eids = []
            for nd in node_ids:
                eids.extend(in_edges[nd])
            eids = np.array(eids, dtype=np.int64)
            cnt = len(eids)
            if cnt == 0:
                return
            if cnt > NBW * EBLK:
                raise RuntimeError("window edge overflow")
            sl = slice(base, base + cnt)
            e_xyz_s[sl] = pos[send[eids]]
            e_xyz_r[sl] = pos[recv[eids]] + shifts[eids]
            e_emb_s[sl] = emb[send[eids]]
            e_emb_r[sl] = emb[recv[eids]]
            e_nloc[sl] = np.array([pos_of[n] for n in recv[eids]], np.float32)
            if need_srow:
                e_srow[sl] = np.array([srow_of(s) for s in send[eids]], np.int32)

        for wl in range(NWINC):
            w = ci * NWINC + wl
            nds = np.where(win_of_node == w)[0]
            fill_window(wl, nds, {int(n): int(pos_in_win[n]) for n in nds}, True)
        halo_by_win = [[] for _ in range(nh)]
        for s, hw in hwin_of.items():
            halo_by_win[hw].append(s)
        for hw in range(nh):
            fill_window(NWINC + hw, halo_by_win[hw],
                        {int(n): int(hpos_of[n]) for n in halo_by_win[hw]}, False)

        def dev(x):
            if x.ndim == 1:
                return np.ascontiguousarray(x.reshape(NBT, EBLK).T)
            return np.ascontiguousarray(np.transpose(x.reshape(NBT, EBLK, -1), (1, 0, 2)))

        def axmajor(x3):
            d = dev(x3)                                  # [128, NBT, 3]
            return np.ascontiguousarray(np.transpose(d, (0, 2, 1)).reshape(EBLK, 3 * NBT))

        cores.append(dict(
            exyz_s=axmajor(e_xyz_s), exyz_r=axmajor(e_xyz_r),
            eemb_s=axmajor(e_emb_s), eemb_r=axmajor(e_emb_r),
            enloc=np.ascontiguousarray(dev(e_nloc)),
            esrow=np.ascontiguousarray(dev(e_srow)[:, :NBLK_OWN]),
        ))

    Wr = np.asarray(inputs['W_radial'], np.float32)   # [4(deg), 8(r), 8(b)]
    Wm = np.asarray(inputs['W_mem'], np.float32)
    Wc = np.asarray(inputs['W_chi'], np.float32)      # [8(b), 5(k)]
    Wa = np.asarray(inputs['W_ar'], np.float32)       # [8(r), 8(b)]
    I32 = np.eye(WIN, dtype=np.float32)

    def bd(W):
        cols = []
        for gout in range(2):
            for d in range(4):
                for gin in range(2):
                    cols.append(np.kron(W[d, gin * 4:gin * 4 + 4, gout * 4:gout * 4 + 4], I32))
        return np.concatenate(cols, axis=1)          # [128, 2048]

    wchi_cols = []
    for g in range(2):
        for k in range(NB):
            wchi_cols.append(np.kron(Wc[g * 4:g * 4 + 4, k:k + 1], I32))   # [128, 32]
    consts_f = dict(
        war_mp=np.tile((Wa * MP_NORM).reshape(1, 64), (EBLK, 1)),
        multi_l=np.tile(np.sqrt(MULTI).reshape(1, NL), (EBLK, 1)),
        iota32=np.tile(np.arange(WIN, dtype=np.float32).reshape(1, WIN), (EBLK, 1)),
        nvec=np.tile((np.arange(1, N_RBF + 1, dtype=np.float32) / CUTOFF).reshape(1, N_RBF),
                     (EBLK, 1)),
    )
    consts_b = dict(
        wbd_rad=bd(Wr),
        wbd_radmp=bd(Wr * MP_NORM),
        wbd_mem=bd(Wm),
        wbd_chi=np.concatenate(wchi_cols, axis=1),                   # [128, 320]
    )
    packed = []
    for ci in range(N_CORES):
        cols_f = [cores[ci][nm] for nm in
                  ['exyz_s', 'exyz_r', 'eemb_s', 'eemb_r', 'enloc']]
        cols_f += [consts_f[nm] for nm in ['iota32', 'multi_l', 'war_mp', 'nvec']]
        edf = np.ascontiguousarray(np.concatenate(cols_f, axis=1), np.float32)
        edb = np.ascontiguousarray(
            np.concatenate([consts_b[nm] for nm in FP16_FIELDS], axis=1)
        ).astype(np.float16)
        packed.append(dict(edf=edf, edb=edb, esrow=cores[ci]['esrow']))
    return packed, slot_of_node, NH


def _build_program(nh, debug=False):
    import concourse.bass as bass
    import concourse.mybir as mybir
    from concourse import bacc
    from concourse.tile import TileContext

    F32 = mybir.dt.float32
    FP16 = mybir.dt.float16
    AF = mybir.ActivationFunctionType
    OP = mybir.AluOpType

    WT = NWINC + nh
    NBT = NBW * WT
    FIELD_W, FIELD_OFF, TOTW_F, TOTW_B = _field_layout(NBT)

    nc = bacc.Bacc("TRN2", target_bir_lowering=False, debug=False,
                   num_devices=N_CORES)

    edf_d = nc.dram_tensor('edf', [EBLK, TOTW_F], F32, kind="ExternalInput")
    edb_d = nc.dram_tensor('edb', [EBLK, TOTW_B], FP16, kind="ExternalInput")
    esrow_d = nc.dram_tensor('esrow', [EBLK, NBLK_OWN], mybir.dt.int32,
                             kind="ExternalInput")
    outB = nc.dram_tensor('outB', [EBLK, 2 * NWINC * 2 * NB * C], F32,
                          kind="ExternalOutput")

    with TileContext(nc) as tc:
        with (tc.tile_pool(name="const", bufs=1) as cp,
              tc.tile_pool(name="work", bufs=2) as wp,
              tc.tile_pool(name="gat", bufs=9) as gp,
              tc.tile_pool(name="psum", bufs=2, space="PSUM") as pp,
              tc.tile_pool(name="dram", bufs=1, space="DRAM") as dp):

            bigf = cp.tile([EBLK, TOTW_F], F32, name='bigf', tag='bigf')
            nc.sync.dma_start(out=bigf[:], in_=edf_d[:])
            bigb = cp.tile([EBLK, TOTW_B], FP16, name='bigb', tag='bigb')
            nc.sync.dma_start(out=bigb[:], in_=edb_d[:])
            esrow_s = cp.tile([EBLK, NBLK_OWN], mybir.dt.int32,
                              name='esrow_s', tag='esrow_s')
            nc.sync.dma_start(out=esrow_s[:], in_=esrow_d[:])

            class _S:
                def __init__(self, tile):
                    self.tile = tile
                def __getitem__(self, nm):
                    off = FIELD_OFF[nm]
                    return self.tile[:, off:off + FIELD_W[nm]]
            s = _S(bigf)
            sb = _S(bigb)

            def ctile(tag, shape, dtype=F32):
                return cp.tile(shape, dtype, name=tag, tag=tag)

            TT = nc.vector.tensor_tensor
            TTP = nc.gpsimd.tensor_tensor
            TS = nc.vector.tensor_scalar

            # ---- geometry, edge-major [128, a*NBT+blk] ----
            vd = ctile('vd', [EBLK, 3 * NBT])
            TT(out=vd[:], in0=s['exyz_r'][:], in1=s['exyz_s'][:], op=OP.subtract)
            sq = ctile('sq', [EBLK, 3 * NBT])
            TT(out=sq[:], in0=vd[:], in1=vd[:], op=OP.mult)
            r2 = ctile('r2', [EBLK, NBT])
            TT(out=r2[:], in0=sq[:, 0:NBT], in1=sq[:, NBT:2 * NBT], op=OP.add)
            TT(out=r2[:], in0=r2[:], in1=sq[:, 2 * NBT:3 * NBT], op=OP.add)
            rr = ctile('rr', [EBLK, NBT])
            nc.scalar.activation(out=rr[:], in_=r2[:], func=AF.Sqrt)
            # reference adds 1e-9 to r before dividing; r >= cutoff-filtered
            # lengths here (>0.1), so the epsilon is numerically invisible
            rinv = ctile('rinv', [EBLK, NBT])
            nc.vector.reciprocal(out=rinv[:], in_=rr[:])
            uv = ctile('uv', [EBLK, 3 * NBT])
            TT(out=uv[:].rearrange("p (a b) -> p a b", a=3),
               in0=vd[:].rearrange("p (a b) -> p a b", a=3),
               in1=rinv[:].unsqueeze(1).broadcast_to([EBLK, 3, NBT]), op=OP.mult)

            # bessel: rad[r, blk] = sin((n+1) * pi/c * r) * (sqrt(2/c) * rinv)
            rscl = ctile('rscl', [EBLK, NBT])
            TS(out=rscl[:], in0=rinv[:], scalar1=float(-_RSCL), scalar2=None, op0=OP.mult)
            radp = ctile('radp', [EBLK, N_RBF * NBT])
            marg = ctile('marg', [EBLK, N_RBF * NBT])
            TT(out=marg[:].rearrange("p (r b) -> p r b", r=N_RBF),
               in0=rr[:].unsqueeze(1).broadcast_to([EBLK, N_RBF, NBT]),
               in1=s['nvec'][:].unsqueeze(2).broadcast_to([EBLK, N_RBF, NBT]),
               op=OP.mult)
            mtmp = ctile('mtmp', [EBLK, N_RBF * NBT])
            TS(out=mtmp[:], in0=marg[:], scalar1=4.0, scalar2=4.0,
               op0=OP.is_ge, op1=OP.mult)
            TT(out=marg[:], in0=marg[:], in1=mtmp[:], op=OP.subtract)
            TS(out=mtmp[:], in0=marg[:], scalar1=2.0, scalar2=2.0,
               op0=OP.is_ge, op1=OP.mult)
            TT(out=marg[:], in0=marg[:], in1=mtmp[:], op=OP.subtract)
            biaspi = ctile('biaspi', [EBLK, 1])
            nc.vector.memset(biaspi[:], float(-np.pi))
            nc.scalar.activation(out=radp[:], in_=marg[:], func=AF.Sin,
                                 scale=float(np.pi), bias=biaspi[:])
            TT(out=radp[:].rearrange("p (r b) -> p r b", r=N_RBF),
               in0=radp[:].rearrange("p (r b) -> p r b", r=N_RBF),
               in1=rscl[:].unsqueeze(1).broadcast_to([EBLK, N_RBF, NBT]), op=OP.mult)

            # poly cutoff (p=6); host guarantees u<1 (Pool engine)
            uu = ctile('uu', [EBLK, NBT])
            nc.gpsimd.tensor_scalar(out=uu[:], in0=rr[:], scalar1=float(1.0 / CUTOFF),
                                    scalar2=None, op0=OP.mult)
            u3 = ctile('u3', [EBLK, NBT])
            TTP(out=u3[:], in0=uu[:], in1=uu[:], op=OP.mult)
            TTP(out=u3[:], in0=u3[:], in1=uu[:], op=OP.mult)
            u6 = ctile('u6', [EBLK, NBT]); TTP(out=u6[:], in0=u3[:], in1=u3[:], op=OP.mult)
            u7 = ctile('u7', [EBLK, NBT]); TTP(out=u7[:], in0=u6[:], in1=uu[:], op=OP.mult)
            u8 = ctile('u8', [EBLK, NBT]); TTP(out=u8[:], in0=u7[:], in1=uu[:], op=OP.mult)
            fc = ctile('fc', [EBLK, NBT])
            nc.gpsimd.tensor_scalar(out=fc[:], in0=u6[:], scalar1=-28.0, scalar2=1.0,
                                    op0=OP.mult, op1=OP.add)
            t7 = ctile('t7', [EBLK, NBT])
            nc.gpsimd.tensor_scalar(out=t7[:], in0=u7[:], scalar1=48.0, scalar2=None,
                                    op0=OP.mult)
            TTP(out=fc[:], in0=fc[:], in1=t7[:], op=OP.add)
            nc.gpsimd.tensor_scalar(out=t7[:], in0=u8[:], scalar1=-21.0, scalar2=None,
                                    op0=OP.mult)
            TTP(out=fc[:], in0=fc[:], in1=t7[:], op=OP.add)

            radf = ctile('radf', [EBLK, N_RBF * NBT])
            TT(out=radf[:].rearrange("p (r b) -> p r b", r=N_RBF),
               in0=radp[:].rearrange("p (r b) -> p r b", r=N_RBF),
               in1=fc[:].unsqueeze(1).broadcast_to([EBLK, N_RBF, NBT]), op=OP.mult)

            # onehot [blk, n32] (DVE: Pool lacks is_equal)
            onehot = ctile('onehot', [EBLK, NBT * WIN])
            TT(out=onehot[:].rearrange("p (b n) -> p b n", b=NBT),
               in0=s['enloc'][:].unsqueeze(2).broadcast_to([EBLK, NBT, WIN]),
               in1=s['iota32'][:].unsqueeze(1).broadcast_to([EBLK, NBT, WIN]),
               op=OP.is_equal)

            # enc [blk, ks, kr] (Pool)
            enc = ctile('enc', [EBLK, NBT * C])
            TTP(out=enc[:].rearrange("p (b i j) -> p b i j", i=K, j=K),
                in0=s['eemb_s'][:].rearrange("p (k b) -> p b k", k=K).unsqueeze(3)
                    .broadcast_to([EBLK, NBT, K, K]),
                in1=s['eemb_r'][:].rearrange("p (k b) -> p b k", k=K).unsqueeze(2)
                    .broadcast_to([EBLK, NBT, K, K]),
                op=OP.mult)

            # angular monomials [l, blk]
            ones = ctile('ones', [EBLK, NBT])
            nc.vector.memset(ones[:], 1.0)
            x2 = ctile('x2', [EBLK, 3 * NBT])
            TT(out=x2[:], in0=uv[:], in1=uv[:], op=OP.mult)
            x3 = ctile('x3', [EBLK, 3 * NBT])
            TT(out=x3[:], in0=x2[:], in1=uv[:], op=OP.mult)

            def pow_plane(axis, p_):
                if p_ == 1:
                    return uv[:, axis * NBT:(axis + 1) * NBT]
                if p_ == 2:
                    return x2[:, axis * NBT:(axis + 1) * NBT]
                return x3[:, axis * NBT:(axis + 1) * NBT]

            ang = ctile('ang', [EBLK, NL * NBT])
            for l in range(NL):
                facs = [pow_plane(a, pw) for a, pw in enumerate((LX[l], LY[l], LZ[l])) if pw > 0]
                dst = ang[:, l * NBT:(l + 1) * NBT]
                if len(facs) == 0:
                    nc.scalar.copy(out=dst, in_=ones[:])
                elif len(facs) == 1:
                    nc.scalar.copy(out=dst, in_=facs[0])
                elif len(facs) == 2:
                    TT(out=dst, in0=facs[0], in1=facs[1], op=OP.mult)
                else:
                    TT(out=dst, in0=facs[0], in1=facs[1], op=OP.mult)
                    TT(out=dst, in0=dst, in1=facs[2], op=OP.mult)

            # fold sqrt(MULTI_l) into ang: every downstream tensor (A, table,
            # mem, A_ar, A_bchi) is then consistently per-l scaled, so the
            # symmetrize b2 sum needs no MULTI weighting (b1 has MULTI_0 = 1)
            TT(out=ang[:].rearrange("p (l b) -> p l b", l=NL),
               in0=ang[:].rearrange("p (l b) -> p l b", l=NL),
               in1=s['multi_l'][:].unsqueeze(2).broadcast_to([EBLK, NL, NBT]),
               op=OP.mult)

            # P = ang (x) enc : [blk, l, c] in fp16 (split DVE / Pool by half)
            P = ctile('P', [EBLK, NBT * LC], FP16)
            HB = NBT // 8
            for half, eng in ((0, TT), (1, TTP)):
                blo = half * HB; bcnt = (NBT - HB) if half else HB
                eng(out=P[:, blo * LC:(blo + bcnt) * LC]
                        .rearrange("p (b l c) -> p b l c", l=NL, c=C),
                    in0=ang[:].rearrange("p (l b) -> p b l", l=NL)[:, blo:blo + bcnt]
                        .unsqueeze(3).broadcast_to([EBLK, bcnt, NL, C]),
                    in1=enc[:].rearrange("p (b c) -> p b c", c=C)[:, blo:blo + bcnt]
                        .unsqueeze(2).broadcast_to([EBLK, bcnt, NL, C]),
                    op=OP.mult)

            # lhsT1_g = radf-half (x) onehot : [blk, r4, n32] (fp16; DVE/Pool split)
            lhsT1 = []
            for g in range(2):
                lt = ctile(f'lhsT1_{g}', [EBLK, NBT * EBLK], FP16)
                eng = TT
                eng(out=lt[:].rearrange("p (b r n) -> p b r n", r=4, n=WIN),
                    in0=radf[:].rearrange("p (r b) -> p b r", r=N_RBF)
                        [:, :, g * 4:(g + 1) * 4].unsqueeze(3)
                        .broadcast_to([EBLK, NBT, 4, WIN]),
                    in1=onehot[:].rearrange("p (b n) -> p b n", b=NBT).unsqueeze(2)
                        .broadcast_to([EBLK, NBT, 4, WIN]),
                    op=OP.mult)
                lhsT1.append(lt)

            # fr = (radf @ W_ar) * MP_NORM : own blocks only [blk, b8] (Pool)
            frA = ctile('frA', [EBLK, NBLK_OWN * N_RBF])
            frB = ctile('frB', [EBLK, NBLK_OWN * N_RBF])
            frt = ctile('frt', [EBLK, NBLK_OWN * N_RBF])
            for r_ in range(N_RBF):
                radv = radf[:, r_ * NBT:r_ * NBT + NBLK_OWN].unsqueeze(2) \
                    .broadcast_to([EBLK, NBLK_OWN, N_RBF])
                warv = s['war_mp'][:, r_ * N_RBF:(r_ + 1) * N_RBF].unsqueeze(1) \
                    .broadcast_to([EBLK, NBLK_OWN, N_RBF])
                if r_ == 0:
                    TTP(out=frA[:].rearrange("p (b k) -> p b k", k=N_RBF),
                        in0=radv, in1=warv, op=OP.mult)
                else:
                    TTP(out=frt[:].rearrange("p (b k) -> p b k", k=N_RBF),
                        in0=radv, in1=warv, op=OP.mult)
                    src, dst = (frA, frB) if r_ % 2 == 1 else (frB, frA)
                    TTP(out=dst[:], in0=src[:], in1=frt[:], op=OP.add)
            fr = frB

            # lhsT_ar_g = fr-half (x) onehot : own blocks only (fp16)
            lhsT_ar = []
            for g in range(2):
                lt = ctile(f'lhsT_ar_{g}', [EBLK, NBLK_OWN * EBLK], FP16)
                TTP(out=lt[:].rearrange("p (w b n) -> p w b n", b=4, n=WIN),
                   in0=fr[:].rearrange("p (w k) -> p w k", k=N_RBF)
                       [:, :, g * 4:(g + 1) * 4].unsqueeze(3)
                       .broadcast_to([EBLK, NBLK_OWN, 4, WIN]),
                   in1=onehot[:].rearrange("p (b n) -> p b n", b=NBT)
                       [:, :NBLK_OWN].unsqueeze(2)
                       .broadcast_to([EBLK, NBLK_OWN, 4, WIN]),
                   op=OP.mult)
                lhsT_ar.append(lt)

            # ---- pass 1: per-window segment sum + radial transform,
            # with symmetrize/chi groups interleaved every GW windows so the
            # DVE/Pool sym work overlaps the PE window matmuls ----
            GW = 4                       # windows per group
            NG = WT // GW                # WT is a multiple of 4 (NH padded)
            NVG = GW * 2
            A_sb = ctile('A_sb', [EBLK, WT * GLC], FP16)
            B0s = ctile('B0s', [EBLK, NWINC * 2 * NB * C])   # own B0 (output, f32)
            chiS = ctile('chiS', [WIN, WT * C])
            chiSb = ctile('chiSb', [WIN, WT * C], FP16)
            memS = ctile('memS', [EBLK, NWINC * GLC])        # parked psMem (f32)

            def sym_group(Ain, Bout, pool_eng):
                # Ain fp16 [p, (8 pseudo-windows, lc)] view; Bout f32 [p, (8, NB*C)]
                tte = TTP if pool_eng else TT
                red = nc.vector.tensor_reduce
                sqs = wp.tile([EBLK, NVG * LC], FP16, name='sqs', tag='sqs')
                tte(out=sqs[:], in0=Ain, in1=Ain, op=OP.mult)
                nc.scalar.copy(
                    out=Bout.rearrange("p (v q) -> p v q", q=NB * C)[:, :, 0:C],
                    in_=Ain.rearrange("p (v q) -> p v q", q=LC)[:, :, 0:C])
                for dd, (ls, lcnt) in enumerate(GRP_SLICES):
                    red(
                        Bout.rearrange("p (v q) -> p v q", q=NB * C)
                            [:, :, (1 + dd) * C:(2 + dd) * C].unsqueeze(3),
                        sqs[:].rearrange("p (v l c) -> p v c l", l=NL, c=C)
                            [:, :, :, ls:ls + lcnt],
                        mybir.AxisListType.X, OP.add)

            def chi_group(gi, Bg):
                Bgb = wp.tile([EBLK, NVG * NB * C], FP16, name='Bgb', tag='Bgb')
                nc.any.tensor_copy(out=Bgb[:], in_=Bg)
                for wl in range(GW):
                    w = gi * GW + wl
                    psC = pp.tile([WIN, C], F32, name='psC', tag='seg')
                    first = True
                    for g in range(2):
                        for k in range(NB):
                            nc.tensor.matmul(
                                out=psC[:],
                                lhsT=sb['wbd_chi'][:, (g * NB + k) * WIN:
                                                   (g * NB + k + 1) * WIN],
                                rhs=Bgb[:, (wl * 2 + g) * NB * C + k * C:
                                           (wl * 2 + g) * NB * C + (k + 1) * C],
                                start=first, stop=(g == 1 and k == NB - 1),
                                skip_group_check=True)
                            first = False
                    nc.any.tensor_copy(out=chiS[:, w * C:(w + 1) * C], in_=psC[:])

            def mem_window(w):
                psMem = pp.tile([EBLK, GLC], F32, name='psMem', tag='memt')
                for gout in range(2):
                    for dd, (ls, lcnt) in enumerate(GRP_SLICES):
                        osl = slice(gout * LC + ls * C, gout * LC + (ls + lcnt) * C)
                        for gin in range(2):
                            wcol = ((gout * 4 + dd) * 2 + gin) * EBLK
                            csl = slice(w * GLC + gin * LC + ls * C,
                                        w * GLC + gin * LC + (ls + lcnt) * C)
                            nc.tensor.matmul(
                                out=psMem[:, osl],
                                lhsT=sb['wbd_mem'][:, wcol:wcol + EBLK],
                                rhs=A_sb[:, csl],
                                start=(gin == 0), stop=(gin == 1),
                                skip_group_check=True)
                nc.any.tensor_copy(out=memS[:, w * GLC:(w + 1) * GLC], in_=psMem[:])

            for w in range(WT):
                psA0 = pp.tile([EBLK, GLC], F32, name='psA0', tag='seg')
                for g in range(2):
                    for bi in range(NBW):
                        blk = w * NBW + bi
                        nc.tensor.matmul(
                            out=psA0[:, g * LC:(g + 1) * LC],
                            lhsT=lhsT1[g][:, blk * EBLK:(blk + 1) * EBLK],
                            rhs=P[:, blk * LC:(blk + 1) * LC],
                            start=(bi == 0), stop=(bi == NBW - 1),
                            skip_group_check=True)
                A0s = wp.tile([EBLK, GLC], FP16, name='A0s', tag='A0s')
                nc.any.tensor_copy(out=A0s[:], in_=psA0[:])
                psA = pp.tile([EBLK, GLC], F32, name='psA', tag='acc')
                for gout in range(2):
                    for dd, (ls, lcnt) in enumerate(GRP_SLICES):
                        osl = slice(gout * LC + ls * C, gout * LC + (ls + lcnt) * C)
                        for gin in range(2):
                            wcol = ((gout * 4 + dd) * 2 + gin) * EBLK
                            csl = slice(gin * LC + ls * C, gin * LC + (ls + lcnt) * C)
                            nc.tensor.matmul(
                                out=psA[:, osl],
                                lhsT=sb['wbd_rad'][:, wcol:wcol + EBLK],
                                rhs=A0s[:, csl],
                                start=(gin == 0), stop=(gin == 1),
                                skip_group_check=True)
                nc.any.tensor_copy(out=A_sb[:, w * GLC:(w + 1) * GLC], in_=psA[:])

            # own A rows are complete: PE parks all psMem results while the
            # DVE/Pool sym groups below run concurrently
            for wm in range(NWINC):
                mem_window(wm)

            for gi in range(NG):
                own_grp = gi < NWINC // GW
                if own_grp:
                    Bg = B0s[:, gi * NVG * NB * C:(gi + 1) * NVG * NB * C]
                else:
                    Bgt = wp.tile([EBLK, NVG * NB * C], F32, name='Bgt', tag='Bgt')
                    Bg = Bgt[:]
                sym_group(A_sb[:, gi * GW * GLC:(gi + 1) * GW * GLC], Bg,
                          pool_eng=(gi % 3 == 2))
                chi_group(gi, Bg)
            nc.any.tensor_copy(out=chiSb[:], in_=chiS[:])

            # ---- node table -> local DRAM (fp16); no collective ----
            T_local = dp.tile([WT * WIN, TW], FP16, name='T_local')
            for x in range(4):
                nc.sync.dma_start(
                    out=T_local[:, x * GLC:(x + 1) * GLC]
                        .rearrange("(w n) q -> n w q", w=WT),
                    in_=A_sb[x * WIN:(x + 1) * WIN, :]
                        .rearrange("n (w q) -> n w q", w=WT))
            nc.sync.dma_start(
                out=T_local[:, RB * LC:RB * LC + C]
                    .rearrange("(w n) c -> n w c", w=WT),
                in_=chiSb[:].rearrange("n (w c) -> n w c", w=WT))

            # ---- pass 2 (own windows only) ----
            # issue all gathers up front: they only depend on the T_local
            # write, so the DMA engines prefetch while sym/chi still run
            ags_all = []
            for blk in range(NBLK_OWN):
                ag = gp.tile([EBLK, TW], FP16, name='ag', tag='ag')
                nc.gpsimd.indirect_dma_start(
                    out=ag[:], out_offset=None, in_=T_local[:],
                    in_offset=bass.IndirectOffsetOnAxis(
                        ap=esrow_s[:, blk:blk + 1], axis=0))
                ags_all.append(ag)
            Anew = ctile('Anew', [EBLK, NWINC * GLC], FP16)
            for w in range(NWINC):
                ags = []
                P2s = []
                for bi in range(NBW):
                    blk = w * NBW + bi
                    ag = ags_all[blk]
                    ags.append(ag)
                    P2 = wp.tile([EBLK, LC], FP16, name='P2', tag='P2')
                    TT(out=P2[:].rearrange("p (l c) -> p l c", c=C),
                        in0=P[:, blk * LC:(blk + 1) * LC].rearrange("p (l c) -> p l c", c=C),
                        in1=ag[:, RB * LC:RB * LC + C].unsqueeze(1)
                            .broadcast_to([EBLK, NL, C]),
                        op=OP.mult)
                    P2s.append(P2)
                psB0 = pp.tile([EBLK, GLC], F32, name='psB0', tag='seg')
                for g in range(2):
                    for bi in range(NBW):
                        blk = w * NBW + bi
                        nc.tensor.matmul(
                            out=psB0[:, g * LC:(g + 1) * LC],
                            lhsT=lhsT1[g][:, blk * EBLK:(blk + 1) * EBLK],
                            rhs=P2s[bi][:],
                            start=(bi == 0), stop=(bi == NBW - 1),
                            skip_group_check=True)
                Ab0 = wp.tile([EBLK, GLC], FP16, name='Ab0', tag='Ab0')
                nc.any.tensor_copy(out=Ab0[:], in_=psB0[:])
                psAb = pp.tile([EBLK, GLC], F32, name='psAb', tag='acc')
                for gout in range(2):
                    for dd, (ls, lcnt) in enumerate(GRP_SLICES):
                        osl = slice(gout * LC + ls * C, gout * LC + (ls + lcnt) * C)
                        for gin in range(2):
                            wcol = ((gout * 4 + dd) * 2 + gin) * EBLK
                            csl = slice(gin * LC + ls * C, gin * LC + (ls + lcnt) * C)
                            nc.tensor.matmul(
                                out=psAb[:, osl],
                                lhsT=sb['wbd_radmp'][:, wcol:wcol + EBLK],
                                rhs=Ab0[:, csl],
                                start=(gin == 0), stop=(gin == 1),
                                skip_group_check=True)
                psAr = pp.tile([EBLK, GLC], F32, name='psAr', tag='ar')
                for b_ in range(RB):
                    g = b_ // 4; xq = b_ % 4
                    scol = xq * 2 + g          # T col-slice index for b_
                    for bi in range(NBW):
                        blk = w * NBW + bi
                        nc.tensor.matmul(
                            out=psAr[xq * WIN:(xq + 1) * WIN, g * LC:(g + 1) * LC],
                            lhsT=lhsT_ar[g][:, blk * EBLK + xq * WIN:
                                            blk * EBLK + (xq + 1) * WIN],
                            rhs=ags[bi][:, scol * LC:(scol + 1) * LC],
                            start=(bi == 0), stop=(bi == NBW - 1),
                            skip_group_check=True,
                            tile_position=(0, xq * WIN))
                comb = wp.tile([EBLK, GLC], F32, name='comb', tag='comb')
                nc.scalar.copy(out=comb[:], in_=psAb[:])
                TT(out=comb[:], in0=comb[:], in1=psAr[:], op=OP.add)
                TT(out=Anew[:, w * GLC:(w + 1) * GLC], in0=comb[:],
                   in1=memS[:, w * GLC:(w + 1) * GLC], op=OP.add)

            # ---- B1 symmetrize (own windows, f32) ----
            B1s = ctile('B1s', [EBLK, NWINC * 2 * NB * C])
            for gi in range(NWINC // GW):
                sym_group(Anew[:, gi * GW * GLC:(gi + 1) * GW * GLC],
                          B1s[:, gi * NVG * NB * C:(gi + 1) * NVG * NB * C],
                          pool_eng=False)

            # ---- output: [t2, (w,g)=16, 45] ----
            half = NWINC * 2 * NB * C
            nc.sync.dma_start(out=outB[:, 0:half], in_=B0s[:])
            nc.sync.dma_start(out=outB[:, half:2 * half], in_=B1s[:])

    nc.compile()
    return nc


_CACHE = {}


def kernel(**inputs) -> np.ndarray:
    return _kernel_impl(inputs)[0]


def _kernel_impl(inputs, trace=False):
    from concourse.bass_utils import run_bass_kernel_spmd

    packed, slot_of_node, nh = _host_prep(inputs)

    key = ('nc', nh)
    if key not in _CACHE:
        _CACHE[key] = _build_program(nh)
    nc = _CACHE[key]

    in_maps = [dict(p) for p in packed]

    res = run_bass_kernel_spmd(nc, in_maps, core_ids=list(range(N_CORES)),
                               trace=trace)

    feats_slots = np.zeros((NSLOT, RB, NB, C, 2), np.float32)
    for ci in range(N_CORES):
        arr = res.results[ci]['outB'].reshape(4, WIN, 2, NWINC, 2, NB, C)
        arr = np.transpose(arr, (3, 1, 4, 0, 5, 6, 2))
        feats_slots[ci * NWINC * WIN:(ci + 1) * NWINC * WIN] = \
            arr.reshape(NWINC * WIN, RB, NB, C, 2)
    return feats_slots[slot_of_node], res


if __name__ == '__main__':
    import pickle, os
    if os.path.exists('/tmp/inputs.pkl'):
        inputs = pickle.load(open('/tmp/inputs.pkl', 'rb'))
    else:
        import reference as Rf
        inputs = {k: np.asarray(v) for k, v in Rf.setup_inputs().items()}
        pickle.dump(inputs, open('/tmp/inputs.pkl', 'wb'))
    out = kernel(**inputs)
    print("kernel out", out.shape, out.dtype, float(np.abs(out).max()))
    if os.path.exists('/tmp/expected.npy'):
        exp = np.load('/tmp/expected.npy')
        err = np.abs(out - exp).max()
        print("max abs err vs expected:", err, "rel:", err / np.abs(exp).max())


# revision 59
# speedup vs baseline: 1.0787x; 1.0020x over previous
"""Trainium2 Bass kernel for nn_Cace_74569222193773 (CACE GNN message passing).

Strategy (8 NeuronCores, SPMD, one program shape + per-core data):
  * Host: drop edges with r >= cutoff (fcut = 0 there), assign nodes to 64
    edge-balanced global windows of <=32 nodes (8 "own" windows per core).
  * HALO REPLICATION instead of a collective: each core additionally
    recomputes pass-1 A for the sender nodes of its own edges that live on
    other cores.  Those halo nodes are repacked into private halo windows
    (<=32 nodes, <=256 in-edges each, edge-balanced); the core processes
    own + halo windows in pass 1, writes the node table T = [A row | chi]
    (fp16) to its own DRAM, and pass 2 gathers sender rows locally.
    No inter-core communication at all.
  * All node-feature tensors live in a "half" layout: partition p = x*32+n
    with x = (r or b) mod 4, plus a half index g = (r or b) // 4 in the
    free dimension, so every PE matmul output starts at a 32-aligned
    partition base.
  * Pass 1 (per core): edge geometry + bessel + cutoff + angular on
    DVE/Pool/ACT in edge-major layout [128 partitions = edges]; per-window
    segment-sum via PE matmuls (fp16 operands, fp32 PSUM) with
    lhsT = onehot32 (x) radf-half, rhs = P = ang (x) enc; radial transform
    via block-diag W (x) I32 fp16 matmuls; symmetrize + chi per window
    group (own windows in fp32 A, halo windows from the fp16 copy -- halo
    B0 only feeds chi).
  * Pass 2 (own windows only): indirect-DMA gather of T[send] (fp16 rows),
    A_ar via per-b matmuls (lhsT = onehot (x) fr slice), A_bchi via the
    pass-1 segment-sum machinery with rhs P * chi_send, mem via
    W_mem (x) I32; combine (fp32), symmetrize -> B1.

kernel() takes FULL unsharded inputs and returns the FULL [2000,8,5,9,2]
float32 output; all sharding happens inside.
"""
import heapq
from math import factorial

import numpy as np

# ---- static problem config (mirrors the reference) ----
MAX_L = 3; N_RBF = 8; RB = 8; K = 3
CUTOFF = 5.5
N_NODES = 2000
MP_NORM = 1.0 / np.sqrt(25.0)
C = K * K                      # 9
NB = 1 + (MAX_L + 1)           # 5

def _lxlylz(max_l):
    out = []
    for l in range(max_l + 1):
        for lx in range(l, -1, -1):
            for ly in range(l - lx, -1, -1):
                out.append((lx, ly, l - lx - ly))
    return out

L_LIST = _lxlylz(MAX_L); NL = len(L_LIST)                       # 20
LX = np.array([t[0] for t in L_LIST]); LY = np.array([t[1] for t in L_LIST])
LZ = np.array([t[2] for t in L_LIST]); DEGS = LX + LY + LZ
MULTI = np.array([factorial(int(d)) / (factorial(int(a)) * factorial(int(b)) * factorial(int(c)))
                  for a, b, c, d in zip(LX, LY, LZ, DEGS)], dtype=np.float32)
GRP_SLICES = []                 # (l_start, l_count) per degree; DEGS is sorted
for d in range(MAX_L + 1):
    idx = np.where(DEGS == d)[0]
    GRP_SLICES.append((int(idx[0]), int(len(idx))))

# ---- sharding geometry ----
N_CORES = 8
WIN = 32                        # nodes per window
NWINC = 8                       # own windows per core
NWIN = N_CORES * NWINC          # 64
NSLOT = NWIN * WIN              # 2048 own-node slots globally
EBLK = 128                      # edges per block (partition dim)
NBW = 2                         # blocks per window
NBLK_OWN = NWINC * NBW          # 16 own blocks per core
LC = NL * C                     # 180
GLC = 2 * LC                    # 360 = both halves
TW = RB * LC + WIN              # table row width 1472 (1440 A + 9 chi + pad)

_RSCL = np.sqrt(2.0 / CUTOFF)

F32_FIELDS = ['exyz_s', 'exyz_r', 'eemb_s', 'eemb_r', 'enloc',
              'iota32', 'multi_l', 'war_mp', 'nvec']
FP16_FIELDS = ['wbd_rad', 'wbd_radmp', 'wbd_mem', 'wbd_chi']


def _field_layout(nbt):
    """Column layout of the packed f32 / fp16 input tensors for nbt blocks."""
    fw = dict(exyz_s=3 * nbt, exyz_r=3 * nbt, eemb_s=3 * nbt, eemb_r=3 * nbt,
              enloc=nbt, iota32=WIN, multi_l=NL, war_mp=64, nvec=N_RBF,
              wbd_rad=2048, wbd_radmp=2048, wbd_mem=2048, wbd_chi=2 * NB * WIN)
    off = {}
    o = 0
    for f in F32_FIELDS:
        off[f] = o; o += fw[f]
    totf = o
    o = 0
    for f in FP16_FIELDS:
        off[f] = o; o += fw[f]
    return fw, off, totf, o


def _pack_windows(node_list, deg, nwin_cap):
    """Balanced assignment of node_list into windows (<=WIN nodes each,
    edge-load balanced).  Grows window count until max load <= NBW*EBLK.
    Returns (win_of, pos_of, n_windows)."""
    nodes = sorted(node_list, key=lambda n: -deg[n])
    nwin = max(1, (len(nodes) + WIN - 1) // WIN)
    while True:
        win_cnt = np.zeros(nwin, np.int64); win_load = np.zeros(nwin, np.int64)
        win_of = {}; pos_of = {}
        heap = [(0, w) for w in range(nwin)]
        heapq.heapify(heap)
        ok = True
        for nd in nodes:
            popped = []
            while True:
                load, w = heapq.heappop(heap)
                if win_cnt[w] < WIN:
                    break
                popped.append((load, w))
            for it in popped:
                heapq.heappush(heap, it)
            win_of[nd] = w; pos_of[nd] = int(win_cnt[w])
            win_cnt[w] += 1; win_load[w] += deg[nd]
            heapq.heappush(heap, (int(win_load[w]), w))
        if win_load.max(initial=0) <= NBW * EBLK:
            return win_of, pos_of, nwin
        nwin += 1
        if nwin > nwin_cap:
            raise RuntimeError("halo window packing overflow")


def _host_prep(inputs):
    pos = np.asarray(inputs['positions'], np.float32)
    shifts = np.asarray(inputs['shifts'], np.float32)
    W_embed = np.asarray(inputs['W_embed'], np.float32)
    species = np.asarray(inputs['species'])
    ei = np.asarray(inputs['edge_index'])
    send, recv = ei[0], ei[1]

    vec = (pos[recv] + shifts - pos[send]).astype(np.float64)
    r = np.sqrt((vec * vec).sum(-1))
    keep = np.where(r < CUTOFF)[0]
    deg = np.bincount(recv[keep], minlength=N_NODES)

    # balanced node->global-window assignment (own windows)
    order = np.argsort(-deg, kind='stable')
    win_cnt = np.zeros(NWIN, np.int64); win_load = np.zeros(NWIN, np.int64)
    win_of_node = np.zeros(N_NODES, np.int64); pos_in_win = np.zeros(N_NODES, np.int64)
    heap = [(0, w) for w in range(NWIN)]
    heapq.heapify(heap)
    for nd in order:
        popped = []
        while True:
            load, w = heapq.heappop(heap)
            if win_cnt[w] < WIN:
                break
            popped.append((load, w))
        for it in popped:
            heapq.heappush(heap, it)
        win_of_node[nd] = w; pos_in_win[nd] = win_cnt[w]
        win_cnt[w] += 1; win_load[w] += deg[nd]
        heapq.heappush(heap, (win_load[w], w))
    if win_load.max() > NBW * EBLK:
        raise RuntimeError(f"window overflow: {win_load.max()} > {NBW * EBLK}")

    slot_of_node = win_of_node * WIN + pos_in_win
    emb = W_embed[species]                       # [N, K]

    ks, kr = send[keep], recv[keep]
    in_edges = [[] for _ in range(N_NODES)]      # node -> kept edge ids
    for i, e in enumerate(keep):
        in_edges[kr[i]].append(e)

    # per-core halo structure
    core_halo = []
    nh_list = []
    for ci in range(N_CORES):
        own_w = set(range(ci * NWINC, (ci + 1) * NWINC))
        own_eids = []
        for w in sorted(own_w):
            for nd in np.where(win_of_node == w)[0]:
                own_eids.extend(in_edges[nd])
        senders = set(send[own_eids].tolist()) if own_eids else set()
        halo = [s for s in senders if win_of_node[s] not in own_w]
        hwin_of, hpos_of, nh = _pack_windows(halo, deg, 64)
        core_halo.append((own_w, hwin_of, hpos_of, nh))
        nh_list.append(nh)
    NH = max(nh_list)
    NH = ((NH + 3) // 4) * 4      # pad so WT = 8 + NH is a multiple of 4
    WT = NWINC + NH
    NBT = NBW * WT
    EPAD = NBT * EBLK

    cores = []
    for ci in range(N_CORES):
        own_w, hwin_of, hpos_of, nh = core_halo[ci]
        e_xyz_s = np.zeros((EPAD, 3), np.float32)
        e_xyz_r = np.zeros((EPAD, 3), np.float32)
        e_emb_s = np.zeros((EPAD, K), np.float32)
        e_emb_r = np.zeros((EPAD, K), np.float32)
        e_nloc = np.full((EPAD,), -1.0, np.float32)
        e_srow = np.zeros((EPAD,), np.int32)
        e_xyz_r[:, 0] = 1.0                      # pads: r = 1, finite math

        def srow_of(s):
            w = win_of_node[s]
            if w in own_w:
                return (w - ci * NWINC) * WIN + pos_in_win[s]
            return (NWINC + hwin_of[s]) * WIN + hpos_of[s]

        # local window wl in [0, WT): own first, then halo
        def fill_window(wl, node_ids, pos_of, need_srow):
            base = wl * NBW * EBLK
            eids = []
            for nd in node_ids:
                eids.extend(in_edges[nd])
            eids = np.array(eids, dtype=np.int64)
            cnt = len(eids)
            if cnt == 0:
                return
            if cnt > NBW * EBLK:
                raise RuntimeError("window edge overflow")
            sl = slice(base, base + cnt)
            e_xyz_s[sl] = pos[send[eids]]
            e_xyz_r[sl] = pos[recv[eids]] + shifts[eids]
            e_emb_s[sl] = emb[send[eids]]
            e_emb_r[sl] = emb[recv[eids]]
            e_nloc[sl] = np.array([pos_of[n] for n in recv[eids]], np.float32)
            if need_srow:
                e_srow[sl] = np.array([srow_of(s) for s in send[eids]], np.int32)

        for wl in range(NWINC):
            w = ci * NWINC + wl
            nds = np.where(win_of_node == w)[0]
            fill_window(wl, nds, {int(n): int(pos_in_win[n]) for n in nds}, True)
        halo_by_win = [[] for _ in range(nh)]
        for s, hw in hwin_of.items():
            halo_by_win[hw].append(s)
        for hw in range(nh):
            fill_window(NWINC + hw, halo_by_win[hw],
                        {int(n): int(hpos_of[n]) for n in halo_by_win[hw]}, False)

        def dev(x):
            if x.ndim == 1:
                return np.ascontiguousarray(x.reshape(NBT, EBLK).T)
            return np.ascontiguousarray(np.transpose(x.reshape(NBT, EBLK, -1), (1, 0, 2)))

        def axmajor(x3):
            d = dev(x3)                                  # [128, NBT, 3]
            return np.ascontiguousarray(np.transpose(d, (0, 2, 1)).reshape(EBLK, 3 * NBT))

        cores.append(dict(
            exyz_s=axmajor(e_xyz_s), exyz_r=axmajor(e_xyz_r),
            eemb_s=axmajor(e_emb_s), eemb_r=axmajor(e_emb_r),
            enloc=np.ascontiguousarray(dev(e_nloc)),
            esrow=np.ascontiguousarray(dev(e_srow)[:, :NBLK_OWN]),
        ))

    Wr = np.asarray(inputs['W_radial'], np.float32)   # [4(deg), 8(r), 8(b)]
    Wm = np.asarray(inputs['W_mem'], np.float32)
    Wc = np.asarray(inputs['W_chi'], np.float32)      # [8(b), 5(k)]
    Wa = np.asarray(inputs['W_ar'], np.float32)       # [8(r), 8(b)]
    I32 = np.eye(WIN, dtype=np.float32)

    def bd(W):
        cols = []
        for gout in range(2):
            for d in range(4):
                for gin in range(2):
                    cols.append(np.kron(W[d, gin * 4:gin * 4 + 4, gout * 4:gout * 4 + 4], I32))
        return np.concatenate(cols, axis=1)          # [128, 2048]

    wchi_cols = []
    for g in range(2):
        for k in range(NB):
            wchi_cols.append(np.kron(Wc[g * 4:g * 4 + 4, k:k + 1], I32))   # [128, 32]
    consts_f = dict(
        war_mp=np.tile((Wa * MP_NORM).reshape(1, 64), (EBLK, 1)),
        multi_l=np.tile(np.sqrt(MULTI).reshape(1, NL), (EBLK, 1)),
        iota32=np.tile(np.arange(WIN, dtype=np.float32).reshape(1, WIN), (EBLK, 1)),
        nvec=np.tile((np.arange(1, N_RBF + 1, dtype=np.float32) / CUTOFF).reshape(1, N_RBF),
                     (EBLK, 1)),
    )
    consts_b = dict(
        wbd_rad=bd(Wr),
        wbd_radmp=bd(Wr * MP_NORM),
        wbd_mem=bd(Wm),
        wbd_chi=np.concatenate(wchi_cols, axis=1),                   # [128, 320]
    )
    packed = []
    for ci in range(N_CORES):
        cols_f = [cores[ci][nm] for nm in
                  ['exyz_s', 'exyz_r', 'eemb_s', 'eemb_r', 'enloc']]
        cols_f += [consts_f[nm] for nm in ['iota32', 'multi_l', 'war_mp', 'nvec']]
        edf = np.ascontiguousarray(np.concatenate(cols_f, axis=1), np.float32)
        edb = np.ascontiguousarray(
            np.concatenate([consts_b[nm] for nm in FP16_FIELDS], axis=1)
        ).astype(np.float16)
        packed.append(dict(edf=edf, edb=edb, esrow=cores[ci]['esrow']))
    return packed, slot_of_node, NH


def _build_program(nh, debug=False):
    import concourse.bass as bass
    import concourse.mybir as mybir
    from concourse import bacc
    from concourse.tile import TileContext

    F32 = mybir.dt.float32
    FP16 = mybir.dt.float16
    AF = mybir.ActivationFunctionType
    OP = mybir.AluOpType

    WT = NWINC + nh
    NBT = NBW * WT
    FIELD_W, FIELD_OFF, TOTW_F, TOTW_B = _field_layout(NBT)

    nc = bacc.Bacc("TRN2", target_bir_lowering=False, debug=False,
                   num_devices=N_CORES)

    edf_d = nc.dram_tensor('edf', [EBLK, TOTW_F], F32, kind="ExternalInput")
    edb_d = nc.dram_tensor('edb', [EBLK, TOTW_B], FP16, kind="ExternalInput")
    esrow_d = nc.dram_tensor('esrow', [EBLK, NBLK_OWN], mybir.dt.int32,
                             kind="ExternalInput")
    outB = nc.dram_tensor('outB', [EBLK, 2 * NWINC * 2 * NB * C], F32,
                          kind="ExternalOutput")

    with TileContext(nc) as tc:
        with (tc.tile_pool(name="const", bufs=1) as cp,
              tc.tile_pool(name="work", bufs=2) as wp,
              tc.tile_pool(name="gat", bufs=9) as gp,
              tc.tile_pool(name="psum", bufs=2, space="PSUM") as pp,
              tc.tile_pool(name="dram", bufs=1, space="DRAM") as dp):

            bigf = cp.tile([EBLK, TOTW_F], F32, name='bigf', tag='bigf')
            nc.sync.dma_start(out=bigf[:], in_=edf_d[:])
            bigb = cp.tile([EBLK, TOTW_B], FP16, name='bigb', tag='bigb')
            nc.sync.dma_start(out=bigb[:], in_=edb_d[:])
            esrow_s = cp.tile([EBLK, NBLK_OWN], mybir.dt.int32,
                              name='esrow_s', tag='esrow_s')
            nc.sync.dma_start(out=esrow_s[:], in_=esrow_d[:])

            class _S:
                def __init__(self, tile):
                    self.tile = tile
                def __getitem__(self, nm):
                    off = FIELD_OFF[nm]
                    return self.tile[:, off:off + FIELD_W[nm]]
            s = _S(bigf)
            sb = _S(bigb)

            def ctile(tag, shape, dtype=F32):
                return cp.tile(shape, dtype, name=tag, tag=tag)

            TT = nc.vector.tensor_tensor
            TTP = nc.gpsimd.tensor_tensor
            TS = nc.vector.tensor_scalar

            # ---- geometry, edge-major [128, a*NBT+blk] ----
            vd = ctile('vd', [EBLK, 3 * NBT])
            TT(out=vd[:], in0=s['exyz_r'][:], in1=s['exyz_s'][:], op=OP.subtract)
            sq = ctile('sq', [EBLK, 3 * NBT])
            TT(out=sq[:], in0=vd[:], in1=vd[:], op=OP.mult)
            r2 = ctile('r2', [EBLK, NBT])
            TT(out=r2[:], in0=sq[:, 0:NBT], in1=sq[:, NBT:2 * NBT], op=OP.add)
            TT(out=r2[:], in0=r2[:], in1=sq[:, 2 * NBT:3 * NBT], op=OP.add)
            rr = ctile('rr', [EBLK, NBT])
            nc.scalar.activation(out=rr[:], in_=r2[:], func=AF.Sqrt)
            # reference adds 1e-9 to r before dividing; r >= cutoff-filtered
            # lengths here (>0.1), so the epsilon is numerically invisible
            rinv = ctile('rinv', [EBLK, NBT])
            nc.vector.reciprocal(out=rinv[:], in_=rr[:])
            uv = ctile('uv', [EBLK, 3 * NBT])
            TT(out=uv[:].rearrange("p (a b) -> p a b", a=3),
               in0=vd[:].rearrange("p (a b) -> p a b", a=3),
               in1=rinv[:].unsqueeze(1).broadcast_to([EBLK, 3, NBT]), op=OP.mult)

            # bessel: rad[r, blk] = sin((n+1) * pi/c * r) * (sqrt(2/c) * rinv)
            rscl = ctile('rscl', [EBLK, NBT])
            TS(out=rscl[:], in0=rinv[:], scalar1=float(-_RSCL), scalar2=None, op0=OP.mult)
            radp = ctile('radp', [EBLK, N_RBF * NBT])
            marg = ctile('marg', [EBLK, N_RBF * NBT])
            TT(out=marg[:].rearrange("p (r b) -> p r b", r=N_RBF),
               in0=rr[:].unsqueeze(1).broadcast_to([EBLK, N_RBF, NBT]),
               in1=s['nvec'][:].unsqueeze(2).broadcast_to([EBLK, N_RBF, NBT]),
               op=OP.mult)
            mtmp = ctile('mtmp', [EBLK, N_RBF * NBT])
            TS(out=mtmp[:], in0=marg[:], scalar1=4.0, scalar2=4.0,
               op0=OP.is_ge, op1=OP.mult)
            TT(out=marg[:], in0=marg[:], in1=mtmp[:], op=OP.subtract)
            TS(out=mtmp[:], in0=marg[:], scalar1=2.0, scalar2=2.0,
               op0=OP.is_ge, op1=OP.mult)
            TT(out=marg[:], in0=marg[:], in1=mtmp[:], op=OP.subtract)
            biaspi = ctile('biaspi', [EBLK, 1])
            nc.vector.memset(biaspi[:], float(-np.pi))
            nc.scalar.activation(out=radp[:], in_=marg[:], func=AF.Sin,
                                 scale=float(np.pi), bias=biaspi[:])


            # poly cutoff (p=6); host guarantees u<1 (Pool engine)
            uu = ctile('uu', [EBLK, NBT])
            nc.gpsimd.tensor_scalar(out=uu[:], in0=rr[:], scalar1=float(1.0 / CUTOFF),
                                    scalar2=None, op0=OP.mult)
            u3 = ctile('u3', [EBLK, NBT])
            TTP(out=u3[:], in0=uu[:], in1=uu[:], op=OP.mult)
            TTP(out=u3[:], in0=u3[:], in1=uu[:], op=OP.mult)
            u6 = ctile('u6', [EBLK, NBT]); TTP(out=u6[:], in0=u3[:], in1=u3[:], op=OP.mult)
            u7 = ctile('u7', [EBLK, NBT]); TTP(out=u7[:], in0=u6[:], in1=uu[:], op=OP.mult)
            u8 = ctile('u8', [EBLK, NBT]); TTP(out=u8[:], in0=u7[:], in1=uu[:], op=OP.mult)
            fc = ctile('fc', [EBLK, NBT])
            nc.gpsimd.tensor_scalar(out=fc[:], in0=u6[:], scalar1=-28.0, scalar2=1.0,
                                    op0=OP.mult, op1=OP.add)
            t7 = ctile('t7', [EBLK, NBT])
            nc.gpsimd.tensor_scalar(out=t7[:], in0=u7[:], scalar1=48.0, scalar2=None,
                                    op0=OP.mult)
            TTP(out=fc[:], in0=fc[:], in1=t7[:], op=OP.add)
            nc.gpsimd.tensor_scalar(out=t7[:], in0=u8[:], scalar1=-21.0, scalar2=None,
                                    op0=OP.mult)
            TTP(out=fc[:], in0=fc[:], in1=t7[:], op=OP.add)

            # combined per-edge scale: rscl * fcut in one small op, so the
            # full-width radial basis gets a single multiply after the Sin
            scc = ctile('scc', [EBLK, NBT])
            TTP(out=scc[:], in0=rscl[:], in1=fc[:], op=OP.mult)
            radf = ctile('radf', [EBLK, N_RBF * NBT])
            TT(out=radf[:].rearrange("p (r b) -> p r b", r=N_RBF),
               in0=radp[:].rearrange("p (r b) -> p r b", r=N_RBF),
               in1=scc[:].unsqueeze(1).broadcast_to([EBLK, N_RBF, NBT]), op=OP.mult)

            # onehot [blk, n32] (DVE: Pool lacks is_equal)
            onehot = ctile('onehot', [EBLK, NBT * WIN])
            TT(out=onehot[:].rearrange("p (b n) -> p b n", b=NBT),
               in0=s['enloc'][:].unsqueeze(2).broadcast_to([EBLK, NBT, WIN]),
               in1=s['iota32'][:].unsqueeze(1).broadcast_to([EBLK, NBT, WIN]),
               op=OP.is_equal)

            # enc [blk, ks, kr] (Pool)
            enc = ctile('enc', [EBLK, NBT * C])
            TTP(out=enc[:].rearrange("p (b i j) -> p b i j", i=K, j=K),
                in0=s['eemb_s'][:].rearrange("p (k b) -> p b k", k=K).unsqueeze(3)
                    .broadcast_to([EBLK, NBT, K, K]),
                in1=s['eemb_r'][:].rearrange("p (k b) -> p b k", k=K).unsqueeze(2)
                    .broadcast_to([EBLK, NBT, K, K]),
                op=OP.mult)

            # angular monomials [l, blk]
            ones = ctile('ones', [EBLK, NBT])
            nc.vector.memset(ones[:], 1.0)
            x2 = ctile('x2', [EBLK, 3 * NBT])
            TT(out=x2[:], in0=uv[:], in1=uv[:], op=OP.mult)
            x3 = ctile('x3', [EBLK, 3 * NBT])
            TT(out=x3[:], in0=x2[:], in1=uv[:], op=OP.mult)

            def pow_plane(axis, p_):
                if p_ == 1:
                    return uv[:, axis * NBT:(axis + 1) * NBT]
                if p_ == 2:
                    return x2[:, axis * NBT:(axis + 1) * NBT]
                return x3[:, axis * NBT:(axis + 1) * NBT]

            ang = ctile('ang', [EBLK, NL * NBT])
            for l in range(NL):
                facs = [pow_plane(a, pw) for a, pw in enumerate((LX[l], LY[l], LZ[l])) if pw > 0]
                dst = ang[:, l * NBT:(l + 1) * NBT]
                if len(facs) == 0:
                    nc.scalar.copy(out=dst, in_=ones[:])
                elif len(facs) == 1:
                    nc.scalar.copy(out=dst, in_=facs[0])
                elif len(facs) == 2:
                    TT(out=dst, in0=facs[0], in1=facs[1], op=OP.mult)
                else:
                    TT(out=dst, in0=facs[0], in1=facs[1], op=OP.mult)
                    TT(out=dst, in0=dst, in1=facs[2], op=OP.mult)

            # fold sqrt(MULTI_l) into ang: every downstream tensor (A, table,
            # mem, A_ar, A_bchi) is then consistently per-l scaled, so the
            # symmetrize b2 sum needs no MULTI weighting (b1 has MULTI_0 = 1)
            TT(out=ang[:].rearrange("p (l b) -> p l b", l=NL),
               in0=ang[:].rearrange("p (l b) -> p l b", l=NL),
               in1=s['multi_l'][:].unsqueeze(2).broadcast_to([EBLK, NL, NBT]),
               op=OP.mult)

            # P = ang (x) enc : [blk, l, c] in fp16 (split DVE / Pool by half)
            P = ctile('P', [EBLK, NBT * LC], FP16)
            HB = NBT // 8
            for half, eng in ((0, TT), (1, TTP)):
                blo = half * HB; bcnt = (NBT - HB) if half else HB
                eng(out=P[:, blo * LC:(blo + bcnt) * LC]
                        .rearrange("p (b l c) -> p b l c", l=NL, c=C),
                    in0=ang[:].rearrange("p (l b) -> p b l", l=NL)[:, blo:blo + bcnt]
                        .unsqueeze(3).broadcast_to([EBLK, bcnt, NL, C]),
                    in1=enc[:].rearrange("p (b c) -> p b c", c=C)[:, blo:blo + bcnt]
                        .unsqueeze(2).broadcast_to([EBLK, bcnt, NL, C]),
                    op=OP.mult)

            # lhsT1_g = radf-half (x) onehot : [blk, r4, n32] (fp16; DVE/Pool split)
            lhsT1 = []
            for g in range(2):
                lt = ctile(f'lhsT1_{g}', [EBLK, NBT * EBLK], FP16)
                eng = TT
                eng(out=lt[:].rearrange("p (b r n) -> p b r n", r=4, n=WIN),
                    in0=radf[:].rearrange("p (r b) -> p b r", r=N_RBF)
                        [:, :, g * 4:(g + 1) * 4].unsqueeze(3)
                        .broadcast_to([EBLK, NBT, 4, WIN]),
                    in1=onehot[:].rearrange("p (b n) -> p b n", b=NBT).unsqueeze(2)
                        .broadcast_to([EBLK, NBT, 4, WIN]),
                    op=OP.mult)
                lhsT1.append(lt)

            # fr = (radf @ W_ar) * MP_NORM : own blocks only [blk, b8] (Pool)
            frA = ctile('frA', [EBLK, NBLK_OWN * N_RBF])
            frB = ctile('frB', [EBLK, NBLK_OWN * N_RBF])
            frt = ctile('frt', [EBLK, NBLK_OWN * N_RBF])
            for r_ in range(N_RBF):
                radv = radf[:, r_ * NBT:r_ * NBT + NBLK_OWN].unsqueeze(2) \
                    .broadcast_to([EBLK, NBLK_OWN, N_RBF])
                warv = s['war_mp'][:, r_ * N_RBF:(r_ + 1) * N_RBF].unsqueeze(1) \
                    .broadcast_to([EBLK, NBLK_OWN, N_RBF])
                if r_ == 0:
                    TTP(out=frA[:].rearrange("p (b k) -> p b k", k=N_RBF),
                        in0=radv, in1=warv, op=OP.mult)
                else:
                    TTP(out=frt[:].rearrange("p (b k) -> p b k", k=N_RBF),
                        in0=radv, in1=warv, op=OP.mult)
                    src, dst = (frA, frB) if r_ % 2 == 1 else (frB, frA)
                    TTP(out=dst[:], in0=src[:], in1=frt[:], op=OP.add)
            fr = frB

            # lhsT_ar_g = fr-half (x) onehot : own blocks only (fp16)
            lhsT_ar = []
            for g in range(2):
                lt = ctile(f'lhsT_ar_{g}', [EBLK, NBLK_OWN * EBLK], FP16)
                TTP(out=lt[:].rearrange("p (w b n) -> p w b n", b=4, n=WIN),
                   in0=fr[:].rearrange("p (w k) -> p w k", k=N_RBF)
                       [:, :, g * 4:(g + 1) * 4].unsqueeze(3)
                       .broadcast_to([EBLK, NBLK_OWN, 4, WIN]),
                   in1=onehot[:].rearrange("p (b n) -> p b n", b=NBT)
                       [:, :NBLK_OWN].unsqueeze(2)
                       .broadcast_to([EBLK, NBLK_OWN, 4, WIN]),
                   op=OP.mult)
                lhsT_ar.append(lt)

            # ---- pass 1: per-window segment sum + radial transform,
            # with symmetrize/chi groups interleaved every GW windows so the
            # DVE/Pool sym work overlaps the PE window matmuls ----
            GW = 4                       # windows per group
            NG = WT // GW                # WT is a multiple of 4 (NH padded)
            NVG = GW * 2
            A_sb = ctile('A_sb', [EBLK, WT * GLC], FP16)
            B0s = ctile('B0s', [EBLK, NWINC * 2 * NB * C])   # own B0 (output, f32)
            chiS = ctile('chiS', [WIN, WT * C])
            chiSb = ctile('chiSb', [WIN, WT * C], FP16)
            memS = ctile('memS', [EBLK, NWINC * GLC])        # parked psMem (f32)

            def sym_group(Ain, Bout, pool_eng):
                # Ain fp16 [p, (8 pseudo-windows, lc)] view; Bout f32 [p, (8, NB*C)]
                tte = TTP if pool_eng else TT
                red = nc.vector.tensor_reduce
                sqs = wp.tile([EBLK, NVG * LC], FP16, name='sqs', tag='sqs')
                tte(out=sqs[:], in0=Ain, in1=Ain, op=OP.mult)
                nc.scalar.copy(
                    out=Bout.rearrange("p (v q) -> p v q", q=NB * C)[:, :, 0:C],
                    in_=Ain.rearrange("p (v q) -> p v q", q=LC)[:, :, 0:C])
                for dd, (ls, lcnt) in enumerate(GRP_SLICES):
                    red(
                        Bout.rearrange("p (v q) -> p v q", q=NB * C)
                            [:, :, (1 + dd) * C:(2 + dd) * C].unsqueeze(3),
                        sqs[:].rearrange("p (v l c) -> p v c l", l=NL, c=C)
                            [:, :, :, ls:ls + lcnt],
                        mybir.AxisListType.X, OP.add)

            def chi_group(gi, Bg):
                Bgb = wp.tile([EBLK, NVG * NB * C], FP16, name='Bgb', tag='Bgb')
                nc.any.tensor_copy(out=Bgb[:], in_=Bg)
                for wl in range(GW):
                    w = gi * GW + wl
                    psC = pp.tile([WIN, C], F32, name='psC', tag='seg')
                    first = True
                    for g in range(2):
                        for k in range(NB):
                            nc.tensor.matmul(
                                out=psC[:],
                                lhsT=sb['wbd_chi'][:, (g * NB + k) * WIN:
                                                   (g * NB + k + 1) * WIN],
                                rhs=Bgb[:, (wl * 2 + g) * NB * C + k * C:
                                           (wl * 2 + g) * NB * C + (k + 1) * C],
                                start=first, stop=(g == 1 and k == NB - 1),
                                skip_group_check=True)
                            first = False
                    nc.any.tensor_copy(out=chiS[:, w * C:(w + 1) * C], in_=psC[:])

            def mem_window(w):
                psMem = pp.tile([EBLK, GLC], F32, name='psMem', tag='memt')
                for gout in range(2):
                    for dd, (ls, lcnt) in enumerate(GRP_SLICES):
                        osl = slice(gout * LC + ls * C, gout * LC + (ls + lcnt) * C)
                        for gin in range(2):
                            wcol = ((gout * 4 + dd) * 2 + gin) * EBLK
                            csl = slice(w * GLC + gin * LC + ls * C,
                                        w * GLC + gin * LC + (ls + lcnt) * C)
                            nc.tensor.matmul(
                                out=psMem[:, osl],
                                lhsT=sb['wbd_mem'][:, wcol:wcol + EBLK],
                                rhs=A_sb[:, csl],
                                start=(gin == 0), stop=(gin == 1),
                                skip_group_check=True)
                nc.any.tensor_copy(out=memS[:, w * GLC:(w + 1) * GLC], in_=psMem[:])

            for w in range(WT):
                psA0 = pp.tile([EBLK, GLC], F32, name='psA0', tag='seg')
                for g in range(2):
                    for bi in range(NBW):
                        blk = w * NBW + bi
                        nc.tensor.matmul(
                            out=psA0[:, g * LC:(g + 1) * LC],
                            lhsT=lhsT1[g][:, blk * EBLK:(blk + 1) * EBLK],
                            rhs=P[:, blk * LC:(blk + 1) * LC],
                            start=(bi == 0), stop=(bi == NBW - 1),
                            skip_group_check=True)
                A0s = wp.tile([EBLK, GLC], FP16, name='A0s', tag='A0s')
                nc.any.tensor_copy(out=A0s[:], in_=psA0[:])
                psA = pp.tile([EBLK, GLC], F32, name='psA', tag='acc')
                for gout in range(2):
                    for dd, (ls, lcnt) in enumerate(GRP_SLICES):
                        osl = slice(gout * LC + ls * C, gout * LC + (ls + lcnt) * C)
                        for gin in range(2):
                            wcol = ((gout * 4 + dd) * 2 + gin) * EBLK
                            csl = slice(gin * LC + ls * C, gin * LC + (ls + lcnt) * C)
                            nc.tensor.matmul(
                                out=psA[:, osl],
                                lhsT=sb['wbd_rad'][:, wcol:wcol + EBLK],
                                rhs=A0s[:, csl],
                                start=(gin == 0), stop=(gin == 1),
                                skip_group_check=True)
                nc.any.tensor_copy(out=A_sb[:, w * GLC:(w + 1) * GLC], in_=psA[:])

            # own A rows are complete: PE parks all psMem results while the
            # DVE/Pool sym groups below run concurrently
            for wm in range(NWINC):
                mem_window(wm)

            for gi in range(NG):
                own_grp = gi < NWINC // GW
                if own_grp:
                    Bg = B0s[:, gi * NVG * NB * C:(gi + 1) * NVG * NB * C]
                else:
                    Bgt = wp.tile([EBLK, NVG * NB * C], F32, name='Bgt', tag='Bgt')
                    Bg = Bgt[:]
                sym_group(A_sb[:, gi * GW * GLC:(gi + 1) * GW * GLC], Bg,
                          pool_eng=(gi % 3 == 2))
                chi_group(gi, Bg)
            nc.any.tensor_copy(out=chiSb[:], in_=chiS[:])

            # ---- node table -> local DRAM (fp16); no collective ----
            T_local = dp.tile([WT * WIN, TW], FP16, name='T_local')
            for x in range(4):
                nc.sync.dma_start(
                    out=T_local[:, x * GLC:(x + 1) * GLC]
                        .rearrange("(w n) q -> n w q", w=WT),
                    in_=A_sb[x * WIN:(x + 1) * WIN, :]
                        .rearrange("n (w q) -> n w q", w=WT))
            nc.sync.dma_start(
                out=T_local[:, RB * LC:RB * LC + C]
                    .rearrange("(w n) c -> n w c", w=WT),
                in_=chiSb[:].rearrange("n (w c) -> n w c", w=WT))

            # ---- pass 2 (own windows only) ----
            # issue all gathers up front: they only depend on the T_local
            # write, so the DMA engines prefetch while sym/chi still run
            ags_all = []
            for blk in range(NBLK_OWN):
                ag = gp.tile([EBLK, TW], FP16, name='ag', tag='ag')
                nc.gpsimd.indirect_dma_start(
                    out=ag[:], out_offset=None, in_=T_local[:],
                    in_offset=bass.IndirectOffsetOnAxis(
                        ap=esrow_s[:, blk:blk + 1], axis=0))
                ags_all.append(ag)
            Anew = ctile('Anew', [EBLK, NWINC * GLC], FP16)
            for w in range(NWINC):
                ags = []
                P2s = []
                for bi in range(NBW):
                    blk = w * NBW + bi
                    ag = ags_all[blk]
                    ags.append(ag)
                    P2 = wp.tile([EBLK, LC], FP16, name='P2', tag='P2')
                    TT(out=P2[:].rearrange("p (l c) -> p l c", c=C),
                        in0=P[:, blk * LC:(blk + 1) * LC].rearrange("p (l c) -> p l c", c=C),
                        in1=ag[:, RB * LC:RB * LC + C].unsqueeze(1)
                            .broadcast_to([EBLK, NL, C]),
                        op=OP.mult)
                    P2s.append(P2)
                psB0 = pp.tile([EBLK, GLC], F32, name='psB0', tag='seg')
                for g in range(2):
                    for bi in range(NBW):
                        blk = w * NBW + bi
                        nc.tensor.matmul(
                            out=psB0[:, g * LC:(g + 1) * LC],
                            lhsT=lhsT1[g][:, blk * EBLK:(blk + 1) * EBLK],
                            rhs=P2s[bi][:],
                            start=(bi == 0), stop=(bi == NBW - 1),
                            skip_group_check=True)
                Ab0 = wp.tile([EBLK, GLC], FP16, name='Ab0', tag='Ab0')
                nc.any.tensor_copy(out=Ab0[:], in_=psB0[:])
                psAb = pp.tile([EBLK, GLC], F32, name='psAb', tag='acc')
                for gout in range(2):
                    for dd, (ls, lcnt) in enumerate(GRP_SLICES):
                        osl = slice(gout * LC + ls * C, gout * LC + (ls + lcnt) * C)
                        for gin in range(2):
                            wcol = ((gout * 4 + dd) * 2 + gin) * EBLK
                            csl = slice(gin * LC + ls * C, gin * LC + (ls + lcnt) * C)
                            nc.tensor.matmul(
                                out=psAb[:, osl],
                                lhsT=sb['wbd_radmp'][:, wcol:wcol + EBLK],
                                rhs=Ab0[:, csl],
                                start=(gin == 0), stop=(gin == 1),
                                skip_group_check=True)
                psAr = pp.tile([EBLK, GLC], F32, name='psAr', tag='ar')
                for b_ in range(RB):
                    g = b_ // 4; xq = b_ % 4
                    scol = xq * 2 + g          # T col-slice index for b_
                    for bi in range(NBW):
                        blk = w * NBW + bi
                        nc.tensor.matmul(
                            out=psAr[xq * WIN:(xq + 1) * WIN, g * LC:(g + 1) * LC],
                            lhsT=lhsT_ar[g][:, blk * EBLK + xq * WIN:
                                            blk * EBLK + (xq + 1) * WIN],
                            rhs=ags[bi][:, scol * LC:(scol + 1) * LC],
                            start=(bi == 0), stop=(bi == NBW - 1),
                            skip_group_check=True,
                            tile_position=(0, xq * WIN))
                comb = wp.tile([EBLK, GLC], F32, name='comb', tag='comb')
                nc.scalar.copy(out=comb[:], in_=psAb[:])
                TT(out=comb[:], in0=comb[:], in1=psAr[:], op=OP.add)
                TT(out=Anew[:, w * GLC:(w + 1) * GLC], in0=comb[:],
                   in1=memS[:, w * GLC:(w + 1) * GLC], op=OP.add)

            # ---- B1 symmetrize (own windows, f32) ----
            B1s = ctile('B1s', [EBLK, NWINC * 2 * NB * C])
            for gi in range(NWINC // GW):
                sym_group(Anew[:, gi * GW * GLC:(gi + 1) * GW * GLC],
                          B1s[:, gi * NVG * NB * C:(gi + 1) * NVG * NB * C],
                          pool_eng=False)

            # ---- output: [t2, (w,g)=16, 45] ----
            half = NWINC * 2 * NB * C
            nc.sync.dma_start(out=outB[:, 0:half], in_=B0s[:])
            nc.sync.dma_start(out=outB[:, half:2 * half], in_=B1s[:])

    nc.compile()
    return nc


_CACHE = {}


def kernel(**inputs) -> np.ndarray:
    return _kernel_impl(inputs)[0]


def _kernel_impl(inputs, trace=False):
    from concourse.bass_utils import run_bass_kernel_spmd

    packed, slot_of_node, nh = _host_prep(inputs)

    key = ('nc', nh)
    if key not in _CACHE:
        _CACHE[key] = _build_program(nh)
    nc = _CACHE[key]

    in_maps = [dict(p) for p in packed]

    res = run_bass_kernel_spmd(nc, in_maps, core_ids=list(range(N_CORES)),
                               trace=trace)

    feats_slots = np.zeros((NSLOT, RB, NB, C, 2), np.float32)
    for ci in range(N_CORES):
        arr = res.results[ci]['outB'].reshape(4, WIN, 2, NWINC, 2, NB, C)
        arr = np.transpose(arr, (3, 1, 4, 0, 5, 6, 2))
        feats_slots[ci * NWINC * WIN:(ci + 1) * NWINC * WIN] = \
            arr.reshape(NWINC * WIN, RB, NB, C, 2)
    return feats_slots[slot_of_node], res


if __name__ == '__main__':
    import pickle, os
    if os.path.exists('/tmp/inputs.pkl'):
        inputs = pickle.load(open('/tmp/inputs.pkl', 'rb'))
    else:
        import reference as Rf
        inputs = {k: np.asarray(v) for k, v in Rf.setup_inputs().items()}
        pickle.dump(inputs, open('/tmp/inputs.pkl', 'wb'))
    out = kernel(**inputs)
    print("kernel out", out.shape, out.dtype, float(np.abs(out).max()))
    if os.path.exists('/tmp/expected.npy'):
        exp = np.load('/tmp/expected.npy')
        err = np.abs(out - exp).max()
        print("max abs err vs expected:", err, "rel:", err / np.abs(exp).max())
